# revision 2
# baseline (speedup 1.0000x reference)
"""Trainium2 Bass kernel v2 for nn_MultiModalFusion (moe_routing).

Strategy (v2, derived from the TimelineSim cost model):
- Data parallel over 8 cores. Host sorts samples by expert into 64 slots of
  exactly 256 (zero padding waste); at most 3 slots mix two experts — those go
  to chunk slot 7 of cores 0..2, where the program computes both experts and
  mask-blends.
- QKV projection in fp8e4m3 with host-side hi/lo error compensation
  (x = x_hi + x_lo, W = W_hi + W_lo; the three significant cross products are
  packed into DoubleRow matmuls at 0.5 cyc/row -> 3C per (token, m-tile)
  instead of fp32r's 4C, with ~bf16-level accuracy).
- All other matmul moving operands bf16 (1 cyc/row at any width).
- Attention probs via the identity o_i = v0 + p_i1*(v1-v0) + p_i2*(v2-v0):
  only 2 of 3 probability broadcasts per query (24 instead of 36 broadcast
  matmul columns), probs in bf16.
- LayerNorm: mean folded into an extra fus2 output row (host-precomputed
  column-mean weight vector), rstd = exp(-0.5*ln(var+eps)) on the scalar
  engine — ln/exp/identity/relu/square share one activation table set, so no
  LoadActFuncSet thrash. gamma/beta folded into expert weights on host.
- PSUM pair tiles (2 x 256 f32 = one 2KB bank) let one Activation instruction
  evacuate two matmul outputs.
- DVE kept in 2x bf16 mode where possible; part of the prob*value work is
  offloaded to the idle GPSIMD (Pool) engine.
"""

import numpy as np
import ml_dtypes

import concourse.bass as bass
import concourse.mybir as mybir
import concourse.tile as tile
from concourse import bacc
from concourse.bass_utils import run_bass_kernel_spmd

E = 512
H = 256
NH = 8
HD = 64
NE = 4
B = 16384
NCORES = 8
C = 256              # chunk columns
NCH = 8              # chunks per core
R = NE * 512         # 2048 columns per core

LAST_RESULTS = None
LAST_NC = None

F32 = mybir.dt.float32
F32R = mybir.dt.float32r
BF16 = mybir.dt.bfloat16
FP8 = mybir.dt.float8e4
AF = mybir.ActivationFunctionType
ALU = mybir.AluOpType
MM = mybir.MatmulPerfMode

FP8NP = ml_dtypes.float8_e4m3
BF16NP = ml_dtypes.bfloat16

_NC_CACHE = {}


def _build_program(zero_bias, debug=False):
    key = (bool(zero_bias), bool(debug))
    if key in _NC_CACHE:
        return _NC_CACHE[key]
    nc = bacc.Bacc("TRN2")

    # ---------------- DRAM I/O ----------------
    # x hi/lo fp8: [ch, 128, t, p, plane(0=lo,1=hi), C]
    xhl = nc.dram_tensor("xhl", [NCH, 128, 3, 4, 2, C], FP8,
                         kind="ExternalInput")
    # QKV weights fp8: hi-hi k-pairs and (hi,lo) cross pairs
    whh = nc.dram_tensor("whh", [128, 2, 2, 1536], FP8, kind="ExternalInput")
    wcr = nc.dram_tensor("wcr", [128, 4, 2, 1536], FP8, kind="ExternalInput")
    bqkv = nc.dram_tensor("bqkv", [128, 12], F32, kind="ExternalInput")
    w1o = nc.dram_tensor("w1o", [128, 12, 256], BF16, kind="ExternalInput")
    beff = nc.dram_tensor("beff", [128, 2], F32, kind="ExternalInput")
    w2 = nc.dram_tensor("w2", [128, 2, 512], BF16, kind="ExternalInput")
    wmu = nc.dram_tensor("wmu", [128, 2], BF16, kind="ExternalInput")
    b2 = nc.dram_tensor("b2", [128, 4], F32, kind="ExternalInput")
    # per-chunk expert weights, host-selected: A for all chunks, B for ch 7
    waffA = nc.dram_tensor("waffA", [NCH, 128, 4, 512], BF16,
                           kind="ExternalInput")
    waffB = nc.dram_tensor("waffB", [128, 4, 512], BF16,
                           kind="ExternalInput")
    baffA = nc.dram_tensor("baffA", [NCH, 128, 4], F32,
                           kind="ExternalInput")
    baffB = nc.dram_tensor("baffB", [128, 4], F32, kind="ExternalInput")
    maskex = nc.dram_tensor("maskex", [128, C], BF16, kind="ExternalInput")
    meanb2 = nc.dram_tensor("meanb2", [1, 1], F32, kind="ExternalInput")
    sel = nc.dram_tensor("sel", [128, 4, 8], BF16, kind="ExternalInput")
    exps = nc.dram_tensor("exps", [8, 4, 128], BF16, kind="ExternalInput")
    ones512 = nc.dram_tensor("ones512", [128, 1], BF16, kind="ExternalInput")
    onesk1 = nc.dram_tensor("onesk1", [1, 128], F32R, kind="ExternalInput")
    outT = nc.dram_tensor("outT", [NCH, 128, 4, C], BF16,
                          kind="ExternalOutput")
    if debug:
        dbg_q = nc.dram_tensor("dbg_q", [128, 3, 4, C], F32,
                               kind="ExternalOutput")
        dbg_k = nc.dram_tensor("dbg_k", [128, 3, 4, C], F32,
                               kind="ExternalOutput")
        dbg_v = nc.dram_tensor("dbg_v", [128, 3, 4, C], F32,
                               kind="ExternalOutput")
        dbg_e = nc.dram_tensor("dbg_e", [8, 3, 3, C], F32,
                               kind="ExternalOutput")
        dbg_d = nc.dram_tensor("dbg_d", [8, 3, 2, C], F32,
                               kind="ExternalOutput")
        dbg_o = nc.dram_tensor("dbg_o", [128, 12, C], F32,
                               kind="ExternalOutput")
        dbg_h = nc.dram_tensor("dbg_h", [128, 2, C], F32,
                               kind="ExternalOutput")
        dbg_y = nc.dram_tensor("dbg_y", [128, 4, C], F32,
                               kind="ExternalOutput")
        dbg_mu = nc.dram_tensor("dbg_mu", [1, C], F32,
                                kind="ExternalOutput")
        dbg_var = nc.dram_tensor("dbg_var", [1, C], F32,
                                 kind="ExternalOutput")
        dbg_rstd = nc.dram_tensor("dbg_rstd", [1, C], F32,
                                  kind="ExternalOutput")
        dbg_fused = nc.dram_tensor("dbg_fused", [128, 4, C], F32,
                                   kind="ExternalOutput")

    with tile.TileContext(nc) as tc:
        with tc.tile_pool(name="wp", bufs=1) as wp, \
             tc.tile_pool(name="xp", bufs=2) as xp, \
             tc.tile_pool(name="qkvp", bufs=2) as qkvp, \
             tc.tile_pool(name="ap", bufs=2) as ap, \
             tc.tile_pool(name="sp", bufs=2) as sp, \
             tc.tile_pool(name="sp2", bufs=3) as sp2, \
             tc.tile_pool(name="psQ", bufs=2, space="PSUM") as psQ, \
             tc.tile_pool(name="psB", bufs=2, space="PSUM") as psB, \
             tc.tile_pool(name="psT", bufs=2, space="PSUM") as psT, \
             tc.tile_pool(name="psst", bufs=1, space="PSUM") as psst, \
             tc.tile_pool(name="psW", bufs=1, space="PSUM") as psW:

            # ---------------- persistent weights ----------------
            nc.scalar.add_instruction(mybir.InstLoadActFuncSet(
                name=nc.get_next_instruction_name(), ins=[], outs=[],
                act_func_set_id=6))
            whh_sb = wp.tile([128, 2, 2, 1536], FP8)
            wcr_sb = wp.tile([128, 4, 2, 1536], FP8)
            # split weight loads so first QKV matmuls can start early
            for mh in range(2):
                nc.sync.dma_start(whh_sb[:, :, :, mh * 768:(mh + 1) * 768],
                                  whh[:, :, :, mh * 768:(mh + 1) * 768])
                nc.sync.dma_start(wcr_sb[:, :, :, mh * 768:(mh + 1) * 768],
                                  wcr[:, :, :, mh * 768:(mh + 1) * 768])
            bqkv_sb = wp.tile([128, 12], F32)
            nc.sync.dma_start(bqkv_sb[:], bqkv[:])
            sel_sb = wp.tile([128, 4, 8], BF16)
            nc.sync.dma_start(sel_sb[:], sel[:])
            exps_sb = wp.tile([8, 4, 128], BF16)
            nc.sync.dma_start(exps_sb[:], exps[:])
            eps_sb = wp.tile([1, 1], F32)
            nc.vector.memset(eps_sb[:], 1e-5)
            w1o_sb = wp.tile([128, 12, 256], BF16)
            beff_sb = wp.tile([128, 2], F32)
            w2_sb = wp.tile([128, 2, 512], BF16)
            wmu_sb = wp.tile([128, 2], BF16)
            b2_sb = wp.tile([128, 4], F32)
            o512_sb = wp.tile([128, 1], BF16)
            ok1_sb = wp.tile([1, 128], F32R)
            waffB_sb = wp.tile([128, 4, 512], BF16)
            baffB_sb = wp.tile([128, 4], F32)
            maskex_sb = wp.tile([128, C], BF16)
            meanb2_sb = wp.tile([1, 1], F32)

            def load_tail_weights():
                nc.sync.dma_start(w1o_sb[:], w1o[:])
                nc.sync.dma_start(beff_sb[:], beff[:])
                nc.sync.dma_start(w2_sb[:], w2[:])
                nc.sync.dma_start(wmu_sb[:], wmu[:])
                nc.sync.dma_start(b2_sb[:], b2[:])
                nc.sync.dma_start(o512_sb[:], ones512[:])
                nc.sync.dma_start(ok1_sb[:], onesk1[:])
                nc.sync.dma_start(waffB_sb[:], waffB[:])
                nc.sync.dma_start(baffB_sb[:], baffB[:])
                nc.sync.dma_start(maskex_sb[:], maskex[:])
                nc.sync.dma_start(meanb2_sb[:], meanb2[:])

            def front_alloc(ch):
                """x + expert-weight loads, qkv tile allocation, chunk ch."""
                x_sb = xp.tile([128, 3, 4, 2, C], FP8, tag="x", name=f"x{ch}")
                for t in range(3):
                    nc.sync.dma_start(x_sb[:, t], xhl[ch, :, t])
                waff_sb = ap.tile([128, 4, 512], BF16, tag="waff",
                                  name=f"waff{ch}")
                nc.sync.dma_start(waff_sb[:], waffA[ch])
                baff_sb = ap.tile([128, 4], F32, tag="baff", name=f"baff{ch}")
                nc.sync.dma_start(baff_sb[:], baffA[ch])
                q_sb = qkvp.tile([128, 3, 4, C], BF16, tag="q", name=f"q{ch}")
                k_sb = qkvp.tile([128, 3, 4, C], BF16, tag="k", name=f"k{ch}")
                v_sb = qkvp.tile([128, 3, 4, C], BF16, tag="v", name=f"v{ch}")
                return {"ch": ch, "x": x_sb, "q": q_sb, "k": k_sb, "v": v_sb,
                        "waff": waff_sb, "baff": baff_sb}

            def front_qkv(st, t, mh):
                """QKV projection for token t, m-pair group mh (fp8 DR)."""
                ch = st["ch"]
                x_sb = st["x"]
                for mp in range(3 * mh, 3 * mh + 3):
                    pair = psQ.tile([128, 2, C], F32, tag="qkv",
                                    name=f"qkv{ch}_{t}_{mp}")
                    for half in range(2):
                        m = 2 * mp + half
                        mc = slice(m * 128, (m + 1) * 128)
                        for kp in range(2):
                            nc.tensor.matmul(
                                pair[:, half, :],
                                whh_sb[:, kp, :, mc],
                                x_sb[:, t, 2 * kp:2 * kp + 2, 1, :],
                                start=(kp == 0), stop=False,
                                perf_mode=MM.DoubleRow)
                        for p in range(4):
                            nc.tensor.matmul(
                                pair[:, half, :],
                                wcr_sb[:, p, :, mc],
                                x_sb[:, t, p, :, :],
                                start=False, stop=(p == 3),
                                perf_mode=MM.DoubleRow)
                    dst = (st["q"], st["k"], st["v"])[mp // 2]
                    pp = mp % 2
                    if zero_bias:
                        nc.scalar.activation(
                            dst[:, t, 2 * pp:2 * pp + 2, :], pair[:],
                            AF.Identity, scale=1.0 / 32.0)
                    else:
                        for half in range(2):
                            m = 2 * mp + half
                            nc.scalar.activation(
                                dst[:, t, 2 * pp + half, :], pair[:, half, :],
                                AF.Identity, bias=bqkv_sb[:, m:m + 1],
                                scale=1.0 / 32.0)

            def stage_back(ch, st, interleave):
                """attention middle + MLP + LN + expert for chunk ch."""
                q_sb, k_sb, v_sb = st["q"], st["k"], st["v"]
                waff_sb, baff_sb = st["waff"], st["baff"]

                # w_j = v_j - v_0 (j=1,2) on the idle Pool engine,
                # issued early so it finishes during the scores phase
                wv_sb = sp.tile([128, 2, 4, C], BF16, tag="wv", name=f"wv{ch}")
                for j in (1, 2):
                    nc.gpsimd.tensor_tensor(wv_sb[:, j - 1, :, :],
                                            v_sb[:, j, :, :], v_sb[:, 0, :, :],
                                            ALU.subtract)

                # ---------------- scores + exp ----------------
                e_sb = sp.tile([8, 3, 3, C], BF16, tag="esb", name=f"e{ch}")
                for i in range(3):
                    for j in range(3):
                        prod = sp2.tile([128, 4, C], BF16, tag="prod",
                                        name=f"prod{ch}_{i}_{j}")
                        nc.vector.tensor_tensor(
                            prod[:], q_sb[:, i, :, :], k_sb[:, j, :, :],
                            ALU.mult)
                        s_ps = psB.tile([8, C], F32, tag="bc",
                                        name=f"s{ch}_{i}_{j}")
                        for p in range(4):
                            nc.tensor.matmul(
                                s_ps[:], sel_sb[:, p, :], prod[:, p, :],
                                start=(p == 0), stop=(p == 3))
                        nc.scalar.activation(e_sb[:, i, j, :], s_ps[:],
                                             AF.Exp)
                    if len(interleave) > i:
                        interleave[i]()

                # ---------------- softmax -> p1, p2 (bf16) ----------------
                z_sb = sp.tile([8, 3, C], F32, tag="z", name=f"z{ch}")
                nc.vector.tensor_tensor(z_sb[:], e_sb[:, :, 0, :],
                                        e_sb[:, :, 1, :], ALU.add)
                nc.vector.tensor_tensor(z_sb[:], z_sb[:], e_sb[:, :, 2, :],
                                        ALU.add)
                rz_sb = sp.tile([8, 3, C], F32, tag="rz", name=f"rz{ch}")
                nc.vector.reciprocal_approx_fast(rz_sb[:], z_sb[:])
                d_sb = sp.tile([8, 3, 2, C], BF16, tag="dsb", name=f"d{ch}")
                for j in (1, 2):
                    nc.vector.tensor_tensor(d_sb[:, :, j - 1, :],
                                            e_sb[:, :, j, :], rz_sb[:],
                                            ALU.mult)
                if len(interleave) > 3:
                    interleave[3]()

                # ------------- o_i = v0 + p_i1*w1 + p_i2*w2 -------------
                o_sb = ap.tile([128, 12, C], BF16, tag="o", name=f"o{ch}")
                for i in range(3):
                    pv = sp2.tile([128, 4, 2, C], BF16, tag="pv",
                                  name=f"pv{ch}_{i}")
                    for p in range(4):
                        bc = psB.tile([128, 2, C], F32, tag="bc",
                                      name=f"bc{ch}_{i}_{p}")
                        for j in (1, 2):
                            nc.tensor.matmul(
                                bc[:, j - 1, :], exps_sb[:, p, :],
                                d_sb[:, i, j - 1, :], start=True, stop=True)
                        nc.vector.tensor_tensor(pv[:, p, :, :], bc[:],
                                                wv_sb[:, :, p, :], ALU.mult)
                    t_sb = sp2.tile([128, 4, C], BF16, tag="pvs",
                                    name=f"pvs{ch}_{i}")
                    nc.vector.tensor_tensor(t_sb[:], pv[:, :, 0, :],
                                            pv[:, :, 1, :], ALU.add)
                    nc.vector.tensor_tensor(o_sb[:, i * 4:(i + 1) * 4, :],
                                            t_sb[:], v_sb[:, 0, :, :],
                                            ALU.add)


                # ---------------- W1o + ReLU + mu ----------------
                hp = psT.tile([128, 2, C], F32, tag="tail", name=f"hp{ch}")
                for m2t in range(2):
                    for kip in range(12):
                        nc.tensor.matmul(
                            hp[:, m2t, :],
                            w1o_sb[:, kip, m2t * 128:(m2t + 1) * 128],
                            o_sb[:, kip, :],
                            start=(kip == 0), stop=(kip == 11))
                hpre = ap.tile([128, 2, C], BF16, tag="hpre", name=f"hpre{ch}")
                for m2t in range(2):
                    nc.scalar.activation(hpre[:, m2t, :], hp[:, m2t, :],
                                         AF.Relu, bias=beff_sb[:, m2t:m2t + 1])
                if len(interleave) > 4:
                    interleave[4]()
                st_ps = psst.tile([1, 2, C], F32, tag="st", name=f"st{ch}")
                for ks in range(2):
                    nc.tensor.matmul(st_ps[:, 0, :], wmu_sb[:, ks:ks + 1],
                                     hpre[:, ks, :],
                                     start=(ks == 0), stop=(ks == 1))

                # ---------------- fus2 -> y, ysq ----------------
                y_sb = ap.tile([128, 4, C], BF16, tag="y", name=f"y{ch}")
                ysq = sp2.tile([128, 4, C], BF16, tag="ysq", name=f"ysq{ch}")
                for yp2 in range(2):
                    yp = psT.tile([128, 2, C], F32, tag="tail",
                                  name=f"yp{ch}_{yp2}")
                    for half in range(2):
                        m4 = 2 * yp2 + half
                        for ks in range(2):
                            nc.tensor.matmul(
                                yp[:, half, :],
                                w2_sb[:, ks, m4 * 128:(m4 + 1) * 128],
                                hpre[:, ks, :], start=(ks == 0),
                                stop=(ks == 1))
                    if zero_bias:
                        nc.scalar.activation(
                            y_sb[:, 2 * yp2:2 * yp2 + 2, :], yp[:],
                            AF.Identity)
                        nc.scalar.activation(
                            ysq[:, 2 * yp2:2 * yp2 + 2, :], yp[:], AF.Square)
                    else:
                        for half in range(2):
                            m4 = 2 * yp2 + half
                            nc.scalar.activation(
                                y_sb[:, m4, :], yp[:, half, :], AF.Identity,
                                bias=b2_sb[:, m4:m4 + 1])
                            nc.scalar.activation(
                                ysq[:, m4, :], yp[:, half, :], AF.Square,
                                bias=b2_sb[:, m4:m4 + 1])
                # ---------------- LN stats ----------------
                for p in range(4):
                    nc.tensor.matmul(st_ps[:, 1, :], o512_sb[:], ysq[:, p, :],
                                     start=(p == 0), stop=(p == 3))
                if len(interleave) > 5:
                    interleave[5]()
                mu_sb = sp.tile([1, C], F32, tag="musb", name=f"musb{ch}")
                if zero_bias:
                    nc.vector.tensor_scalar_add(mu_sb[:], st_ps[:, 0, :], 0.0)
                else:
                    nc.vector.tensor_scalar_add(mu_sb[:], st_ps[:, 0, :],
                                                meanb2_sb[:])
                musq = sp.tile([1, C], F32, tag="musq", name=f"musq{ch}")
                nc.gpsimd.tensor_tensor(musq[:], mu_sb[:], mu_sb[:], ALU.mult)
                var_sb = sp.tile([1, C], F32, tag="varsb", name=f"var{ch}")
                nc.vector.tensor_tensor(var_sb[:], st_ps[:, 1, :], musq[:],
                                        ALU.subtract)
                # rstd = exp(-0.5 * ln(var + eps)); ln & exp share a table set
                lnv = sp.tile([1, C], F32, tag="lnv", name=f"lnv{ch}")
                nc.scalar.activation(lnv[:], var_sb[:], AF.Ln, bias=eps_sb[:])
                rstd_sb = sp.tile([1, C], F32R, tag="rstd", name=f"rstd{ch}")
                nc.scalar.activation(rstd_sb[:], lnv[:], AF.Exp, scale=-0.5)
                murs = sp.tile([1, C], F32R, tag="murs", name=f"murs{ch}")
                nc.gpsimd.tensor_tensor(murs[:], mu_sb[:], rstd_sb[:],
                                        ALU.mult)
                # broadcast murs & rstd over partitions via PE
                bcp = psW.tile([128, 2, C], F32, tag="bcp", name=f"bcp{ch}")
                nc.tensor.matmul(bcp[:, 0, :], ok1_sb[:],
                                 murs[:], start=True, stop=True)
                nc.tensor.matmul(bcp[:, 1, :], ok1_sb[:],
                                 rstd_sb[:], start=True, stop=True)
                mrex = sp.tile([128, 2, C], BF16, tag="mrex",
                               name=f"mrex{ch}")
                nc.scalar.activation(mrex[:], bcp[:], AF.Identity)
                fused = ap.tile([128, 4, C], BF16, tag="fused",
                                name=f"fused{ch}")
                nc.vector.tensor_tensor(
                    fused[:], y_sb[:],
                    mrex[:, 1, None, :].to_broadcast((128, 4, C)), ALU.mult)
                nc.vector.tensor_tensor(
                    fused[:], fused[:],
                    mrex[:, 0, None, :].to_broadcast((128, 4, C)),
                    ALU.subtract)

                if debug and ch == 0:
                    nc.sync.dma_start(dbg_q[:], q_sb[:])
                    nc.sync.dma_start(dbg_k[:], k_sb[:])
                    nc.sync.dma_start(dbg_v[:], v_sb[:])
                    nc.sync.dma_start(dbg_e[:], e_sb[:])
                    nc.sync.dma_start(dbg_d[:], d_sb[:])
                    nc.sync.dma_start(dbg_o[:], o_sb[:])
                    nc.sync.dma_start(dbg_h[:], hpre[:])
                    nc.sync.dma_start(dbg_y[:], y_sb[:])
                    nc.sync.dma_start(dbg_mu[:], mu_sb[:])
                    nc.sync.dma_start(dbg_var[:], var_sb[:])
                    nc.sync.dma_start(dbg_rstd[:], rstd_sb[:])
                    nc.sync.dma_start(dbg_fused[:], fused[:])

                # ---------------- routed expert matmul ----------------
                ot = sp2.tile([128, 4, C], BF16, tag="ot", name=f"ot{ch}")
                is_mixed = (ch == NCH - 1)
                for op2 in range(2):
                    op = psT.tile([128, 2, C], F32, tag="tail",
                                  name=f"op{ch}_{op2}")
                    for half in range(2):
                        m4 = 2 * op2 + half
                        for ks in range(4):
                            nc.tensor.matmul(
                                op[:, half, :],
                                waff_sb[:, ks, m4 * 128:(m4 + 1) * 128],
                                fused[:, ks, :], start=(ks == 0),
                                stop=(ks == 3))
                    if not is_mixed:
                        if zero_bias:
                            nc.scalar.activation(
                                ot[:, 2 * op2:2 * op2 + 2, :], op[:],
                                AF.Identity)
                        else:
                            for half in range(2):
                                m4 = 2 * op2 + half
                                nc.scalar.activation(
                                    ot[:, m4, :], op[:, half, :], AF.Identity,
                                    bias=baff_sb[:, m4:m4 + 1])
                    else:
                        opB = psB.tile([128, 2, C], F32, tag="bc",
                                       name=f"opB{ch}_{op2}")
                        for half in range(2):
                            m4 = 2 * op2 + half
                            for ks in range(4):
                                nc.tensor.matmul(
                                    opB[:, half, :],
                                    waffB_sb[:, ks, m4 * 128:(m4 + 1) * 128],
                                    fused[:, ks, :], start=(ks == 0),
                                    stop=(ks == 3))
                        # out = oB + (oA - oB) * mask, with per-side biases
                        oA = sp2.tile([128, 2, C], BF16, tag="oA",
                                      name=f"oA{ch}_{op2}")
                        oB = sp2.tile([128, 2, C], BF16, tag="oB",
                                      name=f"oB{ch}_{op2}")
                        if zero_bias:
                            nc.scalar.activation(oA[:], op[:], AF.Identity)
                            nc.scalar.activation(oB[:], opB[:], AF.Identity)
                        else:
                            for half in range(2):
                                m4 = 2 * op2 + half
                                nc.scalar.activation(
                                    oA[:, half, :], op[:, half, :],
                                    AF.Identity, bias=baff_sb[:, m4:m4 + 1])
                                nc.scalar.activation(
                                    oB[:, half, :], opB[:, half, :],
                                    AF.Identity, bias=baffB_sb[:, m4:m4 + 1])
                        dAB = sp2.tile([128, 2, C], BF16, tag="dAB",
                                       name=f"dAB{ch}_{op2}")
                        nc.vector.tensor_tensor(dAB[:], oA[:], oB[:],
                                                ALU.subtract)
                        nc.vector.tensor_tensor(
                            dAB[:], dAB[:],
                            maskex_sb[:, None, :].to_broadcast((128, 2, C)),
                            ALU.mult)
                        nc.vector.tensor_tensor(
                            ot[:, 2 * op2:2 * op2 + 2, :], oB[:], dAB[:],
                            ALU.add)
                nc.sync.dma_start(outT[ch], ot[:])

            cur = front_alloc(0)
            for t in range(3):
                for mh in range(2):
                    front_qkv(cur, t, mh)
                if t == 0:
                    load_tail_weights()
            for ch in range(NCH):
                if ch + 1 < NCH:
                    nxt = front_alloc(ch + 1)
                    il = [lambda t=t, mh=mh, s=nxt: front_qkv(s, t, mh)
                          for t in range(3) for mh in range(2)]
                else:
                    nxt, il = None, []
                stage_back(ch, cur, il)
                cur = nxt

    nc.finalize()
    _NC_CACHE[key] = nc
    return nc


def _fp8_split(a):
    """Return (lo, hi) fp8e4m3 arrays with hi + lo ~= a."""
    hi = a.astype(FP8NP)
    lo = (a - hi.astype(np.float32)).astype(FP8NP)
    return lo, hi


def _prep_weights(inputs):
    in_proj_w = np.asarray(inputs["in_proj_w"], np.float32)
    in_proj_b = np.asarray(inputs["in_proj_b"], np.float32)
    out_proj_w = np.asarray(inputs["out_proj_w"], np.float32)
    out_proj_b = np.asarray(inputs["out_proj_b"], np.float32)
    fus_w1 = np.asarray(inputs["fus_w1"], np.float32)
    fus_b1 = np.asarray(inputs["fus_b1"], np.float32)
    fus_w2 = np.asarray(inputs["fus_w2"], np.float32)
    fus_b2 = np.asarray(inputs["fus_b2"], np.float32)
    ln_g = np.asarray(inputs["ln_g"], np.float32)
    ln_b = np.asarray(inputs["ln_b"], np.float32)
    aff_w = np.asarray(inputs["aff_w"], np.float32)
    aff_b = np.asarray(inputs["aff_b"], np.float32)

    scale = 1.0 / np.sqrt(np.float32(HD))
    W = in_proj_w.copy()
    W[:E] *= scale
    bq = in_proj_b.copy()
    bq[:E] *= scale
    # pre-scale W by 2^5 so the fp8 hi/lo planes stay out of e4m3's
    # subnormal range (W sigma ~0.02); undone by the Act copy scale 2^-5
    W *= 32.0

    # W.T is [512(k), 1536(m)]
    WT = np.ascontiguousarray(W.T)
    WT_lo, WT_hi = _fp8_split(WT)
    WT_lo = WT_lo.reshape(4, 128, 1536)
    WT_hi = WT_hi.reshape(4, 128, 1536)
    # hi-hi pairs: [128, kp, e, 1536] with (kp,e) -> k-subtile 2*kp+e
    whh_h = np.empty((128, 2, 2, 1536), FP8NP)
    for kp in range(2):
        for e_ in range(2):
            whh_h[:, kp, e_, :] = WT_hi[2 * kp + e_]
    # cross pairs: [128, p, {hi,lo}, 1536], paired with x (lo, hi)
    wcr_h = np.empty((128, 4, 2, 1536), FP8NP)
    for p in range(4):
        wcr_h[:, p, 0, :] = WT_hi[p]
        wcr_h[:, p, 1, :] = WT_lo[p]
    bqkv_h = np.ascontiguousarray(bq.reshape(12, 128).T)

    # fold out_proj into fus_w1; permute (h,d) -> (p, hl, d) to match v layout
    perm = np.empty(E, np.int64)
    for h in range(NH):
        for d in range(HD):
            perm[(h // 2) * 128 + (h % 2) * 64 + d] = h * HD + d
    blocks = []
    for i in range(3):
        blk = fus_w1[:, i * E:(i + 1) * E] @ out_proj_w  # [256, 512]
        blocks.append(blk[:, perm])
    W1o = np.concatenate(blocks, axis=1)  # [256, 1536]
    w1o_h = np.ascontiguousarray(
        W1o.T.reshape(12, 128, 256).transpose(1, 0, 2)).astype(BF16NP)
    # v bias folds into beff: o_i includes +bv for each i
    beff = fus_b1 + fus_w1 @ np.tile(out_proj_b, 3)
    beff_h = np.ascontiguousarray(beff.reshape(2, 128).T)

    w2_h = np.ascontiguousarray(
        fus_w2.T.reshape(2, 128, 512).transpose(1, 0, 2)).astype(BF16NP)
    wmu_h = np.ascontiguousarray(
        (fus_w2.mean(axis=0)).reshape(2, 128).T).astype(BF16NP)
    b2_h = np.ascontiguousarray(fus_b2.reshape(4, 128).T)

    # gamma/beta folded into expert weights/biases
    Wp = aff_w * ln_g[None, None, :]                   # [NE, 512, 512]
    bp = aff_w @ ln_b + aff_b                          # [NE, 512]
    waff_e = []
    for e_ in range(NE):
        A = np.ascontiguousarray(
            Wp[e_].T.reshape(4, 128, 512).transpose(1, 0, 2))
        waff_e.append(A.astype(BF16NP))
    baff_e = [np.ascontiguousarray(bp[e_].reshape(4, 128).T)
              for e_ in range(NE)]

    sel_h = np.zeros((128, 4, 8), np.float32)
    for r in range(128):
        for p in range(4):
            sel_h[r, p, 2 * p + r // 64] = 1.0
    exps_h = np.zeros((8, 4, 128), np.float32)
    for p in range(4):
        for c in range(128):
            exps_h[2 * p + c // 64, p, c] = 1.0

    zero_bias = (np.all(bq == 0) and np.all(in_proj_b[E:] == 0)
                 and np.all(fus_b2 == 0)
                 and all(np.all(b == 0) for b in baff_e))

    base = {
        "whh": whh_h, "wcr": wcr_h, "bqkv": bqkv_h,
        "w1o": w1o_h, "beff": beff_h, "w2": w2_h, "wmu": wmu_h, "b2": b2_h,
        "sel": sel_h.astype(BF16NP), "exps": exps_h.astype(BF16NP),
        "ones512": np.full((128, 1), 1.0 / E, np.float32).astype(BF16NP),
        "onesk1": np.ones((1, 128), np.float32),
        "meanb2": np.full((1, 1), fus_b2.mean(), np.float32),
    }
    return base, waff_e, baff_e, zero_bias, perm


def _pack_slots(labels):
    """Assign samples to 64 slots of 256; return per-core chunk plans."""
    order_ids = [np.nonzero(labels == e_)[0] for e_ in range(NE)]
    counts = [len(x) for x in order_ids]
    assert sum(counts) == B
    slots = []          # list of (ids[256], eA, eB, nA)
    leftovers = []      # (expert, ids)
    for e_ in range(NE):
        ids = order_ids[e_]
        nfull = len(ids) // C
        for s in range(nfull):
            slots.append((ids[s * C:(s + 1) * C], e_, e_, C))
        if len(ids) % C:
            leftovers.append((e_, ids[nfull * C:]))
    # pack leftovers into mixed slots (each must span <= 2 experts)
    mixed = []
    stream = []
    for e_, ids in leftovers:
        stream.append((e_, list(ids)))
    while stream:
        eA, idsA = stream[0]
        if len(idsA) >= C:
            mixed.append((np.array(idsA[:C]), eA, eA, C))
            stream[0] = (eA, idsA[C:])
            if not stream[0][1]:
                stream.pop(0)
            continue
        if len(stream) == 1:
            assert len(idsA) == 0 or len(idsA) == C, \
                f"unpackable remainder {len(idsA)}"
            if idsA:
                mixed.append((np.array(idsA), eA, eA, C))
            stream.pop(0)
            continue
        eB, idsB = stream[1]
        take = C - len(idsA)
        assert len(idsB) >= take, (
            f"slot would span 3 experts: {len(idsA)} + {len(idsB)} < {C}")
        ids = np.concatenate([idsA, idsB[:take]])
        mixed.append((ids, eA, eB, len(idsA)))
        stream.pop(0)
        stream[1 - 1] = (eB, idsB[take:])
        if not stream[0][1]:
            stream.pop(0)
    assert len(slots) + len(mixed) == B // C
    assert len(mixed) <= NCORES, f"{len(mixed)} mixed slots > {NCORES} cores"

    # per core: 8 slots, mixed slot (if any) at position 7
    plans = []
    si = 0
    for c in range(NCORES):
        mine = []
        if c < len(mixed):
            n_pure = NCH - 1
        else:
            n_pure = NCH
        mine = slots[si:si + n_pure]
        si += n_pure
        if c < len(mixed):
            mine = mine + [mixed[c]]
        plans.append(mine)
    assert si == len(slots)
    return plans


def kernel(**inputs):
    img = np.asarray(inputs["image_embeddings"], np.float32)
    txt = np.asarray(inputs["text_embeddings"], np.float32)
    kno = np.asarray(inputs["knowledge_embeddings"], np.float32)
    labels = np.asarray(inputs["affective_labels"]).astype(np.int64).ravel()
    assert img.shape == (B, E)

    base, waff_e, baff_e, zero_bias, perm = _prep_weights(inputs)
    plans = _pack_slots(labels)

    xs = np.stack([img, txt, kno])                     # [3, B, 512]

    in_maps = []
    for c in range(NCORES):
        plan = plans[c]
        gi = np.concatenate([p[0] for p in plan])      # [2048]
        xg = xs[:, gi, :].transpose(0, 2, 1)           # [3, 512, R]
        xg = xg.reshape(3, 4, 128, NCH, C)             # [t, p, r, ch, c]
        x_hi = xg.astype(FP8NP)
        x_lo = (xg - x_hi.astype(np.float32)).astype(FP8NP)
        xhl_h = np.empty((NCH, 128, 3, 4, 2, C), FP8NP)
        xhl_h[:, :, :, :, 0, :] = x_lo.transpose(3, 2, 0, 1, 4)
        xhl_h[:, :, :, :, 1, :] = x_hi.transpose(3, 2, 0, 1, 4)

        waffA_h = np.stack([waff_e[p[1]] for p in plan])     # [NCH,128,4,512]
        baffA_h = np.stack([baff_e[p[1]] for p in plan])     # [NCH,128,4]
        eB = plan[NCH - 1][2]
        waffB_h = waff_e[eB]
        baffB_h = baff_e[eB]
        nA = plan[NCH - 1][3]
        mask = np.zeros((128, C), np.float32)
        mask[:, :nA] = 1.0

        m = dict(base)
        m["xhl"] = xhl_h
        m["waffA"] = waffA_h
        m["baffA"] = baffA_h
        m["waffB"] = waffB_h
        m["baffB"] = baffB_h
        m["maskex"] = mask.astype(BF16NP)
        in_maps.append(m)

    nc = _build_program(zero_bias)
    res = run_bass_kernel_spmd(nc, in_maps, core_ids=list(range(NCORES)))
    global LAST_RESULTS, LAST_NC
    LAST_RESULTS = res
    LAST_NC = nc

    out_full = np.zeros((B, E), np.float32)
    for c in range(NCORES):
        oT = res.results[c]["outT"]                    # [NCH,128,4,C] bf16
        oT = np.asarray(oT, dtype=np.float32)
        plan = plans[c]
        for ch in range(NCH):
            ids = plan[ch][0]
            # [128, 4, C] -> [C, 512] with feature f = 128*p + r
            blk = oT[ch].transpose(2, 1, 0).reshape(C, 4 * 128)
            out_full[ids] = blk
    return out_full


if __name__ == "__main__":
    rng = np.random.default_rng(0)
    fake = {
        "image_embeddings": rng.standard_normal((B, E)).astype(np.float32),
        "text_embeddings": rng.standard_normal((B, E)).astype(np.float32),
        "knowledge_embeddings": rng.standard_normal((B, E)).astype(np.float32),
        "affective_labels": rng.integers(0, NE, B),
        "in_proj_w": (rng.standard_normal((3 * E, E)) * 0.02).astype(np.float32),
        "in_proj_b": np.zeros(3 * E, np.float32),
        "out_proj_w": (rng.standard_normal((E, E)) * 0.02).astype(np.float32),
        "out_proj_b": np.zeros(E, np.float32),
        "fus_w1": (rng.standard_normal((H, 3 * E)) * 0.02).astype(np.float32),
        "fus_b1": np.zeros(H, np.float32),
        "fus_w2": (rng.standard_normal((E, H)) * 0.02).astype(np.float32),
        "fus_b2": np.zeros(E, np.float32),
        "ln_g": np.ones(E, np.float32),
        "ln_b": np.zeros(E, np.float32),
        "aff_w": (rng.standard_normal((NE, E, E)) * 0.02).astype(np.float32),
        "aff_b": np.zeros((NE, E), np.float32),
    }
    out = kernel(**fake)
    print("kernel ran, out:", out.shape, out.dtype, np.abs(out).max())


# revision 3
# speedup vs baseline: 1.0545x; 1.0545x over previous
"""Trainium2 Bass kernel v2 for nn_MultiModalFusion (moe_routing).

Strategy (v2, derived from the TimelineSim cost model):
- Data parallel over 8 cores. Host sorts samples by expert into 64 slots of
  exactly 256 (zero padding waste); at most 3 slots mix two experts — those go
  to chunk slot 7 of cores 0..2, where the program computes both experts and
  mask-blends.
- QKV projection in fp8e4m3 with host-side hi/lo error compensation
  (x = x_hi + x_lo, W = W_hi + W_lo; the three significant cross products are
  packed into DoubleRow matmuls at 0.5 cyc/row -> 3C per (token, m-tile)
  instead of fp32r's 4C, with ~bf16-level accuracy).
- All other matmul moving operands bf16 (1 cyc/row at any width).
- Attention probs via the identity o_i = v0 + p_i1*(v1-v0) + p_i2*(v2-v0):
  only 2 of 3 probability broadcasts per query (24 instead of 36 broadcast
  matmul columns), probs in bf16.
- LayerNorm: mean folded into an extra fus2 output row (host-precomputed
  column-mean weight vector), rstd = exp(-0.5*ln(var+eps)) on the scalar
  engine — ln/exp/identity/relu/square share one activation table set, so no
  LoadActFuncSet thrash. gamma/beta folded into expert weights on host.
- PSUM pair tiles (2 x 256 f32 = one 2KB bank) let one Activation instruction
  evacuate two matmul outputs.
- DVE kept in 2x bf16 mode where possible; part of the prob*value work is
  offloaded to the idle GPSIMD (Pool) engine.
"""

import numpy as np
import ml_dtypes

import concourse.bass as bass
import concourse.mybir as mybir
import concourse.tile as tile
from concourse import bacc
from concourse.bass_utils import run_bass_kernel_spmd

E = 512
H = 256
NH = 8
HD = 64
NE = 4
B = 16384
NCORES = 8
C = 256              # chunk columns
NCH = 8              # chunks per core
R = NE * 512         # 2048 columns per core

LAST_RESULTS = None
LAST_NC = None

F32 = mybir.dt.float32
F32R = mybir.dt.float32r
BF16 = mybir.dt.bfloat16
FP8 = mybir.dt.float8e4
AF = mybir.ActivationFunctionType
ALU = mybir.AluOpType
MM = mybir.MatmulPerfMode

FP8NP = ml_dtypes.float8_e4m3
BF16NP = ml_dtypes.bfloat16

_NC_CACHE = {}


def _build_program(zero_bias, debug=False):
    key = (bool(zero_bias), bool(debug))
    if key in _NC_CACHE:
        return _NC_CACHE[key]
    nc = bacc.Bacc("TRN2")

    # ---------------- DRAM I/O ----------------
    # x hi/lo fp8: [ch, 128, t, p, plane(0=lo,1=hi), C]
    xhl = nc.dram_tensor("xhl", [NCH, 128, 3, 4, 2, C], FP8,
                         kind="ExternalInput")
    # QKV weights fp8: hi-hi k-pairs and (hi,lo) cross pairs
    whh = nc.dram_tensor("whh", [128, 2, 2, 1536], FP8, kind="ExternalInput")
    wcr = nc.dram_tensor("wcr", [128, 4, 2, 1536], FP8, kind="ExternalInput")
    bqkv = nc.dram_tensor("bqkv", [128, 12], F32, kind="ExternalInput")
    w1o = nc.dram_tensor("w1o", [128, 12, 256], BF16, kind="ExternalInput")
    beff = nc.dram_tensor("beff", [128, 2], F32, kind="ExternalInput")
    w2 = nc.dram_tensor("w2", [128, 2, 512], BF16, kind="ExternalInput")
    wmu = nc.dram_tensor("wmu", [128, 2], BF16, kind="ExternalInput")
    b2 = nc.dram_tensor("b2", [128, 4], F32, kind="ExternalInput")
    # per-chunk expert weights, host-selected: A for all chunks, B for ch 7
    waffA = nc.dram_tensor("waffA", [NCH, 128, 4, 512], BF16,
                           kind="ExternalInput")
    waffB = nc.dram_tensor("waffB", [128, 4, 512], BF16,
                           kind="ExternalInput")
    baffA = nc.dram_tensor("baffA", [NCH, 128, 4], F32,
                           kind="ExternalInput")
    baffB = nc.dram_tensor("baffB", [128, 4], F32, kind="ExternalInput")
    maskex = nc.dram_tensor("maskex", [128, C], BF16, kind="ExternalInput")
    meanb2 = nc.dram_tensor("meanb2", [1, 1], F32, kind="ExternalInput")
    sel = nc.dram_tensor("sel", [128, 4, 8], BF16, kind="ExternalInput")
    exps = nc.dram_tensor("exps", [8, 4, 128], BF16, kind="ExternalInput")
    ones512 = nc.dram_tensor("ones512", [128, 1], BF16, kind="ExternalInput")
    onesk1 = nc.dram_tensor("onesk1", [1, 128], F32R, kind="ExternalInput")
    outT = nc.dram_tensor("outT", [NCH, 128, 4, C], BF16,
                          kind="ExternalOutput")
    if debug:
        dbg_q = nc.dram_tensor("dbg_q", [128, 3, 4, C], F32,
                               kind="ExternalOutput")
        dbg_k = nc.dram_tensor("dbg_k", [128, 3, 4, C], F32,
                               kind="ExternalOutput")
        dbg_v = nc.dram_tensor("dbg_v", [128, 3, 4, C], F32,
                               kind="ExternalOutput")
        dbg_e = nc.dram_tensor("dbg_e", [8, 3, 3, C], F32,
                               kind="ExternalOutput")
        dbg_d = nc.dram_tensor("dbg_d", [8, 3, 2, C], F32,
                               kind="ExternalOutput")
        dbg_o = nc.dram_tensor("dbg_o", [128, 12, C], F32,
                               kind="ExternalOutput")
        dbg_h = nc.dram_tensor("dbg_h", [128, 2, C], F32,
                               kind="ExternalOutput")
        dbg_y = nc.dram_tensor("dbg_y", [128, 4, C], F32,
                               kind="ExternalOutput")
        dbg_mu = nc.dram_tensor("dbg_mu", [1, C], F32,
                                kind="ExternalOutput")
        dbg_var = nc.dram_tensor("dbg_var", [1, C], F32,
                                 kind="ExternalOutput")
        dbg_rstd = nc.dram_tensor("dbg_rstd", [1, C], F32,
                                  kind="ExternalOutput")
        dbg_fused = nc.dram_tensor("dbg_fused", [128, 4, C], F32,
                                   kind="ExternalOutput")

    with tile.TileContext(nc) as tc:
        with tc.tile_pool(name="wp", bufs=1) as wp, \
             tc.tile_pool(name="xp", bufs=2) as xp, \
             tc.tile_pool(name="qkvp", bufs=2) as qkvp, \
             tc.tile_pool(name="ap", bufs=2) as ap, \
             tc.tile_pool(name="spM", bufs=2) as spM, \
             tc.tile_pool(name="spZ", bufs=2) as spZ, \
             tc.tile_pool(name="spS", bufs=2) as spS, \
             tc.tile_pool(name="mrexp", bufs=2) as mrexp, \
             tc.tile_pool(name="sp2", bufs=3) as sp2, \
             tc.tile_pool(name="sp3", bufs=2) as sp3, \
             tc.tile_pool(name="psQ", bufs=2, space="PSUM") as psQ, \
             tc.tile_pool(name="psB", bufs=2, space="PSUM") as psB, \
             tc.tile_pool(name="psT", bufs=2, space="PSUM") as psT, \
             tc.tile_pool(name="psst", bufs=1, space="PSUM") as psst, \
             tc.tile_pool(name="psW", bufs=1, space="PSUM") as psW:

            # ---------------- persistent weights ----------------
            nc.scalar.add_instruction(mybir.InstLoadActFuncSet(
                name=nc.get_next_instruction_name(), ins=[], outs=[],
                act_func_set_id=6))
            whh_sb = wp.tile([128, 2, 2, 1536], FP8)
            wcr_sb = wp.tile([128, 4, 2, 1536], FP8)
            # first halves only — QKV m-tiles 0..5 can start ASAP
            nc.sync.dma_start(whh_sb[:, :, :, 0:768], whh[:, :, :, 0:768])
            nc.sync.dma_start(wcr_sb[:, :, :, 0:768], wcr[:, :, :, 0:768])
            bqkv_sb = wp.tile([128, 12], F32)
            sel_sb = wp.tile([128, 4, 8], BF16)
            exps_sb = wp.tile([8, 4, 128], BF16)
            eps_sb = wp.tile([1, 1], F32)

            def load_more_front():
                nc.sync.dma_start(whh_sb[:, :, :, 768:], whh[:, :, :, 768:])
                nc.sync.dma_start(wcr_sb[:, :, :, 768:], wcr[:, :, :, 768:])
                nc.sync.dma_start(bqkv_sb[:], bqkv[:])
                nc.sync.dma_start(sel_sb[:], sel[:])
                nc.sync.dma_start(exps_sb[:], exps[:])
                nc.vector.memset(eps_sb[:], 1e-5)
            w1o_sb = wp.tile([128, 12, 256], BF16)
            beff_sb = wp.tile([128, 2], F32)
            w2_sb = wp.tile([128, 2, 512], BF16)
            wmu_sb = wp.tile([128, 2], BF16)
            b2_sb = wp.tile([128, 4], F32)
            o512_sb = wp.tile([128, 1], BF16)
            ok1_sb = wp.tile([1, 128], F32R)
            waffB_sb = wp.tile([128, 4, 512], BF16)
            baffB_sb = wp.tile([128, 4], F32)
            maskex_sb = wp.tile([128, C], BF16)
            meanb2_sb = wp.tile([1, 1], F32)

            def load_tail_weights():
                nc.sync.dma_start(w1o_sb[:], w1o[:])
                nc.sync.dma_start(beff_sb[:], beff[:])
                nc.sync.dma_start(w2_sb[:], w2[:])
                nc.sync.dma_start(wmu_sb[:], wmu[:])
                nc.sync.dma_start(b2_sb[:], b2[:])
                nc.sync.dma_start(o512_sb[:], ones512[:])
                nc.sync.dma_start(ok1_sb[:], onesk1[:])
                nc.sync.dma_start(waffB_sb[:], waffB[:])
                nc.sync.dma_start(baffB_sb[:], baffB[:])
                nc.sync.dma_start(maskex_sb[:], maskex[:])
                nc.sync.dma_start(meanb2_sb[:], meanb2[:])

            def front_alloc(ch):
                """x + expert-weight loads, qkv tile allocation, chunk ch."""
                x_sb = xp.tile([128, 3, 4, 2, C], FP8, tag="x", name=f"x{ch}")
                for t in range(3):
                    nc.sync.dma_start(x_sb[:, t], xhl[ch, :, t])
                waff_sb = ap.tile([128, 4, 512], BF16, tag="waff",
                                  name=f"waff{ch}")
                nc.sync.dma_start(waff_sb[:], waffA[ch])
                baff_sb = ap.tile([128, 4], F32, tag="baff", name=f"baff{ch}")
                nc.sync.dma_start(baff_sb[:], baffA[ch])
                q_sb = qkvp.tile([128, 3, 4, C], BF16, tag="q", name=f"q{ch}")
                k_sb = qkvp.tile([128, 3, 4, C], BF16, tag="k", name=f"k{ch}")
                v_sb = qkvp.tile([128, 3, 4, C], BF16, tag="v", name=f"v{ch}")
                return {"ch": ch, "x": x_sb, "q": q_sb, "k": k_sb, "v": v_sb,
                        "waff": waff_sb, "baff": baff_sb}

            def front_qkv_mp(st, t, mp):
                """QKV m-pair mp (12 DR matmuls + 1 pair copy) — atomic."""
                ch = st["ch"]
                x_sb = st["x"]
                pair = psQ.tile([128, 2, C], F32, tag="qkv",
                                name=f"qkv{ch}_{t}_{mp}")
                for half in range(2):
                    m = 2 * mp + half
                    mc = slice(m * 128, (m + 1) * 128)
                    for kp in range(2):
                        nc.tensor.matmul(
                            pair[:, half, :],
                            whh_sb[:, kp, :, mc],
                            x_sb[:, t, 2 * kp:2 * kp + 2, 1, :],
                            start=(kp == 0), stop=False,
                            perf_mode=MM.DoubleRow)
                    for p in range(4):
                        nc.tensor.matmul(
                            pair[:, half, :],
                            wcr_sb[:, p, :, mc],
                            x_sb[:, t, p, :, :],
                            start=False, stop=(p == 3),
                            perf_mode=MM.DoubleRow)
                dst = (st["q"], st["k"], st["v"])[mp // 2]
                pp = mp % 2
                if zero_bias:
                    nc.scalar.activation(
                        dst[:, t, 2 * pp:2 * pp + 2, :], pair[:],
                        AF.Identity, scale=1.0 / 32.0)
                else:
                    for half in range(2):
                        m = 2 * mp + half
                        nc.scalar.activation(
                            dst[:, t, 2 * pp + half, :], pair[:, half, :],
                            AF.Identity, bias=bqkv_sb[:, m:m + 1],
                            scale=1.0 / 32.0)

            def middle(ch, st, pieces, tailph):
                """Scores/softmax/o for chunk ch; interleaves previous
                chunk's tail phases (tailph) and next-chunk QKV pieces."""
                def pull(n=1):
                    for _ in range(n):
                        if pieces:
                            pieces.pop(0)()
                q_sb, k_sb, v_sb = st["q"], st["k"], st["v"]

                # w_j = v_j - v_0 early on the idle Pool engine
                wv_sb = spM.tile([128, 2, 4, C], BF16, tag="wv",
                                 name=f"wv{ch}")
                for j in (1, 2):
                    nc.gpsimd.tensor_tensor(wv_sb[:, j - 1, :, :],
                                            v_sb[:, j, :, :],
                                            v_sb[:, 0, :, :],
                                            ALU.subtract)

                # scores + exp; tail W1o/fus2 phases fill the DVE-bound spots
                e_sb = spM.tile([8, 3, 3, C], BF16, tag="esb", name=f"e{ch}")
                for i in range(3):
                    for j in range(3):
                        prod = sp2.tile([128, 4, C], BF16, tag="prod",
                                        name=f"prod{ch}_{i}_{j}")
                        nc.vector.tensor_tensor(
                            prod[:], q_sb[:, i, :, :], k_sb[:, j, :, :],
                            ALU.mult)
                        s_ps = psB.tile([8, C], F32, tag="bc",
                                        name=f"s{ch}_{i}_{j}")
                        for p in range(4):
                            nc.tensor.matmul(
                                s_ps[:], sel_sb[:, p, :], prod[:, p, :],
                                start=(p == 0), stop=(p == 3))
                        nc.scalar.activation(e_sb[:, i, j, :], s_ps[:],
                                             AF.Exp)
                    if tailph:
                        tailph.pop(0)()
                    pull(1)

                # softmax -> p1, p2 ; previous tail's stats/LN fill PE here
                z_sb = spZ.tile([8, 3, C], F32, tag="z", name=f"z{ch}")
                nc.vector.tensor_tensor(z_sb[:], e_sb[:, :, 0, :],
                                        e_sb[:, :, 1, :], ALU.add)
                nc.vector.tensor_tensor(z_sb[:], z_sb[:], e_sb[:, :, 2, :],
                                        ALU.add)
                nc.vector.reciprocal_approx_fast(z_sb[:], z_sb[:])
                rz_sb = z_sb
                d_sb = spZ.tile([8, 3, 2, C], BF16, tag="dsb", name=f"d{ch}")
                if tailph:
                    tailph.pop(0)()
                for j in (1, 2):
                    nc.vector.tensor_tensor(d_sb[:, :, j - 1, :],
                                            e_sb[:, :, j, :], rz_sb[:],
                                            ALU.mult)
                pull(1)

                # o_i = v0 + p_i1*w1 + p_i2*w2 ; previous expert fills PE
                o_sb = ap.tile([128, 12, C], BF16, tag="o", name=f"o{ch}")
                for i in range(3):
                    pv = sp2.tile([128, 4, 2, C], BF16, tag="pv",
                                  name=f"pv{ch}_{i}")
                    for p in range(4):
                        bc = psB.tile([128, 2, C], F32, tag="bc",
                                      name=f"bc{ch}_{i}_{p}")
                        for j in (1, 2):
                            nc.tensor.matmul(
                                bc[:, j - 1, :], exps_sb[:, p, :],
                                d_sb[:, i, j - 1, :], start=True, stop=True)
                        nc.vector.tensor_tensor(pv[:, p, :, :], bc[:],
                                                wv_sb[:, :, p, :], ALU.mult)
                        pull(1)
                    if tailph:
                        tailph.pop(0)()
                    nc.vector.tensor_tensor(pv[:, :, 0, :], pv[:, :, 0, :],
                                            pv[:, :, 1, :], ALU.add)
                    nc.vector.tensor_tensor(o_sb[:, i * 4:(i + 1) * 4, :],
                                            pv[:, :, 0, :],
                                            v_sb[:, 0, :, :], ALU.add)
                return o_sb

            def make_tail(ch, o_sb):
                """Return tail phase closures for chunk ch (uses o_sb)."""
                waff_sb = ap.tile([128, 4, 512], BF16, tag="waff",
                                  name=f"waff{ch}")
                nc.sync.dma_start(waff_sb[:], waffA[ch])
                baff_sb = ap.tile([128, 4], F32, tag="baff", name=f"baff{ch}")
                nc.sync.dma_start(baff_sb[:], baffA[ch])
                state = {}

                def ph_w1o():
                    hp = psT.tile([128, 2, C], F32, tag="tail",
                                  name=f"hp{ch}")
                    for m2t in range(2):
                        for kip in range(12):
                            nc.tensor.matmul(
                                hp[:, m2t, :],
                                w1o_sb[:, kip, m2t * 128:(m2t + 1) * 128],
                                o_sb[:, kip, :],
                                start=(kip == 0), stop=(kip == 11))
                    hpre = ap.tile([128, 2, C], BF16, tag="hpre",
                                   name=f"hpre{ch}")
                    for m2t in range(2):
                        nc.scalar.activation(hpre[:, m2t, :], hp[:, m2t, :],
                                             AF.Relu,
                                             bias=beff_sb[:, m2t:m2t + 1])
                    state["hpre"] = hpre

                def ph_fus2():
                    hpre = state["hpre"]
                    st_ps = psst.tile([1, 2, C], F32, tag="st",
                                      name=f"st{ch}")
                    for ks in range(2):
                        nc.tensor.matmul(st_ps[:, 0, :], wmu_sb[:, ks:ks + 1],
                                         hpre[:, ks, :],
                                         start=(ks == 0), stop=(ks == 1))
                    y_sb = ap.tile([128, 4, C], BF16, tag="y", name=f"y{ch}")
                    ysq = sp3.tile([128, 4, C], BF16, tag="ysq",
                                   name=f"ysq{ch}")
                    for yp2 in range(2):
                        yp = psT.tile([128, 2, C], F32, tag="tail",
                                      name=f"yp{ch}_{yp2}")
                        for half in range(2):
                            m4 = 2 * yp2 + half
                            for ks in range(2):
                                nc.tensor.matmul(
                                    yp[:, half, :],
                                    w2_sb[:, ks, m4 * 128:(m4 + 1) * 128],
                                    hpre[:, ks, :], start=(ks == 0),
                                    stop=(ks == 1))
                        if zero_bias:
                            nc.scalar.activation(
                                y_sb[:, 2 * yp2:2 * yp2 + 2, :], yp[:],
                                AF.Identity)
                            nc.scalar.activation(
                                ysq[:, 2 * yp2:2 * yp2 + 2, :], yp[:],
                                AF.Square)
                        else:
                            for half in range(2):
                                m4 = 2 * yp2 + half
                                nc.scalar.activation(
                                    y_sb[:, m4, :], yp[:, half, :],
                                    AF.Identity, bias=b2_sb[:, m4:m4 + 1])
                                nc.scalar.activation(
                                    ysq[:, m4, :], yp[:, half, :], AF.Square,
                                    bias=b2_sb[:, m4:m4 + 1])
                    state["st_ps"] = st_ps
                    state["y"] = y_sb
                    state["ysq"] = ysq

                def ph_stats():
                    st_ps, y_sb, ysq = (state["st_ps"], state["y"],
                                        state["ysq"])
                    for p in range(4):
                        nc.tensor.matmul(st_ps[:, 1, :], o512_sb[:],
                                         ysq[:, p, :],
                                         start=(p == 0), stop=(p == 3))
                    mu_sb = spS.tile([1, C], F32, tag="musb",
                                     name=f"musb{ch}")
                    if zero_bias:
                        nc.vector.tensor_scalar_add(mu_sb[:], st_ps[:, 0, :],
                                                    0.0)
                    else:
                        nc.vector.tensor_scalar_add(mu_sb[:], st_ps[:, 0, :],
                                                    meanb2_sb[:])
                    musq = spS.tile([1, C], F32, tag="musq", name=f"musq{ch}")
                    nc.gpsimd.tensor_tensor(musq[:], mu_sb[:], mu_sb[:],
                                            ALU.mult)
                    var_sb = spS.tile([1, C], F32, tag="varsb",
                                      name=f"var{ch}")
                    nc.vector.tensor_tensor(var_sb[:], st_ps[:, 1, :],
                                            musq[:], ALU.subtract)
                    lnv = spS.tile([1, C], F32, tag="lnv", name=f"lnv{ch}")
                    nc.scalar.activation(lnv[:], var_sb[:], AF.Ln,
                                         bias=eps_sb[:])
                    rstd_sb = spS.tile([1, C], F32R, tag="rstd",
                                       name=f"rstd{ch}")
                    nc.scalar.activation(rstd_sb[:], lnv[:], AF.Exp,
                                         scale=-0.5)
                    murs = spS.tile([1, C], F32R, tag="murs",
                                    name=f"murs{ch}")
                    nc.gpsimd.tensor_tensor(murs[:], mu_sb[:], rstd_sb[:],
                                            ALU.mult)
                    bcp = psW.tile([128, 2, C], F32, tag="bcp",
                                   name=f"bcp{ch}")
                    nc.tensor.matmul(bcp[:, 0, :], ok1_sb[:], murs[:],
                                     start=True, stop=True)
                    nc.tensor.matmul(bcp[:, 1, :], ok1_sb[:], rstd_sb[:],
                                     start=True, stop=True)
                    mrex = mrexp.tile([128, 2, C], BF16, tag="mrex",
                                      name=f"mrex{ch}")
                    nc.scalar.activation(mrex[:], bcp[:], AF.Identity)
                    fused = ap.tile([128, 4, C], BF16, tag="fused",
                                    name=f"fused{ch}")
                    nc.vector.tensor_tensor(
                        fused[:], y_sb[:],
                        mrex[:, 1, None, :].to_broadcast((128, 4, C)),
                        ALU.mult)
                    nc.vector.tensor_tensor(
                        fused[:], fused[:],
                        mrex[:, 0, None, :].to_broadcast((128, 4, C)),
                        ALU.subtract)
                    state["fused"] = fused

                def ph_expert():
                    fused = state["fused"]
                    ot = sp3.tile([128, 4, C], BF16, tag="ot", name=f"ot{ch}")
                    is_mixed = (ch == NCH - 1)
                    for op2 in range(2):
                        op = psT.tile([128, 2, C], F32, tag="tail",
                                      name=f"op{ch}_{op2}")
                        for half in range(2):
                            m4 = 2 * op2 + half
                            for ks in range(4):
                                nc.tensor.matmul(
                                    op[:, half, :],
                                    waff_sb[:, ks, m4 * 128:(m4 + 1) * 128],
                                    fused[:, ks, :], start=(ks == 0),
                                    stop=(ks == 3))
                        if not is_mixed:
                            if zero_bias:
                                nc.scalar.activation(
                                    ot[:, 2 * op2:2 * op2 + 2, :], op[:],
                                    AF.Identity)
                            else:
                                for half in range(2):
                                    m4 = 2 * op2 + half
                                    nc.scalar.activation(
                                        ot[:, m4, :], op[:, half, :],
                                        AF.Identity,
                                        bias=baff_sb[:, m4:m4 + 1])
                        else:
                            opB = psB.tile([128, 2, C], F32, tag="bc",
                                           name=f"opB{ch}_{op2}")
                            for half in range(2):
                                m4 = 2 * op2 + half
                                for ks in range(4):
                                    nc.tensor.matmul(
                                        opB[:, half, :],
                                        waffB_sb[:, ks,
                                                 m4 * 128:(m4 + 1) * 128],
                                        fused[:, ks, :], start=(ks == 0),
                                        stop=(ks == 3))
                            oA = sp3.tile([128, 2, C], BF16, tag="oA",
                                          name=f"oA{ch}_{op2}")
                            oB = sp3.tile([128, 2, C], BF16, tag="oB",
                                          name=f"oB{ch}_{op2}")
                            if zero_bias:
                                nc.scalar.activation(oA[:], op[:],
                                                     AF.Identity)
                                nc.scalar.activation(oB[:], opB[:],
                                                     AF.Identity)
                            else:
                                for half in range(2):
                                    m4 = 2 * op2 + half
                                    nc.scalar.activation(
                                        oA[:, half, :], op[:, half, :],
                                        AF.Identity,
                                        bias=baff_sb[:, m4:m4 + 1])
                                    nc.scalar.activation(
                                        oB[:, half, :], opB[:, half, :],
                                        AF.Identity,
                                        bias=baffB_sb[:, m4:m4 + 1])
                            dAB = sp3.tile([128, 2, C], BF16, tag="dAB",
                                           name=f"dAB{ch}_{op2}")
                            nc.vector.tensor_tensor(dAB[:], oA[:], oB[:],
                                                    ALU.subtract)
                            nc.vector.tensor_tensor(
                                dAB[:], dAB[:],
                                maskex_sb[:, None, :].to_broadcast(
                                    (128, 2, C)), ALU.mult)
                            nc.vector.tensor_tensor(
                                ot[:, 2 * op2:2 * op2 + 2, :], oB[:], dAB[:],
                                ALU.add)
                    nc.sync.dma_start(outT[ch], ot[:])

                return [ph_w1o, ph_fus2, ph_stats, ph_expert]

            cur = front_alloc(0)
            first = True
            for t in range(3):
                for mp in range(6):
                    front_qkv_mp(cur, t, mp)
                    if first:
                        load_more_front()
                        first = False
                if t == 0:
                    load_tail_weights()
            prev_tail = []
            for ch in range(NCH):
                if ch + 1 < NCH:
                    nxt = front_alloc(ch + 1)
                    pieces = [lambda t=t, mp=mp, s=nxt: front_qkv_mp(s, t, mp)
                              for t in range(3) for mp in range(6)]
                else:
                    nxt, pieces = None, []
                o_sb = middle(ch, cur, pieces, prev_tail)
                for ph in prev_tail:
                    ph()
                prev_tail = make_tail(ch, o_sb)
                for p_ in pieces:
                    p_()
                cur = nxt
            # drain: last chunk's tail
            for ph in prev_tail:
                ph()

    nc.finalize()
    _NC_CACHE[key] = nc
    return nc


def _fp8_split(a):
    """Return (lo, hi) fp8e4m3 arrays with hi + lo ~= a."""
    hi = a.astype(FP8NP)
    lo = (a - hi.astype(np.float32)).astype(FP8NP)
    return lo, hi


def _prep_weights(inputs):
    in_proj_w = np.asarray(inputs["in_proj_w"], np.float32)
    in_proj_b = np.asarray(inputs["in_proj_b"], np.float32)
    out_proj_w = np.asarray(inputs["out_proj_w"], np.float32)
    out_proj_b = np.asarray(inputs["out_proj_b"], np.float32)
    fus_w1 = np.asarray(inputs["fus_w1"], np.float32)
    fus_b1 = np.asarray(inputs["fus_b1"], np.float32)
    fus_w2 = np.asarray(inputs["fus_w2"], np.float32)
    fus_b2 = np.asarray(inputs["fus_b2"], np.float32)
    ln_g = np.asarray(inputs["ln_g"], np.float32)
    ln_b = np.asarray(inputs["ln_b"], np.float32)
    aff_w = np.asarray(inputs["aff_w"], np.float32)
    aff_b = np.asarray(inputs["aff_b"], np.float32)

    scale = 1.0 / np.sqrt(np.float32(HD))
    W = in_proj_w.copy()
    W[:E] *= scale
    bq = in_proj_b.copy()
    bq[:E] *= scale
    # pre-scale W by 2^5 so the fp8 hi/lo planes stay out of e4m3's
    # subnormal range (W sigma ~0.02); undone by the Act copy scale 2^-5
    W *= 32.0

    # W.T is [512(k), 1536(m)]
    WT = np.ascontiguousarray(W.T)
    WT_lo, WT_hi = _fp8_split(WT)
    WT_lo = WT_lo.reshape(4, 128, 1536)
    WT_hi = WT_hi.reshape(4, 128, 1536)
    # hi-hi pairs: [128, kp, e, 1536] with (kp,e) -> k-subtile 2*kp+e
    whh_h = np.empty((128, 2, 2, 1536), FP8NP)
    for kp in range(2):
        for e_ in range(2):
            whh_h[:, kp, e_, :] = WT_hi[2 * kp + e_]
    # cross pairs: [128, p, {hi,lo}, 1536], paired with x (lo, hi)
    wcr_h = np.empty((128, 4, 2, 1536), FP8NP)
    for p in range(4):
        wcr_h[:, p, 0, :] = WT_hi[p]
        wcr_h[:, p, 1, :] = WT_lo[p]
    bqkv_h = np.ascontiguousarray(bq.reshape(12, 128).T)

    # fold out_proj into fus_w1; permute (h,d) -> (p, hl, d) to match v layout
    perm = np.empty(E, np.int64)
    for h in range(NH):
        for d in range(HD):
            perm[(h // 2) * 128 + (h % 2) * 64 + d] = h * HD + d
    blocks = []
    for i in range(3):
        blk = fus_w1[:, i * E:(i + 1) * E] @ out_proj_w  # [256, 512]
        blocks.append(blk[:, perm])
    W1o = np.concatenate(blocks, axis=1)  # [256, 1536]
    w1o_h = np.ascontiguousarray(
        W1o.T.reshape(12, 128, 256).transpose(1, 0, 2)).astype(BF16NP)
    # v bias folds into beff: o_i includes +bv for each i
    beff = fus_b1 + fus_w1 @ np.tile(out_proj_b, 3)
    beff_h = np.ascontiguousarray(beff.reshape(2, 128).T)

    w2_h = np.ascontiguousarray(
        fus_w2.T.reshape(2, 128, 512).transpose(1, 0, 2)).astype(BF16NP)
    wmu_h = np.ascontiguousarray(
        (fus_w2.mean(axis=0)).reshape(2, 128).T).astype(BF16NP)
    b2_h = np.ascontiguousarray(fus_b2.reshape(4, 128).T)

    # gamma/beta folded into expert weights/biases
    Wp = aff_w * ln_g[None, None, :]                   # [NE, 512, 512]
    bp = aff_w @ ln_b + aff_b                          # [NE, 512]
    waff_e = []
    for e_ in range(NE):
        A = np.ascontiguousarray(
            Wp[e_].T.reshape(4, 128, 512).transpose(1, 0, 2))
        waff_e.append(A.astype(BF16NP))
    baff_e = [np.ascontiguousarray(bp[e_].reshape(4, 128).T)
              for e_ in range(NE)]

    sel_h = np.zeros((128, 4, 8), np.float32)
    for r in range(128):
        for p in range(4):
            sel_h[r, p, 2 * p + r // 64] = 1.0
    exps_h = np.zeros((8, 4, 128), np.float32)
    for p in range(4):
        for c in range(128):
            exps_h[2 * p + c // 64, p, c] = 1.0

    zero_bias = (np.all(bq == 0) and np.all(in_proj_b[E:] == 0)
                 and np.all(fus_b2 == 0)
                 and all(np.all(b == 0) for b in baff_e))

    base = {
        "whh": whh_h, "wcr": wcr_h, "bqkv": bqkv_h,
        "w1o": w1o_h, "beff": beff_h, "w2": w2_h, "wmu": wmu_h, "b2": b2_h,
        "sel": sel_h.astype(BF16NP), "exps": exps_h.astype(BF16NP),
        "ones512": np.full((128, 1), 1.0 / E, np.float32).astype(BF16NP),
        "onesk1": np.ones((1, 128), np.float32),
        "meanb2": np.full((1, 1), fus_b2.mean(), np.float32),
    }
    return base, waff_e, baff_e, zero_bias, perm


def _pack_slots(labels):
    """Assign samples to 64 slots of 256; return per-core chunk plans."""
    order_ids = [np.nonzero(labels == e_)[0] for e_ in range(NE)]
    counts = [len(x) for x in order_ids]
    assert sum(counts) == B
    slots = []          # list of (ids[256], eA, eB, nA)
    leftovers = []      # (expert, ids)
    for e_ in range(NE):
        ids = order_ids[e_]
        nfull = len(ids) // C
        for s in range(nfull):
            slots.append((ids[s * C:(s + 1) * C], e_, e_, C))
        if len(ids) % C:
            leftovers.append((e_, ids[nfull * C:]))
    # pack leftovers into mixed slots (each must span <= 2 experts)
    mixed = []
    stream = []
    for e_, ids in leftovers:
        stream.append((e_, list(ids)))
    while stream:
        eA, idsA = stream[0]
        if len(idsA) >= C:
            mixed.append((np.array(idsA[:C]), eA, eA, C))
            stream[0] = (eA, idsA[C:])
            if not stream[0][1]:
                stream.pop(0)
            continue
        if len(stream) == 1:
            assert len(idsA) == 0 or len(idsA) == C, \
                f"unpackable remainder {len(idsA)}"
            if idsA:
                mixed.append((np.array(idsA), eA, eA, C))
            stream.pop(0)
            continue
        eB, idsB = stream[1]
        take = C - len(idsA)
        assert len(idsB) >= take, (
            f"slot would span 3 experts: {len(idsA)} + {len(idsB)} < {C}")
        ids = np.concatenate([idsA, idsB[:take]])
        mixed.append((ids, eA, eB, len(idsA)))
        stream.pop(0)
        stream[1 - 1] = (eB, idsB[take:])
        if not stream[0][1]:
            stream.pop(0)
    assert len(slots) + len(mixed) == B // C
    assert len(mixed) <= NCORES, f"{len(mixed)} mixed slots > {NCORES} cores"

    # per core: 8 slots, mixed slot (if any) at position 7
    plans = []
    si = 0
    for c in range(NCORES):
        mine = []
        if c < len(mixed):
            n_pure = NCH - 1
        else:
            n_pure = NCH
        mine = slots[si:si + n_pure]
        si += n_pure
        if c < len(mixed):
            mine = mine + [mixed[c]]
        plans.append(mine)
    assert si == len(slots)
    return plans


def kernel(**inputs):
    img = np.asarray(inputs["image_embeddings"], np.float32)
    txt = np.asarray(inputs["text_embeddings"], np.float32)
    kno = np.asarray(inputs["knowledge_embeddings"], np.float32)
    labels = np.asarray(inputs["affective_labels"]).astype(np.int64).ravel()
    assert img.shape == (B, E)

    base, waff_e, baff_e, zero_bias, perm = _prep_weights(inputs)
    plans = _pack_slots(labels)

    xs = np.stack([img, txt, kno])                     # [3, B, 512]

    in_maps = []
    for c in range(NCORES):
        plan = plans[c]
        gi = np.concatenate([p[0] for p in plan])      # [2048]
        xg = xs[:, gi, :].transpose(0, 2, 1)           # [3, 512, R]
        xg = xg.reshape(3, 4, 128, NCH, C)             # [t, p, r, ch, c]
        x_hi = xg.astype(FP8NP)
        x_lo = (xg - x_hi.astype(np.float32)).astype(FP8NP)
        xhl_h = np.empty((NCH, 128, 3, 4, 2, C), FP8NP)
        xhl_h[:, :, :, :, 0, :] = x_lo.transpose(3, 2, 0, 1, 4)
        xhl_h[:, :, :, :, 1, :] = x_hi.transpose(3, 2, 0, 1, 4)

        waffA_h = np.stack([waff_e[p[1]] for p in plan])     # [NCH,128,4,512]
        baffA_h = np.stack([baff_e[p[1]] for p in plan])     # [NCH,128,4]
        eB = plan[NCH - 1][2]
        waffB_h = waff_e[eB]
        baffB_h = baff_e[eB]
        nA = plan[NCH - 1][3]
        mask = np.zeros((128, C), np.float32)
        mask[:, :nA] = 1.0

        m = dict(base)
        m["xhl"] = xhl_h
        m["waffA"] = waffA_h
        m["baffA"] = baffA_h
        m["waffB"] = waffB_h
        m["baffB"] = baffB_h
        m["maskex"] = mask.astype(BF16NP)
        in_maps.append(m)

    nc = _build_program(zero_bias)
    res = run_bass_kernel_spmd(nc, in_maps, core_ids=list(range(NCORES)))
    global LAST_RESULTS, LAST_NC
    LAST_RESULTS = res
    LAST_NC = nc

    out_full = np.zeros((B, E), np.float32)
    for c in range(NCORES):
        oT = res.results[c]["outT"]                    # [NCH,128,4,C] bf16
        oT = np.asarray(oT, dtype=np.float32)
        plan = plans[c]
        for ch in range(NCH):
            ids = plan[ch][0]
            # [128, 4, C] -> [C, 512] with feature f = 128*p + r
            blk = oT[ch].transpose(2, 1, 0).reshape(C, 4 * 128)
            out_full[ids] = blk
    return out_full


if __name__ == "__main__":
    rng = np.random.default_rng(0)
    fake = {
        "image_embeddings": rng.standard_normal((B, E)).astype(np.float32),
        "text_embeddings": rng.standard_normal((B, E)).astype(np.float32),
        "knowledge_embeddings": rng.standard_normal((B, E)).astype(np.float32),
        "affective_labels": rng.integers(0, NE, B),
        "in_proj_w": (rng.standard_normal((3 * E, E)) * 0.02).astype(np.float32),
        "in_proj_b": np.zeros(3 * E, np.float32),
        "out_proj_w": (rng.standard_normal((E, E)) * 0.02).astype(np.float32),
        "out_proj_b": np.zeros(E, np.float32),
        "fus_w1": (rng.standard_normal((H, 3 * E)) * 0.02).astype(np.float32),
        "fus_b1": np.zeros(H, np.float32),
        "fus_w2": (rng.standard_normal((E, H)) * 0.02).astype(np.float32),
        "fus_b2": np.zeros(E, np.float32),
        "ln_g": np.ones(E, np.float32),
        "ln_b": np.zeros(E, np.float32),
        "aff_w": (rng.standard_normal((NE, E, E)) * 0.02).astype(np.float32),
        "aff_b": np.zeros((NE, E), np.float32),
    }
    out = kernel(**fake)
    print("kernel ran, out:", out.shape, out.dtype, np.abs(out).max())


# revision 4
# speedup vs baseline: 1.0729x; 1.0174x over previous
"""Trainium2 Bass kernel v2 for nn_MultiModalFusion (moe_routing).

Strategy (v2, derived from the TimelineSim cost model):
- Data parallel over 8 cores. Host sorts samples by expert into 64 slots of
  exactly 256 (zero padding waste); at most 3 slots mix two experts — those go
  to chunk slot 7 of cores 0..2, where the program computes both experts and
  mask-blends.
- QKV projection in fp8e4m3 with host-side hi/lo error compensation
  (x = x_hi + x_lo, W = W_hi + W_lo; the three significant cross products are
  packed into DoubleRow matmuls at 0.5 cyc/row -> 3C per (token, m-tile)
  instead of fp32r's 4C, with ~bf16-level accuracy).
- All other matmul moving operands bf16 (1 cyc/row at any width).
- Attention probs via the identity o_i = v0 + p_i1*(v1-v0) + p_i2*(v2-v0):
  only 2 of 3 probability broadcasts per query (24 instead of 36 broadcast
  matmul columns), probs in bf16.
- LayerNorm: mean folded into an extra fus2 output row (host-precomputed
  column-mean weight vector), rstd = exp(-0.5*ln(var+eps)) on the scalar
  engine — ln/exp/identity/relu/square share one activation table set, so no
  LoadActFuncSet thrash. gamma/beta folded into expert weights on host.
- PSUM pair tiles (2 x 256 f32 = one 2KB bank) let one Activation instruction
  evacuate two matmul outputs.
- DVE kept in 2x bf16 mode where possible; part of the prob*value work is
  offloaded to the idle GPSIMD (Pool) engine.
"""

import numpy as np
import ml_dtypes

import concourse.bass as bass
import concourse.mybir as mybir
import concourse.tile as tile
from concourse import bacc
from concourse.bass_utils import run_bass_kernel_spmd

E = 512
H = 256
NH = 8
HD = 64
NE = 4
B = 16384
NCORES = 8
C = 256              # chunk columns
NCH = 8              # chunks per core
R = NE * 512         # 2048 columns per core

LAST_RESULTS = None
LAST_NC = None

F32 = mybir.dt.float32
F32R = mybir.dt.float32r
BF16 = mybir.dt.bfloat16
FP8 = mybir.dt.float8e4
AF = mybir.ActivationFunctionType
ALU = mybir.AluOpType
MM = mybir.MatmulPerfMode

FP8NP = ml_dtypes.float8_e4m3
BF16NP = ml_dtypes.bfloat16

_NC_CACHE = {}


def _build_program(zero_bias, debug=False):
    key = (bool(zero_bias), bool(debug))
    if key in _NC_CACHE:
        return _NC_CACHE[key]
    nc = bacc.Bacc("TRN2")

    # ---------------- DRAM I/O ----------------
    # x hi/lo fp8: [ch, 128, t, p, plane(0=lo,1=hi), C]
    xhl = nc.dram_tensor("xhl", [NCH, 128, 3, 4, 2, C], FP8,
                         kind="ExternalInput")
    # QKV weights fp8: hi-hi k-pairs and (hi,lo) cross pairs
    whh = nc.dram_tensor("whh", [128, 2, 2, 1536], FP8, kind="ExternalInput")
    wcr = nc.dram_tensor("wcr", [128, 4, 2, 1536], FP8, kind="ExternalInput")
    bqkv = nc.dram_tensor("bqkv", [128, 12], F32, kind="ExternalInput")
    w1o = nc.dram_tensor("w1o", [128, 12, 256], BF16, kind="ExternalInput")
    beff = nc.dram_tensor("beff", [128, 2], F32, kind="ExternalInput")
    w2 = nc.dram_tensor("w2", [128, 2, 512], BF16, kind="ExternalInput")
    wmu = nc.dram_tensor("wmu", [128, 2], BF16, kind="ExternalInput")
    b2 = nc.dram_tensor("b2", [128, 4], F32, kind="ExternalInput")
    # per-chunk expert weights, host-selected: A for all chunks, B for ch 7
    waffA = nc.dram_tensor("waffA", [NCH, 128, 4, 512], BF16,
                           kind="ExternalInput")
    waffB = nc.dram_tensor("waffB", [128, 4, 512], BF16,
                           kind="ExternalInput")
    baffA = nc.dram_tensor("baffA", [NCH, 128, 4], F32,
                           kind="ExternalInput")
    baffB = nc.dram_tensor("baffB", [128, 4], F32, kind="ExternalInput")
    maskex = nc.dram_tensor("maskex", [128, C], BF16, kind="ExternalInput")
    meanb2 = nc.dram_tensor("meanb2", [1, 1], F32, kind="ExternalInput")
    sel = nc.dram_tensor("sel", [128, 4, 8], BF16, kind="ExternalInput")
    exps = nc.dram_tensor("exps", [8, 4, 128], BF16, kind="ExternalInput")
    ones512 = nc.dram_tensor("ones512", [128, 1], BF16, kind="ExternalInput")
    onesk1 = nc.dram_tensor("onesk1", [1, 128], F32R, kind="ExternalInput")
    outT = nc.dram_tensor("outT", [NCH, 128, 4, C], BF16,
                          kind="ExternalOutput")
    if debug:
        dbg_q = nc.dram_tensor("dbg_q", [128, 3, 4, C], F32,
                               kind="ExternalOutput")
        dbg_k = nc.dram_tensor("dbg_k", [128, 3, 4, C], F32,
                               kind="ExternalOutput")
        dbg_v = nc.dram_tensor("dbg_v", [128, 3, 4, C], F32,
                               kind="ExternalOutput")
        dbg_e = nc.dram_tensor("dbg_e", [8, 3, 3, C], F32,
                               kind="ExternalOutput")
        dbg_d = nc.dram_tensor("dbg_d", [8, 3, 2, C], F32,
                               kind="ExternalOutput")
        dbg_o = nc.dram_tensor("dbg_o", [128, 12, C], F32,
                               kind="ExternalOutput")
        dbg_h = nc.dram_tensor("dbg_h", [128, 2, C], F32,
                               kind="ExternalOutput")
        dbg_y = nc.dram_tensor("dbg_y", [128, 4, C], F32,
                               kind="ExternalOutput")
        dbg_mu = nc.dram_tensor("dbg_mu", [1, C], F32,
                                kind="ExternalOutput")
        dbg_var = nc.dram_tensor("dbg_var", [1, C], F32,
                                 kind="ExternalOutput")
        dbg_rstd = nc.dram_tensor("dbg_rstd", [1, C], F32,
                                  kind="ExternalOutput")
        dbg_fused = nc.dram_tensor("dbg_fused", [128, 4, C], F32,
                                   kind="ExternalOutput")

    with tile.TileContext(nc) as tc:
        with tc.tile_pool(name="wp", bufs=1) as wp, \
             tc.tile_pool(name="xp", bufs=2) as xp, \
             tc.tile_pool(name="qkvp", bufs=2) as qkvp, \
             tc.tile_pool(name="ap", bufs=2) as ap, \
             tc.tile_pool(name="spM", bufs=2) as spM, \
             tc.tile_pool(name="spZ", bufs=2) as spZ, \
             tc.tile_pool(name="spS", bufs=2) as spS, \
             tc.tile_pool(name="mrexp", bufs=2) as mrexp, \
             tc.tile_pool(name="sp2", bufs=3) as sp2, \
             tc.tile_pool(name="sp3", bufs=2) as sp3, \
             tc.tile_pool(name="psQ", bufs=2, space="PSUM") as psQ, \
             tc.tile_pool(name="psB", bufs=3, space="PSUM") as psB, \
             tc.tile_pool(name="psT", bufs=2, space="PSUM") as psT, \
             tc.tile_pool(name="psst", bufs=1, space="PSUM") as psst:

            # ---------------- persistent weights ----------------
            nc.scalar.add_instruction(mybir.InstLoadActFuncSet(
                name=nc.get_next_instruction_name(), ins=[], outs=[],
                act_func_set_id=6))
            whh_sb = wp.tile([128, 2, 2, 1536], FP8)
            wcr_sb = wp.tile([128, 4, 2, 1536], FP8)
            # first halves only — QKV m-tiles 0..5 can start ASAP
            nc.sync.dma_start(whh_sb[:, :, :, 0:768], whh[:, :, :, 0:768])
            nc.sync.dma_start(wcr_sb[:, :, :, 0:768], wcr[:, :, :, 0:768])
            bqkv_sb = wp.tile([128, 12], F32)
            sel_sb = wp.tile([128, 4, 8], BF16)
            exps_sb = wp.tile([8, 4, 128], BF16)
            eps_sb = wp.tile([1, 1], F32)

            def load_more_front():
                nc.sync.dma_start(whh_sb[:, :, :, 768:], whh[:, :, :, 768:])
                nc.sync.dma_start(wcr_sb[:, :, :, 768:], wcr[:, :, :, 768:])
                nc.sync.dma_start(bqkv_sb[:], bqkv[:])
                nc.sync.dma_start(sel_sb[:], sel[:])
                nc.sync.dma_start(exps_sb[:], exps[:])
                nc.vector.memset(eps_sb[:], 1e-5)
            w1o_sb = wp.tile([128, 12, 256], BF16)
            beff_sb = wp.tile([128, 2], F32)
            w2_sb = wp.tile([128, 2, 512], BF16)
            wmu_sb = wp.tile([128, 2], BF16)
            b2_sb = wp.tile([128, 4], F32)
            o512_sb = wp.tile([128, 1], BF16)
            ok1_sb = wp.tile([1, 128], F32R)
            waffB_sb = wp.tile([128, 4, 512], BF16)
            baffB_sb = wp.tile([128, 4], F32)
            maskex_sb = wp.tile([128, C], BF16)
            meanb2_sb = wp.tile([1, 1], F32)

            def load_tail_weights():
                nc.sync.dma_start(w1o_sb[:], w1o[:])
                nc.sync.dma_start(beff_sb[:], beff[:])
                nc.sync.dma_start(w2_sb[:], w2[:])
                nc.sync.dma_start(wmu_sb[:], wmu[:])
                nc.sync.dma_start(b2_sb[:], b2[:])
                nc.sync.dma_start(o512_sb[:], ones512[:])
                nc.sync.dma_start(ok1_sb[:], onesk1[:])
                nc.sync.dma_start(waffB_sb[:], waffB[:])
                nc.sync.dma_start(baffB_sb[:], baffB[:])
                nc.sync.dma_start(maskex_sb[:], maskex[:])
                nc.sync.dma_start(meanb2_sb[:], meanb2[:])

            def front_alloc(ch):
                """x + expert-weight loads, qkv tile allocation, chunk ch."""
                x_sb = xp.tile([128, 3, 4, 2, C], FP8, tag="x", name=f"x{ch}")
                for t in range(3):
                    nc.sync.dma_start(x_sb[:, t], xhl[ch, :, t])
                waff_sb = ap.tile([128, 4, 512], BF16, tag="waff",
                                  name=f"waff{ch}")
                nc.sync.dma_start(waff_sb[:], waffA[ch])
                baff_sb = ap.tile([128, 4], F32, tag="baff", name=f"baff{ch}")
                nc.sync.dma_start(baff_sb[:], baffA[ch])
                q_sb = qkvp.tile([128, 3, 4, C], BF16, tag="q", name=f"q{ch}")
                k_sb = qkvp.tile([128, 3, 4, C], BF16, tag="k", name=f"k{ch}")
                v_sb = qkvp.tile([128, 3, 4, C], BF16, tag="v", name=f"v{ch}")
                return {"ch": ch, "x": x_sb, "q": q_sb, "k": k_sb, "v": v_sb,
                        "waff": waff_sb, "baff": baff_sb}

            def front_qkv_mp(st, t, mp):
                """QKV m-pair mp (12 DR matmuls + 1 pair copy) — atomic."""
                ch = st["ch"]
                x_sb = st["x"]
                pair = psQ.tile([128, 2, C], F32, tag="qkv",
                                name=f"qkv{ch}_{t}_{mp}")
                for half in range(2):
                    m = 2 * mp + half
                    mc = slice(m * 128, (m + 1) * 128)
                    for kp in range(2):
                        nc.tensor.matmul(
                            pair[:, half, :],
                            whh_sb[:, kp, :, mc],
                            x_sb[:, t, 2 * kp:2 * kp + 2, 1, :],
                            start=(kp == 0), stop=False,
                            perf_mode=MM.DoubleRow)
                    for p in range(4):
                        nc.tensor.matmul(
                            pair[:, half, :],
                            wcr_sb[:, p, :, mc],
                            x_sb[:, t, p, :, :],
                            start=False, stop=(p == 3),
                            perf_mode=MM.DoubleRow)
                dst = (st["q"], st["k"], st["v"])[mp // 2]
                pp = mp % 2
                if zero_bias:
                    nc.scalar.activation(
                        dst[:, t, 2 * pp:2 * pp + 2, :], pair[:],
                        AF.Identity, scale=1.0 / 32.0)
                else:
                    for half in range(2):
                        m = 2 * mp + half
                        nc.scalar.activation(
                            dst[:, t, 2 * pp + half, :], pair[:, half, :],
                            AF.Identity, bias=bqkv_sb[:, m:m + 1],
                            scale=1.0 / 32.0)

            def middle(ch, st, pieces, tailph):
                """Scores/softmax/o for chunk ch; interleaves previous
                chunk's tail phases (tailph) and next-chunk QKV pieces."""
                def pull(n=1):
                    for _ in range(n):
                        if pieces:
                            pieces.pop(0)()
                q_sb, k_sb, v_sb = st["q"], st["k"], st["v"]

                # w_j = v_j - v_0 early on the idle Pool engine
                wv_sb = spM.tile([128, 2, 4, C], BF16, tag="wv",
                                 name=f"wv{ch}")
                for j in (1, 2):
                    nc.gpsimd.tensor_tensor(wv_sb[:, j - 1, :, :],
                                            v_sb[:, j, :, :],
                                            v_sb[:, 0, :, :],
                                            ALU.subtract)

                # scores + exp; tail W1o/fus2 phases fill the DVE-bound spots
                e_sb = spM.tile([8, 3, 3, C], BF16, tag="esb", name=f"e{ch}")
                for i in range(3):
                    for j in range(3):
                        prod = sp2.tile([128, 4, C], BF16, tag="prod",
                                        name=f"prod{ch}_{i}_{j}")
                        nc.vector.tensor_tensor(
                            prod[:], q_sb[:, i, :, :], k_sb[:, j, :, :],
                            ALU.mult)
                        s_ps = psB.tile([8, C], F32, tag="bc",
                                        name=f"s{ch}_{i}_{j}")
                        for p in range(4):
                            nc.tensor.matmul(
                                s_ps[:], sel_sb[:, p, :], prod[:, p, :],
                                start=(p == 0), stop=(p == 3))
                        nc.scalar.activation(e_sb[:, i, j, :], s_ps[:],
                                             AF.Exp)
                    if tailph:
                        tailph.pop(0)()
                    pull(1)

                # softmax -> p1, p2 ; previous tail's stats/LN fill PE here
                z_sb = spZ.tile([8, 3, C], F32, tag="z", name=f"z{ch}")
                nc.vector.tensor_tensor(z_sb[:], e_sb[:, :, 0, :],
                                        e_sb[:, :, 1, :], ALU.add)
                nc.vector.tensor_tensor(z_sb[:], z_sb[:], e_sb[:, :, 2, :],
                                        ALU.add)
                nc.vector.reciprocal_approx_fast(z_sb[:], z_sb[:])
                rz_sb = z_sb
                d_sb = spZ.tile([8, 3, 2, C], BF16, tag="dsb", name=f"d{ch}")
                for j in (1, 2):
                    nc.vector.tensor_tensor(d_sb[:, :, j - 1, :],
                                            e_sb[:, :, j, :], rz_sb[:],
                                            ALU.mult)
                pull(1)

                # o_i = v0 + p_i1*w1 + p_i2*w2 ; previous expert fills PE
                o_sb = ap.tile([128, 12, C], BF16, tag="o", name=f"o{ch}")
                for i in range(3):
                    pv = sp2.tile([128, 4, 2, C], BF16, tag="pv",
                                  name=f"pv{ch}_{i}")
                    for p in range(4):
                        bc = psB.tile([128, 2, C], F32, tag="bc",
                                      name=f"bc{ch}_{i}_{p}")
                        for j in (1, 2):
                            nc.tensor.matmul(
                                bc[:, j - 1, :], exps_sb[:, p, :],
                                d_sb[:, i, j - 1, :], start=True, stop=True)
                        nc.vector.tensor_tensor(pv[:, p, :, :], bc[:],
                                                wv_sb[:, :, p, :], ALU.mult)
                        pull(1)
                    if tailph:
                        tailph.pop(0)()
                    nc.vector.tensor_tensor(pv[:, :, 0, :], pv[:, :, 0, :],
                                            pv[:, :, 1, :], ALU.add)
                    nc.vector.tensor_tensor(o_sb[:, i * 4:(i + 1) * 4, :],
                                            pv[:, :, 0, :],
                                            v_sb[:, 0, :, :], ALU.add)
                    if tailph:
                        tailph.pop(0)()
                return o_sb

            def make_tail(ch, o_sb):
                """Return tail phase closures for chunk ch (uses o_sb)."""
                waff_sb = ap.tile([128, 4, 512], BF16, tag="waff",
                                  name=f"waff{ch}")
                nc.sync.dma_start(waff_sb[:], waffA[ch])
                baff_sb = ap.tile([128, 4], F32, tag="baff", name=f"baff{ch}")
                nc.sync.dma_start(baff_sb[:], baffA[ch])
                state = {}

                def ph_w1o():
                    hp = psT.tile([128, 2, C], F32, tag="tail",
                                  name=f"hp{ch}")
                    for m2t in range(2):
                        for kip in range(12):
                            nc.tensor.matmul(
                                hp[:, m2t, :],
                                w1o_sb[:, kip, m2t * 128:(m2t + 1) * 128],
                                o_sb[:, kip, :],
                                start=(kip == 0), stop=(kip == 11))
                    hpre = ap.tile([128, 2, C], BF16, tag="hpre",
                                   name=f"hpre{ch}")
                    for m2t in range(2):
                        nc.scalar.activation(hpre[:, m2t, :], hp[:, m2t, :],
                                             AF.Relu,
                                             bias=beff_sb[:, m2t:m2t + 1])
                    state["hpre"] = hpre

                def ph_fus2():
                    hpre = state["hpre"]
                    st_ps = psst.tile([1, 2, C], F32, tag="st",
                                      name=f"st{ch}")
                    for ks in range(2):
                        nc.tensor.matmul(st_ps[:, 0, :], wmu_sb[:, ks:ks + 1],
                                         hpre[:, ks, :],
                                         start=(ks == 0), stop=(ks == 1))
                    y_sb = ap.tile([128, 4, C], BF16, tag="y", name=f"y{ch}")
                    ysq = sp3.tile([128, 4, C], BF16, tag="ysq",
                                   name=f"ysq{ch}")
                    for yp2 in range(2):
                        yp = psT.tile([128, 2, C], F32, tag="tail",
                                      name=f"yp{ch}_{yp2}")
                        for half in range(2):
                            m4 = 2 * yp2 + half
                            for ks in range(2):
                                nc.tensor.matmul(
                                    yp[:, half, :],
                                    w2_sb[:, ks, m4 * 128:(m4 + 1) * 128],
                                    hpre[:, ks, :], start=(ks == 0),
                                    stop=(ks == 1))
                        if zero_bias:
                            nc.scalar.activation(
                                y_sb[:, 2 * yp2:2 * yp2 + 2, :], yp[:],
                                AF.Identity)
                            nc.scalar.activation(
                                ysq[:, 2 * yp2:2 * yp2 + 2, :], yp[:],
                                AF.Square)
                        else:
                            for half in range(2):
                                m4 = 2 * yp2 + half
                                nc.scalar.activation(
                                    y_sb[:, m4, :], yp[:, half, :],
                                    AF.Identity, bias=b2_sb[:, m4:m4 + 1])
                                nc.scalar.activation(
                                    ysq[:, m4, :], yp[:, half, :], AF.Square,
                                    bias=b2_sb[:, m4:m4 + 1])
                    state["st_ps"] = st_ps
                    state["y"] = y_sb
                    state["ysq"] = ysq

                def ph_stats():
                    st_ps, y_sb, ysq = (state["st_ps"], state["y"],
                                        state["ysq"])
                    for p in range(4):
                        nc.tensor.matmul(st_ps[:, 1, :], o512_sb[:],
                                         ysq[:, p, :],
                                         start=(p == 0), stop=(p == 3))
                    mu_sb = spS.tile([1, C], F32, tag="musb",
                                     name=f"musb{ch}")
                    if zero_bias:
                        nc.vector.tensor_scalar_add(mu_sb[:], st_ps[:, 0, :],
                                                    0.0)
                    else:
                        nc.vector.tensor_scalar_add(mu_sb[:], st_ps[:, 0, :],
                                                    meanb2_sb[:])
                    musq = spS.tile([1, C], F32, tag="musq", name=f"musq{ch}")
                    nc.gpsimd.tensor_tensor(musq[:], mu_sb[:], mu_sb[:],
                                            ALU.mult)
                    var_sb = spS.tile([1, C], F32, tag="varsb",
                                      name=f"var{ch}")
                    nc.vector.tensor_tensor(var_sb[:], st_ps[:, 1, :],
                                            musq[:], ALU.subtract)
                    lnv = spS.tile([1, C], F32, tag="lnv", name=f"lnv{ch}")
                    nc.scalar.activation(lnv[:], var_sb[:], AF.Ln,
                                         bias=eps_sb[:])
                    rstd_sb = spS.tile([1, C], F32R, tag="rstd",
                                       name=f"rstd{ch}")
                    nc.scalar.activation(rstd_sb[:], lnv[:], AF.Exp,
                                         scale=-0.5)
                    murs = spS.tile([1, C], F32R, tag="murs",
                                    name=f"murs{ch}")
                    nc.gpsimd.tensor_tensor(murs[:], mu_sb[:], rstd_sb[:],
                                            ALU.mult)
                    bcp = psT.tile([128, 2, C], F32, tag="tail",
                                   name=f"bcp{ch}")
                    nc.tensor.matmul(bcp[:, 0, :], ok1_sb[:], murs[:],
                                     start=True, stop=True)
                    nc.tensor.matmul(bcp[:, 1, :], ok1_sb[:], rstd_sb[:],
                                     start=True, stop=True)
                    mrex = mrexp.tile([128, 2, C], BF16, tag="mrex",
                                      name=f"mrex{ch}")
                    nc.scalar.activation(mrex[:], bcp[:], AF.Identity)
                    fused = ap.tile([128, 4, C], BF16, tag="fused",
                                    name=f"fused{ch}")
                    nc.vector.tensor_tensor(
                        fused[:], y_sb[:],
                        mrex[:, 1, None, :].to_broadcast((128, 4, C)),
                        ALU.mult)
                    nc.vector.tensor_tensor(
                        fused[:], fused[:],
                        mrex[:, 0, None, :].to_broadcast((128, 4, C)),
                        ALU.subtract)
                    state["fused"] = fused

                def ph_expert():
                    fused = state["fused"]
                    ot = sp3.tile([128, 4, C], BF16, tag="ot", name=f"ot{ch}")
                    is_mixed = (ch == NCH - 1)
                    for op2 in range(2):
                        op = psT.tile([128, 2, C], F32, tag="tail",
                                      name=f"op{ch}_{op2}")
                        for half in range(2):
                            m4 = 2 * op2 + half
                            for ks in range(4):
                                nc.tensor.matmul(
                                    op[:, half, :],
                                    waff_sb[:, ks, m4 * 128:(m4 + 1) * 128],
                                    fused[:, ks, :], start=(ks == 0),
                                    stop=(ks == 3))
                        if not is_mixed:
                            if zero_bias:
                                nc.scalar.activation(
                                    ot[:, 2 * op2:2 * op2 + 2, :], op[:],
                                    AF.Identity)
                            else:
                                for half in range(2):
                                    m4 = 2 * op2 + half
                                    nc.scalar.activation(
                                        ot[:, m4, :], op[:, half, :],
                                        AF.Identity,
                                        bias=baff_sb[:, m4:m4 + 1])
                        else:
                            opB = psB.tile([128, 2, C], F32, tag="bc",
                                           name=f"opB{ch}_{op2}")
                            for half in range(2):
                                m4 = 2 * op2 + half
                                for ks in range(4):
                                    nc.tensor.matmul(
                                        opB[:, half, :],
                                        waffB_sb[:, ks,
                                                 m4 * 128:(m4 + 1) * 128],
                                        fused[:, ks, :], start=(ks == 0),
                                        stop=(ks == 3))
                            oA = sp3.tile([128, 2, C], BF16, tag="oA",
                                          name=f"oA{ch}_{op2}")
                            oB = sp3.tile([128, 2, C], BF16, tag="oB",
                                          name=f"oB{ch}_{op2}")
                            if zero_bias:
                                nc.scalar.activation(oA[:], op[:],
                                                     AF.Identity)
                                nc.scalar.activation(oB[:], opB[:],
                                                     AF.Identity)
                            else:
                                for half in range(2):
                                    m4 = 2 * op2 + half
                                    nc.scalar.activation(
                                        oA[:, half, :], op[:, half, :],
                                        AF.Identity,
                                        bias=baff_sb[:, m4:m4 + 1])
                                    nc.scalar.activation(
                                        oB[:, half, :], opB[:, half, :],
                                        AF.Identity,
                                        bias=baffB_sb[:, m4:m4 + 1])
                            dAB = sp3.tile([128, 2, C], BF16, tag="dAB",
                                           name=f"dAB{ch}_{op2}")
                            nc.vector.tensor_tensor(dAB[:], oA[:], oB[:],
                                                    ALU.subtract)
                            nc.vector.tensor_tensor(
                                dAB[:], dAB[:],
                                maskex_sb[:, None, :].to_broadcast(
                                    (128, 2, C)), ALU.mult)
                            nc.vector.tensor_tensor(
                                ot[:, 2 * op2:2 * op2 + 2, :], oB[:], dAB[:],
                                ALU.add)
                    nc.sync.dma_start(outT[ch], ot[:])

                return [lambda: None, lambda: None, ph_w1o, ph_fus2,
                        ph_stats, ph_expert]

            cur = front_alloc(0)
            first = True
            for t in range(3):
                for mp in range(6):
                    front_qkv_mp(cur, t, mp)
                    if first:
                        load_more_front()
                        first = False
                if t == 0:
                    load_tail_weights()
            prev_tail = []
            for ch in range(NCH):
                if ch + 1 < NCH:
                    nxt = front_alloc(ch + 1)
                    pieces = [lambda t=t, mp=mp, s=nxt: front_qkv_mp(s, t, mp)
                              for t in range(3) for mp in range(6)]
                else:
                    nxt, pieces = None, []
                o_sb = middle(ch, cur, pieces, prev_tail)
                for ph in prev_tail:
                    ph()
                prev_tail = make_tail(ch, o_sb)
                for p_ in pieces:
                    p_()
                cur = nxt
            # drain: last chunk's tail
            for ph in prev_tail:
                ph()

    nc.finalize()
    _NC_CACHE[key] = nc
    return nc


def _fp8_split(a):
    """Return (lo, hi) fp8e4m3 arrays with hi + lo ~= a."""
    hi = a.astype(FP8NP)
    lo = (a - hi.astype(np.float32)).astype(FP8NP)
    return lo, hi


def _prep_weights(inputs):
    in_proj_w = np.asarray(inputs["in_proj_w"], np.float32)
    in_proj_b = np.asarray(inputs["in_proj_b"], np.float32)
    out_proj_w = np.asarray(inputs["out_proj_w"], np.float32)
    out_proj_b = np.asarray(inputs["out_proj_b"], np.float32)
    fus_w1 = np.asarray(inputs["fus_w1"], np.float32)
    fus_b1 = np.asarray(inputs["fus_b1"], np.float32)
    fus_w2 = np.asarray(inputs["fus_w2"], np.float32)
    fus_b2 = np.asarray(inputs["fus_b2"], np.float32)
    ln_g = np.asarray(inputs["ln_g"], np.float32)
    ln_b = np.asarray(inputs["ln_b"], np.float32)
    aff_w = np.asarray(inputs["aff_w"], np.float32)
    aff_b = np.asarray(inputs["aff_b"], np.float32)

    scale = 1.0 / np.sqrt(np.float32(HD))
    W = in_proj_w.copy()
    W[:E] *= scale
    bq = in_proj_b.copy()
    bq[:E] *= scale
    # pre-scale W by 2^5 so the fp8 hi/lo planes stay out of e4m3's
    # subnormal range (W sigma ~0.02); undone by the Act copy scale 2^-5
    W *= 32.0

    # W.T is [512(k), 1536(m)]
    WT = np.ascontiguousarray(W.T)
    WT_lo, WT_hi = _fp8_split(WT)
    WT_lo = WT_lo.reshape(4, 128, 1536)
    WT_hi = WT_hi.reshape(4, 128, 1536)
    # hi-hi pairs: [128, kp, e, 1536] with (kp,e) -> k-subtile 2*kp+e
    whh_h = np.empty((128, 2, 2, 1536), FP8NP)
    for kp in range(2):
        for e_ in range(2):
            whh_h[:, kp, e_, :] = WT_hi[2 * kp + e_]
    # cross pairs: [128, p, {hi,lo}, 1536], paired with x (lo, hi)
    wcr_h = np.empty((128, 4, 2, 1536), FP8NP)
    for p in range(4):
        wcr_h[:, p, 0, :] = WT_hi[p]
        wcr_h[:, p, 1, :] = WT_lo[p]
    bqkv_h = np.ascontiguousarray(bq.reshape(12, 128).T)

    # fold out_proj into fus_w1; permute (h,d) -> (p, hl, d) to match v layout
    perm = np.empty(E, np.int64)
    for h in range(NH):
        for d in range(HD):
            perm[(h // 2) * 128 + (h % 2) * 64 + d] = h * HD + d
    blocks = []
    for i in range(3):
        blk = fus_w1[:, i * E:(i + 1) * E] @ out_proj_w  # [256, 512]
        blocks.append(blk[:, perm])
    W1o = np.concatenate(blocks, axis=1)  # [256, 1536]
    w1o_h = np.ascontiguousarray(
        W1o.T.reshape(12, 128, 256).transpose(1, 0, 2)).astype(BF16NP)
    # v bias folds into beff: o_i includes +bv for each i
    beff = fus_b1 + fus_w1 @ np.tile(out_proj_b, 3)
    beff_h = np.ascontiguousarray(beff.reshape(2, 128).T)

    w2_h = np.ascontiguousarray(
        fus_w2.T.reshape(2, 128, 512).transpose(1, 0, 2)).astype(BF16NP)
    wmu_h = np.ascontiguousarray(
        (fus_w2.mean(axis=0)).reshape(2, 128).T).astype(BF16NP)
    b2_h = np.ascontiguousarray(fus_b2.reshape(4, 128).T)

    # gamma/beta folded into expert weights/biases
    Wp = aff_w * ln_g[None, None, :]                   # [NE, 512, 512]
    bp = aff_w @ ln_b + aff_b                          # [NE, 512]
    waff_e = []
    for e_ in range(NE):
        A = np.ascontiguousarray(
            Wp[e_].T.reshape(4, 128, 512).transpose(1, 0, 2))
        waff_e.append(A.astype(BF16NP))
    baff_e = [np.ascontiguousarray(bp[e_].reshape(4, 128).T)
              for e_ in range(NE)]

    sel_h = np.zeros((128, 4, 8), np.float32)
    for r in range(128):
        for p in range(4):
            sel_h[r, p, 2 * p + r // 64] = 1.0
    exps_h = np.zeros((8, 4, 128), np.float32)
    for p in range(4):
        for c in range(128):
            exps_h[2 * p + c // 64, p, c] = 1.0

    zero_bias = (np.all(bq == 0) and np.all(in_proj_b[E:] == 0)
                 and np.all(fus_b2 == 0)
                 and all(np.all(b == 0) for b in baff_e))

    base = {
        "whh": whh_h, "wcr": wcr_h, "bqkv": bqkv_h,
        "w1o": w1o_h, "beff": beff_h, "w2": w2_h, "wmu": wmu_h, "b2": b2_h,
        "sel": sel_h.astype(BF16NP), "exps": exps_h.astype(BF16NP),
        "ones512": np.full((128, 1), 1.0 / E, np.float32).astype(BF16NP),
        "onesk1": np.ones((1, 128), np.float32),
        "meanb2": np.full((1, 1), fus_b2.mean(), np.float32),
    }
    return base, waff_e, baff_e, zero_bias, perm


def _pack_slots(labels):
    """Assign samples to 64 slots of 256; return per-core chunk plans."""
    order_ids = [np.nonzero(labels == e_)[0] for e_ in range(NE)]
    counts = [len(x) for x in order_ids]
    assert sum(counts) == B
    slots = []          # list of (ids[256], eA, eB, nA)
    leftovers = []      # (expert, ids)
    for e_ in range(NE):
        ids = order_ids[e_]
        nfull = len(ids) // C
        for s in range(nfull):
            slots.append((ids[s * C:(s + 1) * C], e_, e_, C))
        if len(ids) % C:
            leftovers.append((e_, ids[nfull * C:]))
    # pack leftovers into mixed slots (each must span <= 2 experts)
    mixed = []
    stream = []
    for e_, ids in leftovers:
        stream.append((e_, list(ids)))
    while stream:
        eA, idsA = stream[0]
        if len(idsA) >= C:
            mixed.append((np.array(idsA[:C]), eA, eA, C))
            stream[0] = (eA, idsA[C:])
            if not stream[0][1]:
                stream.pop(0)
            continue
        if len(stream) == 1:
            assert len(idsA) == 0 or len(idsA) == C, \
                f"unpackable remainder {len(idsA)}"
            if idsA:
                mixed.append((np.array(idsA), eA, eA, C))
            stream.pop(0)
            continue
        eB, idsB = stream[1]
        take = C - len(idsA)
        assert len(idsB) >= take, (
            f"slot would span 3 experts: {len(idsA)} + {len(idsB)} < {C}")
        ids = np.concatenate([idsA, idsB[:take]])
        mixed.append((ids, eA, eB, len(idsA)))
        stream.pop(0)
        stream[1 - 1] = (eB, idsB[take:])
        if not stream[0][1]:
            stream.pop(0)
    assert len(slots) + len(mixed) == B // C
    assert len(mixed) <= NCORES, f"{len(mixed)} mixed slots > {NCORES} cores"

    # per core: 8 slots, mixed slot (if any) at position 7
    plans = []
    si = 0
    for c in range(NCORES):
        mine = []
        if c < len(mixed):
            n_pure = NCH - 1
        else:
            n_pure = NCH
        mine = slots[si:si + n_pure]
        si += n_pure
        if c < len(mixed):
            mine = mine + [mixed[c]]
        plans.append(mine)
    assert si == len(slots)
    return plans


def kernel(**inputs):
    img = np.asarray(inputs["image_embeddings"], np.float32)
    txt = np.asarray(inputs["text_embeddings"], np.float32)
    kno = np.asarray(inputs["knowledge_embeddings"], np.float32)
    labels = np.asarray(inputs["affective_labels"]).astype(np.int64).ravel()
    assert img.shape == (B, E)

    base, waff_e, baff_e, zero_bias, perm = _prep_weights(inputs)
    plans = _pack_slots(labels)

    xs = np.stack([img, txt, kno])                     # [3, B, 512]

    in_maps = []
    for c in range(NCORES):
        plan = plans[c]
        gi = np.concatenate([p[0] for p in plan])      # [2048]
        xg = xs[:, gi, :].transpose(0, 2, 1)           # [3, 512, R]
        xg = xg.reshape(3, 4, 128, NCH, C)             # [t, p, r, ch, c]
        x_hi = xg.astype(FP8NP)
        x_lo = (xg - x_hi.astype(np.float32)).astype(FP8NP)
        xhl_h = np.empty((NCH, 128, 3, 4, 2, C), FP8NP)
        xhl_h[:, :, :, :, 0, :] = x_lo.transpose(3, 2, 0, 1, 4)
        xhl_h[:, :, :, :, 1, :] = x_hi.transpose(3, 2, 0, 1, 4)

        waffA_h = np.stack([waff_e[p[1]] for p in plan])     # [NCH,128,4,512]
        baffA_h = np.stack([baff_e[p[1]] for p in plan])     # [NCH,128,4]
        eB = plan[NCH - 1][2]
        waffB_h = waff_e[eB]
        baffB_h = baff_e[eB]
        nA = plan[NCH - 1][3]
        mask = np.zeros((128, C), np.float32)
        mask[:, :nA] = 1.0

        m = dict(base)
        m["xhl"] = xhl_h
        m["waffA"] = waffA_h
        m["baffA"] = baffA_h
        m["waffB"] = waffB_h
        m["baffB"] = baffB_h
        m["maskex"] = mask.astype(BF16NP)
        in_maps.append(m)

    nc = _build_program(zero_bias)
    res = run_bass_kernel_spmd(nc, in_maps, core_ids=list(range(NCORES)))
    global LAST_RESULTS, LAST_NC
    LAST_RESULTS = res
    LAST_NC = nc

    out_full = np.zeros((B, E), np.float32)
    for c in range(NCORES):
        oT = res.results[c]["outT"]                    # [NCH,128,4,C] bf16
        oT = np.asarray(oT, dtype=np.float32)
        plan = plans[c]
        for ch in range(NCH):
            ids = plan[ch][0]
            # [128, 4, C] -> [C, 512] with feature f = 128*p + r
            blk = oT[ch].transpose(2, 1, 0).reshape(C, 4 * 128)
            out_full[ids] = blk
    return out_full


if __name__ == "__main__":
    rng = np.random.default_rng(0)
    fake = {
        "image_embeddings": rng.standard_normal((B, E)).astype(np.float32),
        "text_embeddings": rng.standard_normal((B, E)).astype(np.float32),
        "knowledge_embeddings": rng.standard_normal((B, E)).astype(np.float32),
        "affective_labels": rng.integers(0, NE, B),
        "in_proj_w": (rng.standard_normal((3 * E, E)) * 0.02).astype(np.float32),
        "in_proj_b": np.zeros(3 * E, np.float32),
        "out_proj_w": (rng.standard_normal((E, E)) * 0.02).astype(np.float32),
        "out_proj_b": np.zeros(E, np.float32),
        "fus_w1": (rng.standard_normal((H, 3 * E)) * 0.02).astype(np.float32),
        "fus_b1": np.zeros(H, np.float32),
        "fus_w2": (rng.standard_normal((E, H)) * 0.02).astype(np.float32),
        "fus_b2": np.zeros(E, np.float32),
        "ln_g": np.ones(E, np.float32),
        "ln_b": np.zeros(E, np.float32),
        "aff_w": (rng.standard_normal((NE, E, E)) * 0.02).astype(np.float32),
        "aff_b": np.zeros((NE, E), np.float32),
    }
    out = kernel(**fake)
    print("kernel ran, out:", out.shape, out.dtype, np.abs(out).max())


# revision 6
# speedup vs baseline: 1.0934x; 1.0191x over previous
"""Trainium2 Bass kernel v2 for nn_MultiModalFusion (moe_routing).

Design (driven by the TimelineSim cost model; 246.7us vs 385.3us baseline):
- Pure data parallel over 8 cores; host sorts samples by expert into 64
  slots of exactly 256 columns (zero padding waste). At most 3 slots mix
  two experts; those sit at chunk 7 of cores 0..2 where the program computes
  both experts and blends with a host-built column mask.
- QKV in fp8e4m3 DoubleRow matmuls (0.5 cyc/row) with host-side hi/lo error
  compensation: x = x_hi + x_lo, W = W_hi + W_lo, keeping W_hi*x_hi,
  W_hi*x_lo + W_lo*x_hi (2 hi-hi pairs + 4 cross pairs per (token, m-tile) =
  3C cycles vs 4C for fp32r). W is pre-scaled by 2^5 so its fp8 planes stay
  clear of e4m3's subnormal range (undone by the free Activation copy scale).
- All other matmul moving operands bf16 (1 cyc/row at any width); DVE
  element-wise work in 2x bf16 mode where possible.
- Attention via o_i = v0 + p_i1*(v1-v0) + p_i2*(v2-v0): only 2 of 3
  probability broadcasts per query token.
- LayerNorm: mean via a host-precomputed column-mean row folded into fus2;
  rstd = exp(-0.5*ln(var+eps)) on the scalar engine — ln/exp/identity/relu/
  square all live in one activation table set (set 6, preloaded manually),
  so there are zero LoadActFuncSet switches. gamma/beta are folded into the
  expert weights on the host.
- PSUM pair tiles (2x256 f32 = one 2KB bank) let one Activation instruction
  evacuate two matmul outputs.
- Software pipelining: each chunk's tail (W1o/fus2/LN/expert) is emitted one
  chunk late, phase-interleaved into the next chunk's attention middle; the
  next chunk's QKV is split into 18 atomic pieces pulled into every
  dependency stall window. v_j - v_0 runs early on the otherwise idle
  GPSIMD engine.
"""

import numpy as np
import ml_dtypes

import concourse.bass as bass
import concourse.mybir as mybir
import concourse.tile as tile
from concourse import bacc
from concourse.bass_utils import run_bass_kernel_spmd

E = 512
H = 256
NH = 8
HD = 64
NE = 4
B = 16384
NCORES = 8
C = 256              # chunk columns
NCH = 8              # chunks per core
R = NE * 512         # 2048 columns per core

LAST_RESULTS = None
LAST_NC = None

F32 = mybir.dt.float32
F32R = mybir.dt.float32r
BF16 = mybir.dt.bfloat16
FP8 = mybir.dt.float8e4
AF = mybir.ActivationFunctionType
ALU = mybir.AluOpType
MM = mybir.MatmulPerfMode

FP8NP = ml_dtypes.float8_e4m3
BF16NP = ml_dtypes.bfloat16

_NC_CACHE = {}


def _build_program(zero_bias, debug=False):
    key = (bool(zero_bias), bool(debug))
    if key in _NC_CACHE:
        return _NC_CACHE[key]
    nc = bacc.Bacc("TRN2")

    # ---------------- DRAM I/O ----------------
    # x hi/lo fp8: [ch, 128, t, p, plane(0=lo,1=hi), C]
    xhl = nc.dram_tensor("xhl", [NCH, 128, 3, 4, 2, C], FP8,
                         kind="ExternalInput")
    # QKV weights fp8: hi-hi k-pairs and (hi,lo) cross pairs
    whh = nc.dram_tensor("whh", [128, 2, 2, 1536], FP8, kind="ExternalInput")
    wcr = nc.dram_tensor("wcr", [128, 4, 2, 1536], FP8, kind="ExternalInput")
    bqkv = nc.dram_tensor("bqkv", [128, 12], F32, kind="ExternalInput")
    w1o = nc.dram_tensor("w1o", [128, 12, 256], BF16, kind="ExternalInput")
    beff = nc.dram_tensor("beff", [128, 2], F32, kind="ExternalInput")
    w2 = nc.dram_tensor("w2", [128, 2, 512], BF16, kind="ExternalInput")
    wmu = nc.dram_tensor("wmu", [128, 2], BF16, kind="ExternalInput")
    b2 = nc.dram_tensor("b2", [128, 4], F32, kind="ExternalInput")
    # per-chunk expert weights, host-selected: A for all chunks, B for ch 7
    waffA = nc.dram_tensor("waffA", [NCH, 128, 4, 512], BF16,
                           kind="ExternalInput")
    waffB = nc.dram_tensor("waffB", [128, 4, 512], BF16,
                           kind="ExternalInput")
    baffA = nc.dram_tensor("baffA", [NCH, 128, 4], F32,
                           kind="ExternalInput")
    baffB = nc.dram_tensor("baffB", [128, 4], F32, kind="ExternalInput")
    maskex = nc.dram_tensor("maskex", [128, C], BF16, kind="ExternalInput")
    meanb2 = nc.dram_tensor("meanb2", [1, 1], F32, kind="ExternalInput")
    sel = nc.dram_tensor("sel", [128, 4, 8], BF16, kind="ExternalInput")
    exps = nc.dram_tensor("exps", [8, 4, 128], BF16, kind="ExternalInput")
    ones512 = nc.dram_tensor("ones512", [128, 1], BF16, kind="ExternalInput")
    onesk1 = nc.dram_tensor("onesk1", [1, 128], F32R, kind="ExternalInput")
    outT = nc.dram_tensor("outT", [NCH, 128, 4, C], BF16,
                          kind="ExternalOutput")
    if debug:
        dbg_q = nc.dram_tensor("dbg_q", [128, 3, 4, C], F32,
                               kind="ExternalOutput")
        dbg_k = nc.dram_tensor("dbg_k", [128, 3, 4, C], F32,
                               kind="ExternalOutput")
        dbg_v = nc.dram_tensor("dbg_v", [128, 3, 4, C], F32,
                               kind="ExternalOutput")
        dbg_e = nc.dram_tensor("dbg_e", [8, 3, 3, C], F32,
                               kind="ExternalOutput")
        dbg_d = nc.dram_tensor("dbg_d", [8, 3, 2, C], F32,
                               kind="ExternalOutput")
        dbg_o = nc.dram_tensor("dbg_o", [128, 12, C], F32,
                               kind="ExternalOutput")
        dbg_h = nc.dram_tensor("dbg_h", [128, 2, C], F32,
                               kind="ExternalOutput")
        dbg_y = nc.dram_tensor("dbg_y", [128, 4, C], F32,
                               kind="ExternalOutput")
        dbg_mu = nc.dram_tensor("dbg_mu", [1, C], F32,
                                kind="ExternalOutput")
        dbg_var = nc.dram_tensor("dbg_var", [1, C], F32,
                                 kind="ExternalOutput")
        dbg_rstd = nc.dram_tensor("dbg_rstd", [1, C], F32,
                                  kind="ExternalOutput")
        dbg_fused = nc.dram_tensor("dbg_fused", [128, 4, C], F32,
                                   kind="ExternalOutput")

    with tile.TileContext(nc) as tc:
        with tc.tile_pool(name="wp", bufs=1) as wp, \
             tc.tile_pool(name="xp", bufs=2) as xp, \
             tc.tile_pool(name="qkvp", bufs=2) as qkvp, \
             tc.tile_pool(name="ap", bufs=2) as ap, \
             tc.tile_pool(name="spM", bufs=2) as spM, \
             tc.tile_pool(name="spZ", bufs=2) as spZ, \
             tc.tile_pool(name="spS", bufs=2) as spS, \
             tc.tile_pool(name="mrexp", bufs=2) as mrexp, \
             tc.tile_pool(name="sp2", bufs=3) as sp2, \
             tc.tile_pool(name="sp3", bufs=2) as sp3, \
             tc.tile_pool(name="psQ", bufs=2, space="PSUM") as psQ, \
             tc.tile_pool(name="psB", bufs=3, space="PSUM") as psB, \
             tc.tile_pool(name="psT", bufs=2, space="PSUM") as psT, \
             tc.tile_pool(name="psst", bufs=1, space="PSUM") as psst:

            # ---------------- persistent weights ----------------
            nc.scalar.add_instruction(mybir.InstLoadActFuncSet(
                name=nc.get_next_instruction_name(), ins=[], outs=[],
                act_func_set_id=6))
            whh_sb = wp.tile([128, 2, 2, 1536], FP8)
            wcr_sb = wp.tile([128, 4, 2, 1536], FP8)
            # first halves only — QKV m-tiles 0..5 can start ASAP
            nc.sync.dma_start(whh_sb[:, :, :, 0:768], whh[:, :, :, 0:768])
            nc.sync.dma_start(wcr_sb[:, :, :, 0:768], wcr[:, :, :, 0:768])
            bqkv_sb = wp.tile([128, 12], F32)
            sel_sb = wp.tile([128, 4, 8], BF16)
            exps_sb = wp.tile([8, 4, 128], BF16)
            eps_sb = wp.tile([1, 1], F32)

            def load_more_front():
                nc.sync.dma_start(whh_sb[:, :, :, 768:], whh[:, :, :, 768:])
                nc.sync.dma_start(wcr_sb[:, :, :, 768:], wcr[:, :, :, 768:])
                nc.sync.dma_start(bqkv_sb[:], bqkv[:])
                nc.sync.dma_start(sel_sb[:], sel[:])
                nc.sync.dma_start(exps_sb[:], exps[:])
                nc.vector.memset(eps_sb[:], 1e-5)
            w1o_sb = wp.tile([128, 12, 256], BF16)
            beff_sb = wp.tile([128, 2], F32)
            w2_sb = wp.tile([128, 2, 512], BF16)
            wmu_sb = wp.tile([128, 2], BF16)
            b2_sb = wp.tile([128, 4], F32)
            o512_sb = wp.tile([128, 1], BF16)
            ok1_sb = wp.tile([1, 128], F32R)
            waffB_sb = wp.tile([128, 4, 512], BF16)
            baffB_sb = wp.tile([128, 4], F32)
            maskex_sb = wp.tile([128, C], BF16)
            meanb2_sb = wp.tile([1, 1], F32)

            def load_tail_weights():
                nc.sync.dma_start(w1o_sb[:], w1o[:])
                nc.sync.dma_start(beff_sb[:], beff[:])
                nc.sync.dma_start(w2_sb[:], w2[:])
                nc.sync.dma_start(wmu_sb[:], wmu[:])
                nc.sync.dma_start(b2_sb[:], b2[:])
                nc.sync.dma_start(o512_sb[:], ones512[:])
                nc.sync.dma_start(ok1_sb[:], onesk1[:])
                nc.sync.dma_start(waffB_sb[:], waffB[:])
                nc.sync.dma_start(baffB_sb[:], baffB[:])
                nc.sync.dma_start(maskex_sb[:], maskex[:])
                nc.sync.dma_start(meanb2_sb[:], meanb2[:])

            def front_alloc(ch):
                """x + expert-weight loads, qkv tile allocation, chunk ch."""
                x_sb = xp.tile([128, 3, 4, 2, C], FP8, tag="x", name=f"x{ch}")
                for t in range(3):
                    nc.sync.dma_start(x_sb[:, t], xhl[ch, :, t])
                waff_sb = ap.tile([128, 4, 512], BF16, tag="waff",
                                  name=f"waff{ch}")
                nc.sync.dma_start(waff_sb[:], waffA[ch])
                baff_sb = ap.tile([128, 4], F32, tag="baff", name=f"baff{ch}")
                nc.sync.dma_start(baff_sb[:], baffA[ch])
                q_sb = qkvp.tile([128, 3, 4, C], BF16, tag="q", name=f"q{ch}")
                k_sb = qkvp.tile([128, 3, 4, C], BF16, tag="k", name=f"k{ch}")
                v_sb = qkvp.tile([128, 3, 4, C], BF16, tag="v", name=f"v{ch}")
                return {"ch": ch, "x": x_sb, "q": q_sb, "k": k_sb, "v": v_sb,
                        "waff": waff_sb, "baff": baff_sb}

            def front_qkv_mp(st, t, mp):
                """QKV m-pair mp (12 DR matmuls + 1 pair copy) — atomic."""
                ch = st["ch"]
                x_sb = st["x"]
                pair = psQ.tile([128, 2, C], F32, tag="qkv",
                                name=f"qkv{ch}_{t}_{mp}")
                for half in range(2):
                    m = 2 * mp + half
                    mc = slice(m * 128, (m + 1) * 128)
                    for kp in range(2):
                        nc.tensor.matmul(
                            pair[:, half, :],
                            whh_sb[:, kp, :, mc],
                            x_sb[:, t, 2 * kp:2 * kp + 2, 1, :],
                            start=(kp == 0), stop=False,
                            perf_mode=MM.DoubleRow)
                    for p in range(4):
                        nc.tensor.matmul(
                            pair[:, half, :],
                            wcr_sb[:, p, :, mc],
                            x_sb[:, t, p, :, :],
                            start=False, stop=(p == 3),
                            perf_mode=MM.DoubleRow)
                dst = (st["q"], st["k"], st["v"])[mp // 2]
                pp = mp % 2
                if zero_bias:
                    nc.scalar.activation(
                        dst[:, t, 2 * pp:2 * pp + 2, :], pair[:],
                        AF.Identity, scale=1.0 / 32.0)
                else:
                    for half in range(2):
                        m = 2 * mp + half
                        nc.scalar.activation(
                            dst[:, t, 2 * pp + half, :], pair[:, half, :],
                            AF.Identity, bias=bqkv_sb[:, m:m + 1],
                            scale=1.0 / 32.0)

            def middle(ch, st, pieces, tailph):
                """Scores/softmax/o for chunk ch; interleaves previous
                chunk's tail phases (tailph) and next-chunk QKV pieces."""
                def pull(n=1):
                    for _ in range(n):
                        if pieces:
                            pieces.pop(0)()
                q_sb, k_sb, v_sb = st["q"], st["k"], st["v"]

                # w_j = v_j - v_0 early on the idle Pool engine
                wv_sb = spM.tile([128, 2, 4, C], BF16, tag="wv",
                                 name=f"wv{ch}")
                for j in (1, 2):
                    nc.gpsimd.tensor_tensor(wv_sb[:, j - 1, :, :],
                                            v_sb[:, j, :, :],
                                            v_sb[:, 0, :, :],
                                            ALU.subtract)

                # scores + exp; tail W1o/fus2 phases fill the DVE-bound spots
                e_sb = spM.tile([8, 3, 3, C], BF16, tag="esb", name=f"e{ch}")
                for i in range(3):
                    for j in range(3):
                        prod = sp2.tile([128, 4, C], BF16, tag="prod",
                                        name=f"prod{ch}_{i}_{j}")
                        nc.vector.tensor_tensor(
                            prod[:], q_sb[:, i, :, :], k_sb[:, j, :, :],
                            ALU.mult)
                        s_ps = psB.tile([8, C], F32, tag="bc",
                                        name=f"s{ch}_{i}_{j}")
                        for p in range(4):
                            nc.tensor.matmul(
                                s_ps[:], sel_sb[:, p, :], prod[:, p, :],
                                start=(p == 0), stop=(p == 3))
                        nc.scalar.activation(e_sb[:, i, j, :], s_ps[:],
                                             AF.Exp)
                    if tailph:
                        tailph.pop(0)()
                    pull(1)

                # softmax -> p1, p2 ; previous tail's stats/LN fill PE here
                z_sb = spZ.tile([8, 3, C], F32, tag="z", name=f"z{ch}")
                nc.vector.tensor_tensor(z_sb[:], e_sb[:, :, 0, :],
                                        e_sb[:, :, 1, :], ALU.add)
                nc.vector.tensor_tensor(z_sb[:], z_sb[:], e_sb[:, :, 2, :],
                                        ALU.add)
                nc.vector.reciprocal_approx_fast(z_sb[:], z_sb[:])
                rz_sb = z_sb
                d_sb = spZ.tile([8, 3, 2, C], BF16, tag="dsb", name=f"d{ch}")
                for j in (1, 2):
                    nc.vector.tensor_tensor(d_sb[:, :, j - 1, :],
                                            e_sb[:, :, j, :], rz_sb[:],
                                            ALU.mult)
                pull(1)

                # o_i = v0 + p_i1*w1 + p_i2*w2 ; previous expert fills PE
                o_sb = ap.tile([128, 12, C], BF16, tag="o", name=f"o{ch}")
                for i in range(3):
                    pv = sp2.tile([128, 4, 2, C], BF16, tag="pv",
                                  name=f"pv{ch}_{i}")
                    for p in range(4):
                        bc = psB.tile([128, 2, C], F32, tag="bc",
                                      name=f"bc{ch}_{i}_{p}")
                        for j in (1, 2):
                            nc.tensor.matmul(
                                bc[:, j - 1, :], exps_sb[:, p, :],
                                d_sb[:, i, j - 1, :], start=True, stop=True)
                        nc.vector.tensor_tensor(pv[:, p, :, :], bc[:],
                                                wv_sb[:, :, p, :], ALU.mult)
                        pull(1)
                    if tailph:
                        tailph.pop(0)()
                    nc.vector.tensor_tensor(pv[:, :, 0, :], pv[:, :, 0, :],
                                            pv[:, :, 1, :], ALU.add)
                    nc.vector.tensor_tensor(o_sb[:, i * 4:(i + 1) * 4, :],
                                            pv[:, :, 0, :],
                                            v_sb[:, 0, :, :], ALU.add)
                    if tailph:
                        tailph.pop(0)()
                return o_sb

            def make_tail(ch, o_sb):
                """Return tail phase closures for chunk ch (uses o_sb)."""
                waff_sb = ap.tile([128, 4, 512], BF16, tag="waff",
                                  name=f"waff{ch}")
                nc.sync.dma_start(waff_sb[:], waffA[ch])
                baff_sb = ap.tile([128, 4], F32, tag="baff", name=f"baff{ch}")
                nc.sync.dma_start(baff_sb[:], baffA[ch])
                state = {}

                def ph_w1o():
                    hp = psT.tile([128, 2, C], F32, tag="tail",
                                  name=f"hp{ch}")
                    for m2t in range(2):
                        for kip in range(12):
                            nc.tensor.matmul(
                                hp[:, m2t, :],
                                w1o_sb[:, kip, m2t * 128:(m2t + 1) * 128],
                                o_sb[:, kip, :],
                                start=(kip == 0), stop=(kip == 11))
                    hpre = ap.tile([128, 2, C], BF16, tag="hpre",
                                   name=f"hpre{ch}")
                    for m2t in range(2):
                        nc.scalar.activation(hpre[:, m2t, :], hp[:, m2t, :],
                                             AF.Relu,
                                             bias=beff_sb[:, m2t:m2t + 1])
                    state["hpre"] = hpre

                def ph_fus2():
                    hpre = state["hpre"]
                    st_ps = psst.tile([1, 2, C], F32, tag="st",
                                      name=f"st{ch}")
                    for ks in range(2):
                        nc.tensor.matmul(st_ps[:, 0, :], wmu_sb[:, ks:ks + 1],
                                         hpre[:, ks, :],
                                         start=(ks == 0), stop=(ks == 1))
                    y_sb = ap.tile([128, 4, C], BF16, tag="y", name=f"y{ch}")
                    ysq = sp3.tile([128, 4, C], BF16, tag="ysq",
                                   name=f"ysq{ch}")
                    for yp2 in range(2):
                        yp = psT.tile([128, 2, C], F32, tag="tail",
                                      name=f"yp{ch}_{yp2}")
                        for half in range(2):
                            m4 = 2 * yp2 + half
                            for ks in range(2):
                                nc.tensor.matmul(
                                    yp[:, half, :],
                                    w2_sb[:, ks, m4 * 128:(m4 + 1) * 128],
                                    hpre[:, ks, :], start=(ks == 0),
                                    stop=(ks == 1))
                        if zero_bias:
                            nc.scalar.activation(
                                y_sb[:, 2 * yp2:2 * yp2 + 2, :], yp[:],
                                AF.Identity)
                            nc.scalar.activation(
                                ysq[:, 2 * yp2:2 * yp2 + 2, :], yp[:],
                                AF.Square)
                        else:
                            for half in range(2):
                                m4 = 2 * yp2 + half
                                nc.scalar.activation(
                                    y_sb[:, m4, :], yp[:, half, :],
                                    AF.Identity, bias=b2_sb[:, m4:m4 + 1])
                                nc.scalar.activation(
                                    ysq[:, m4, :], yp[:, half, :], AF.Square,
                                    bias=b2_sb[:, m4:m4 + 1])
                    state["st_ps"] = st_ps
                    state["y"] = y_sb
                    state["ysq"] = ysq

                def ph_stats():
                    st_ps, y_sb, ysq = (state["st_ps"], state["y"],
                                        state["ysq"])
                    for p in range(4):
                        nc.tensor.matmul(st_ps[:, 1, :], o512_sb[:],
                                         ysq[:, p, :],
                                         start=(p == 0), stop=(p == 3))
                    mu_sb = spS.tile([1, C], F32, tag="musb",
                                     name=f"musb{ch}")
                    if zero_bias:
                        nc.vector.tensor_scalar_add(mu_sb[:], st_ps[:, 0, :],
                                                    0.0)
                    else:
                        nc.vector.tensor_scalar_add(mu_sb[:], st_ps[:, 0, :],
                                                    meanb2_sb[:])
                    musq = spS.tile([1, C], F32, tag="musq", name=f"musq{ch}")
                    nc.gpsimd.tensor_tensor(musq[:], mu_sb[:], mu_sb[:],
                                            ALU.mult)
                    var_sb = spS.tile([1, C], F32, tag="varsb",
                                      name=f"var{ch}")
                    nc.vector.tensor_tensor(var_sb[:], st_ps[:, 1, :],
                                            musq[:], ALU.subtract)
                    lnv = spS.tile([1, C], F32, tag="lnv", name=f"lnv{ch}")
                    nc.scalar.activation(lnv[:], var_sb[:], AF.Ln,
                                         bias=eps_sb[:])
                    rstd_sb = spS.tile([1, C], F32R, tag="rstd",
                                       name=f"rstd{ch}")
                    nc.scalar.activation(rstd_sb[:], lnv[:], AF.Exp,
                                         scale=-0.5)
                    murs = spS.tile([1, C], F32R, tag="murs",
                                    name=f"murs{ch}")
                    nc.vector.tensor_tensor(murs[:], mu_sb[:], rstd_sb[:],
                                            ALU.mult)
                    bcp = psT.tile([128, 2, C], F32, tag="tail",
                                   name=f"bcp{ch}")
                    nc.tensor.matmul(bcp[:, 0, :], ok1_sb[:], murs[:],
                                     start=True, stop=True)
                    nc.tensor.matmul(bcp[:, 1, :], ok1_sb[:], rstd_sb[:],
                                     start=True, stop=True)
                    mrex = mrexp.tile([128, 2, C], BF16, tag="mrex",
                                      name=f"mrex{ch}")
                    nc.scalar.activation(mrex[:], bcp[:], AF.Identity)
                    fused = ap.tile([128, 4, C], BF16, tag="fused",
                                    name=f"fused{ch}")
                    nc.vector.tensor_tensor(
                        fused[:], y_sb[:],
                        mrex[:, 1, None, :].to_broadcast((128, 4, C)),
                        ALU.mult)
                    nc.vector.tensor_tensor(
                        fused[:], fused[:],
                        mrex[:, 0, None, :].to_broadcast((128, 4, C)),
                        ALU.subtract)
                    state["fused"] = fused

                def ph_expert():
                    fused = state["fused"]
                    ot = sp3.tile([128, 4, C], BF16, tag="ot", name=f"ot{ch}")
                    is_mixed = (ch == NCH - 1)
                    for op2 in range(2):
                        op = psT.tile([128, 2, C], F32, tag="tail",
                                      name=f"op{ch}_{op2}")
                        for half in range(2):
                            m4 = 2 * op2 + half
                            for ks in range(4):
                                nc.tensor.matmul(
                                    op[:, half, :],
                                    waff_sb[:, ks, m4 * 128:(m4 + 1) * 128],
                                    fused[:, ks, :], start=(ks == 0),
                                    stop=(ks == 3))
                        if not is_mixed:
                            if zero_bias:
                                nc.scalar.activation(
                                    ot[:, 2 * op2:2 * op2 + 2, :], op[:],
                                    AF.Identity)
                            else:
                                for half in range(2):
                                    m4 = 2 * op2 + half
                                    nc.scalar.activation(
                                        ot[:, m4, :], op[:, half, :],
                                        AF.Identity,
                                        bias=baff_sb[:, m4:m4 + 1])
                        else:
                            opB = psB.tile([128, 2, C], F32, tag="bc",
                                           name=f"opB{ch}_{op2}")
                            for half in range(2):
                                m4 = 2 * op2 + half
                                for ks in range(4):
                                    nc.tensor.matmul(
                                        opB[:, half, :],
                                        waffB_sb[:, ks,
                                                 m4 * 128:(m4 + 1) * 128],
                                        fused[:, ks, :], start=(ks == 0),
                                        stop=(ks == 3))
                            oA = sp3.tile([128, 2, C], BF16, tag="oA",
                                          name=f"oA{ch}_{op2}")
                            oB = sp3.tile([128, 2, C], BF16, tag="oB",
                                          name=f"oB{ch}_{op2}")
                            if zero_bias:
                                nc.scalar.activation(oA[:], op[:],
                                                     AF.Identity)
                                nc.scalar.activation(oB[:], opB[:],
                                                     AF.Identity)
                            else:
                                for half in range(2):
                                    m4 = 2 * op2 + half
                                    nc.scalar.activation(
                                        oA[:, half, :], op[:, half, :],
                                        AF.Identity,
                                        bias=baff_sb[:, m4:m4 + 1])
                                    nc.scalar.activation(
                                        oB[:, half, :], opB[:, half, :],
                                        AF.Identity,
                                        bias=baffB_sb[:, m4:m4 + 1])
                            dAB = sp3.tile([128, 2, C], BF16, tag="dAB",
                                           name=f"dAB{ch}_{op2}")
                            nc.vector.tensor_tensor(dAB[:], oA[:], oB[:],
                                                    ALU.subtract)
                            nc.vector.tensor_tensor(
                                dAB[:], dAB[:],
                                maskex_sb[:, None, :].to_broadcast(
                                    (128, 2, C)), ALU.mult)
                            nc.vector.tensor_tensor(
                                ot[:, 2 * op2:2 * op2 + 2, :], oB[:], dAB[:],
                                ALU.add)
                    nc.sync.dma_start(outT[ch], ot[:])

                return [lambda: None, lambda: None, ph_w1o, ph_fus2,
                        ph_stats, ph_expert]

            cur = front_alloc(0)
            first = True
            for t in range(3):
                for mp in range(6):
                    front_qkv_mp(cur, t, mp)
                    if first:
                        load_more_front()
                        first = False
                if t == 0:
                    load_tail_weights()
            prev_tail = []
            for ch in range(NCH):
                if ch + 1 < NCH:
                    nxt = front_alloc(ch + 1)
                    pieces = [lambda t=t, mp=mp, s=nxt: front_qkv_mp(s, t, mp)
                              for t in range(3) for mp in range(6)]
                else:
                    nxt, pieces = None, []
                o_sb = middle(ch, cur, pieces, prev_tail)
                for ph in prev_tail:
                    ph()
                prev_tail = make_tail(ch, o_sb)
                for p_ in pieces:
                    p_()
                cur = nxt
            # drain: last chunk's tail
            for ph in prev_tail:
                ph()

    nc.finalize()
    _NC_CACHE[key] = nc
    return nc


def _fp8_split(a):
    """Return (lo, hi) fp8e4m3 arrays with hi + lo ~= a."""
    hi = a.astype(FP8NP)
    lo = (a - hi.astype(np.float32)).astype(FP8NP)
    return lo, hi


def _prep_weights(inputs):
    in_proj_w = np.asarray(inputs["in_proj_w"], np.float32)
    in_proj_b = np.asarray(inputs["in_proj_b"], np.float32)
    out_proj_w = np.asarray(inputs["out_proj_w"], np.float32)
    out_proj_b = np.asarray(inputs["out_proj_b"], np.float32)
    fus_w1 = np.asarray(inputs["fus_w1"], np.float32)
    fus_b1 = np.asarray(inputs["fus_b1"], np.float32)
    fus_w2 = np.asarray(inputs["fus_w2"], np.float32)
    fus_b2 = np.asarray(inputs["fus_b2"], np.float32)
    ln_g = np.asarray(inputs["ln_g"], np.float32)
    ln_b = np.asarray(inputs["ln_b"], np.float32)
    aff_w = np.asarray(inputs["aff_w"], np.float32)
    aff_b = np.asarray(inputs["aff_b"], np.float32)

    scale = 1.0 / np.sqrt(np.float32(HD))
    W = in_proj_w.copy()
    W[:E] *= scale
    bq = in_proj_b.copy()
    bq[:E] *= scale
    # pre-scale W by 2^5 so the fp8 hi/lo planes stay out of e4m3's
    # subnormal range (W sigma ~0.02); undone by the Act copy scale 2^-5
    W *= 32.0

    # W.T is [512(k), 1536(m)]
    WT = np.ascontiguousarray(W.T)
    WT_lo, WT_hi = _fp8_split(WT)
    WT_lo = WT_lo.reshape(4, 128, 1536)
    WT_hi = WT_hi.reshape(4, 128, 1536)
    # hi-hi pairs: [128, kp, e, 1536] with (kp,e) -> k-subtile 2*kp+e
    whh_h = np.empty((128, 2, 2, 1536), FP8NP)
    for kp in range(2):
        for e_ in range(2):
            whh_h[:, kp, e_, :] = WT_hi[2 * kp + e_]
    # cross pairs: [128, p, {hi,lo}, 1536], paired with x (lo, hi)
    wcr_h = np.empty((128, 4, 2, 1536), FP8NP)
    for p in range(4):
        wcr_h[:, p, 0, :] = WT_hi[p]
        wcr_h[:, p, 1, :] = WT_lo[p]
    bqkv_h = np.ascontiguousarray(bq.reshape(12, 128).T)

    # fold out_proj into fus_w1; permute (h,d) -> (p, hl, d) to match v layout
    perm = np.empty(E, np.int64)
    for h in range(NH):
        for d in range(HD):
            perm[(h // 2) * 128 + (h % 2) * 64 + d] = h * HD + d
    blocks = []
    for i in range(3):
        blk = fus_w1[:, i * E:(i + 1) * E] @ out_proj_w  # [256, 512]
        blocks.append(blk[:, perm])
    W1o = np.concatenate(blocks, axis=1)  # [256, 1536]
    w1o_h = np.ascontiguousarray(
        W1o.T.reshape(12, 128, 256).transpose(1, 0, 2)).astype(BF16NP)
    # v bias folds into beff: o_i includes +bv for each i
    beff = fus_b1 + fus_w1 @ np.tile(out_proj_b, 3)
    beff_h = np.ascontiguousarray(beff.reshape(2, 128).T)

    w2_h = np.ascontiguousarray(
        fus_w2.T.reshape(2, 128, 512).transpose(1, 0, 2)).astype(BF16NP)
    wmu_h = np.ascontiguousarray(
        (fus_w2.mean(axis=0)).reshape(2, 128).T).astype(BF16NP)
    b2_h = np.ascontiguousarray(fus_b2.reshape(4, 128).T)

    # gamma/beta folded into expert weights/biases
    Wp = aff_w * ln_g[None, None, :]                   # [NE, 512, 512]
    bp = aff_w @ ln_b + aff_b                          # [NE, 512]
    waff_e = []
    for e_ in range(NE):
        A = np.ascontiguousarray(
            Wp[e_].T.reshape(4, 128, 512).transpose(1, 0, 2))
        waff_e.append(A.astype(BF16NP))
    baff_e = [np.ascontiguousarray(bp[e_].reshape(4, 128).T)
              for e_ in range(NE)]

    sel_h = np.zeros((128, 4, 8), np.float32)
    for r in range(128):
        for p in range(4):
            sel_h[r, p, 2 * p + r // 64] = 1.0
    exps_h = np.zeros((8, 4, 128), np.float32)
    for p in range(4):
        for c in range(128):
            exps_h[2 * p + c // 64, p, c] = 1.0

    zero_bias = (np.all(bq == 0) and np.all(in_proj_b[E:] == 0)
                 and np.all(fus_b2 == 0)
                 and all(np.all(b == 0) for b in baff_e))

    base = {
        "whh": whh_h, "wcr": wcr_h, "bqkv": bqkv_h,
        "w1o": w1o_h, "beff": beff_h, "w2": w2_h, "wmu": wmu_h, "b2": b2_h,
        "sel": sel_h.astype(BF16NP), "exps": exps_h.astype(BF16NP),
        "ones512": np.full((128, 1), 1.0 / E, np.float32).astype(BF16NP),
        "onesk1": np.ones((1, 128), np.float32),
        "meanb2": np.full((1, 1), fus_b2.mean(), np.float32),
    }
    return base, waff_e, baff_e, zero_bias, perm


def _pack_slots(labels):
    """Assign samples to 64 slots of 256; return per-core chunk plans."""
    order_ids = [np.nonzero(labels == e_)[0] for e_ in range(NE)]
    counts = [len(x) for x in order_ids]
    assert sum(counts) == B
    slots = []          # list of (ids[256], eA, eB, nA)
    leftovers = []      # (expert, ids)
    for e_ in range(NE):
        ids = order_ids[e_]
        nfull = len(ids) // C
        for s in range(nfull):
            slots.append((ids[s * C:(s + 1) * C], e_, e_, C))
        if len(ids) % C:
            leftovers.append((e_, ids[nfull * C:]))
    # pack leftovers into mixed slots (each must span <= 2 experts)
    mixed = []
    stream = []
    for e_, ids in leftovers:
        stream.append((e_, list(ids)))
    while stream:
        eA, idsA = stream[0]
        if len(idsA) >= C:
            mixed.append((np.array(idsA[:C]), eA, eA, C))
            stream[0] = (eA, idsA[C:])
            if not stream[0][1]:
                stream.pop(0)
            continue
        if len(stream) == 1:
            assert len(idsA) == 0 or len(idsA) == C, \
                f"unpackable remainder {len(idsA)}"
            if idsA:
                mixed.append((np.array(idsA), eA, eA, C))
            stream.pop(0)
            continue
        eB, idsB = stream[1]
        take = C - len(idsA)
        assert len(idsB) >= take, (
            f"slot would span 3 experts: {len(idsA)} + {len(idsB)} < {C}")
        ids = np.concatenate([idsA, idsB[:take]])
        mixed.append((ids, eA, eB, len(idsA)))
        stream.pop(0)
        stream[1 - 1] = (eB, idsB[take:])
        if not stream[0][1]:
            stream.pop(0)
    assert len(slots) + len(mixed) == B // C
    assert len(mixed) <= NCORES, f"{len(mixed)} mixed slots > {NCORES} cores"

    # per core: 8 slots, mixed slot (if any) at position 7
    plans = []
    si = 0
    for c in range(NCORES):
        mine = []
        if c < len(mixed):
            n_pure = NCH - 1
        else:
            n_pure = NCH
        mine = slots[si:si + n_pure]
        si += n_pure
        if c < len(mixed):
            mine = mine + [mixed[c]]
        plans.append(mine)
    assert si == len(slots)
    return plans


def kernel(**inputs):
    img = np.asarray(inputs["image_embeddings"], np.float32)
    txt = np.asarray(inputs["text_embeddings"], np.float32)
    kno = np.asarray(inputs["knowledge_embeddings"], np.float32)
    labels = np.asarray(inputs["affective_labels"]).astype(np.int64).ravel()
    assert img.shape == (B, E)

    base, waff_e, baff_e, zero_bias, perm = _prep_weights(inputs)
    plans = _pack_slots(labels)

    xs = np.stack([img, txt, kno])                     # [3, B, 512]

    in_maps = []
    for c in range(NCORES):
        plan = plans[c]
        gi = np.concatenate([p[0] for p in plan])      # [2048]
        xg = xs[:, gi, :].transpose(0, 2, 1)           # [3, 512, R]
        xg = xg.reshape(3, 4, 128, NCH, C)             # [t, p, r, ch, c]
        x_hi = xg.astype(FP8NP)
        x_lo = (xg - x_hi.astype(np.float32)).astype(FP8NP)
        xhl_h = np.empty((NCH, 128, 3, 4, 2, C), FP8NP)
        xhl_h[:, :, :, :, 0, :] = x_lo.transpose(3, 2, 0, 1, 4)
        xhl_h[:, :, :, :, 1, :] = x_hi.transpose(3, 2, 0, 1, 4)

        waffA_h = np.stack([waff_e[p[1]] for p in plan])     # [NCH,128,4,512]
        baffA_h = np.stack([baff_e[p[1]] for p in plan])     # [NCH,128,4]
        eB = plan[NCH - 1][2]
        waffB_h = waff_e[eB]
        baffB_h = baff_e[eB]
        nA = plan[NCH - 1][3]
        mask = np.zeros((128, C), np.float32)
        mask[:, :nA] = 1.0

        m = dict(base)
        m["xhl"] = xhl_h
        m["waffA"] = waffA_h
        m["baffA"] = baffA_h
        m["waffB"] = waffB_h
        m["baffB"] = baffB_h
        m["maskex"] = mask.astype(BF16NP)
        in_maps.append(m)

    nc = _build_program(zero_bias)
    res = run_bass_kernel_spmd(nc, in_maps, core_ids=list(range(NCORES)))
    global LAST_RESULTS, LAST_NC
    LAST_RESULTS = res
    LAST_NC = nc

    out_full = np.zeros((B, E), np.float32)
    for c in range(NCORES):
        oT = res.results[c]["outT"]                    # [NCH,128,4,C] bf16
        oT = np.asarray(oT, dtype=np.float32)
        plan = plans[c]
        for ch in range(NCH):
            ids = plan[ch][0]
            # [128, 4, C] -> [C, 512] with feature f = 128*p + r
            blk = oT[ch].transpose(2, 1, 0).reshape(C, 4 * 128)
            out_full[ids] = blk
    return out_full


if __name__ == "__main__":
    rng = np.random.default_rng(0)
    fake = {
        "image_embeddings": rng.standard_normal((B, E)).astype(np.float32),
        "text_embeddings": rng.standard_normal((B, E)).astype(np.float32),
        "knowledge_embeddings": rng.standard_normal((B, E)).astype(np.float32),
        "affective_labels": rng.integers(0, NE, B),
        "in_proj_w": (rng.standard_normal((3 * E, E)) * 0.02).astype(np.float32),
        "in_proj_b": np.zeros(3 * E, np.float32),
        "out_proj_w": (rng.standard_normal((E, E)) * 0.02).astype(np.float32),
        "out_proj_b": np.zeros(E, np.float32),
        "fus_w1": (rng.standard_normal((H, 3 * E)) * 0.02).astype(np.float32),
        "fus_b1": np.zeros(H, np.float32),
        "fus_w2": (rng.standard_normal((E, H)) * 0.02).astype(np.float32),
        "fus_b2": np.zeros(E, np.float32),
        "ln_g": np.ones(E, np.float32),
        "ln_b": np.zeros(E, np.float32),
        "aff_w": (rng.standard_normal((NE, E, E)) * 0.02).astype(np.float32),
        "aff_b": np.zeros((NE, E), np.float32),
    }
    out = kernel(**fake)
    print("kernel ran, out:", out.shape, out.dtype, np.abs(out).max())


# revision 7
# speedup vs baseline: 1.0965x; 1.0029x over previous
"""Trainium2 Bass kernel v2 for nn_MultiModalFusion (moe_routing).

Design (driven by the TimelineSim cost model; 246.7us vs 385.3us baseline):
- Pure data parallel over 8 cores; host sorts samples by expert into 64
  slots of exactly 256 columns (zero padding waste). At most 3 slots mix
  two experts; those sit at chunk 7 of cores 0..2 where the program computes
  both experts and blends with a host-built column mask.
- QKV in fp8e4m3 DoubleRow matmuls (0.5 cyc/row) with host-side hi/lo error
  compensation: x = x_hi + x_lo, W = W_hi + W_lo, keeping W_hi*x_hi,
  W_hi*x_lo + W_lo*x_hi (2 hi-hi pairs + 4 cross pairs per (token, m-tile) =
  3C cycles vs 4C for fp32r). W is pre-scaled by 2^5 so its fp8 planes stay
  clear of e4m3's subnormal range (undone by the free Activation copy scale).
- All other matmul moving operands bf16 (1 cyc/row at any width); DVE
  element-wise work in 2x bf16 mode where possible.
- Attention via o_i = v0 + p_i1*(v1-v0) + p_i2*(v2-v0): only 2 of 3
  probability broadcasts per query token.
- LayerNorm: mean via a host-precomputed column-mean row folded into fus2;
  rstd = exp(-0.5*ln(var+eps)) on the scalar engine — ln/exp/identity/relu/
  square all live in one activation table set (set 6, preloaded manually),
  so there are zero LoadActFuncSet switches. gamma/beta are folded into the
  expert weights on the host.
- PSUM pair tiles (2x256 f32 = one 2KB bank) let one Activation instruction
  evacuate two matmul outputs.
- Software pipelining: each chunk's tail (W1o/fus2/LN/expert) is emitted one
  chunk late, phase-interleaved into the next chunk's attention middle; the
  next chunk's QKV is split into 18 atomic pieces pulled into every
  dependency stall window. v_j - v_0 runs early on the otherwise idle
  GPSIMD engine.
"""

import numpy as np
import ml_dtypes

import concourse.bass as bass
import concourse.mybir as mybir
import concourse.tile as tile
from concourse import bacc
from concourse.bass_utils import run_bass_kernel_spmd

E = 512
H = 256
NH = 8
HD = 64
NE = 4
B = 16384
NCORES = 8
C = 256              # chunk columns
NCH = 8              # chunks per core
R = NE * 512         # 2048 columns per core

LAST_RESULTS = None
LAST_NC = None

F32 = mybir.dt.float32
F32R = mybir.dt.float32r
BF16 = mybir.dt.bfloat16
FP8 = mybir.dt.float8e4
AF = mybir.ActivationFunctionType
ALU = mybir.AluOpType
MM = mybir.MatmulPerfMode

FP8NP = ml_dtypes.float8_e4m3
BF16NP = ml_dtypes.bfloat16

_NC_CACHE = {}


def _build_program(zero_bias, debug=False):
    key = (bool(zero_bias), bool(debug))
    if key in _NC_CACHE:
        return _NC_CACHE[key]
    nc = bacc.Bacc("TRN2")

    # ---------------- DRAM I/O ----------------
    # x hi/lo fp8: [ch, 128, t, p, plane(0=lo,1=hi), C]
    xhl = nc.dram_tensor("xhl", [NCH, 128, 3, 4, 2, C], FP8,
                         kind="ExternalInput")
    # QKV weights fp8: hi-hi k-pairs and (hi,lo) cross pairs
    whh = nc.dram_tensor("whh", [128, 2, 2, 1536], FP8, kind="ExternalInput")
    wcr = nc.dram_tensor("wcr", [128, 4, 2, 1536], FP8, kind="ExternalInput")
    bqkv = nc.dram_tensor("bqkv", [128, 12], F32, kind="ExternalInput")
    w1o = nc.dram_tensor("w1o", [128, 12, 256], BF16, kind="ExternalInput")
    beff = nc.dram_tensor("beff", [128, 2], F32, kind="ExternalInput")
    w2 = nc.dram_tensor("w2", [128, 2, 512], BF16, kind="ExternalInput")
    wmu = nc.dram_tensor("wmu", [128, 2], BF16, kind="ExternalInput")
    b2 = nc.dram_tensor("b2", [128, 4], F32, kind="ExternalInput")
    # per-chunk expert weights, host-selected: A for all chunks, B for ch 7
    waffA = nc.dram_tensor("waffA", [NCH, 128, 4, 512], BF16,
                           kind="ExternalInput")
    waffB = nc.dram_tensor("waffB", [128, 4, 512], BF16,
                           kind="ExternalInput")
    baffA = nc.dram_tensor("baffA", [NCH, 128, 4], F32,
                           kind="ExternalInput")
    baffB = nc.dram_tensor("baffB", [128, 4], F32, kind="ExternalInput")
    maskex = nc.dram_tensor("maskex", [128, C], BF16, kind="ExternalInput")
    meanb2 = nc.dram_tensor("meanb2", [1, 1], F32, kind="ExternalInput")
    sel = nc.dram_tensor("sel", [128, 4, 8], BF16, kind="ExternalInput")
    exps = nc.dram_tensor("exps", [8, 4, 128], BF16, kind="ExternalInput")
    ones512 = nc.dram_tensor("ones512", [128, 1], BF16, kind="ExternalInput")
    onesk1 = nc.dram_tensor("onesk1", [1, 128], F32R, kind="ExternalInput")
    outT = nc.dram_tensor("outT", [NCH, 128, 4, C], BF16,
                          kind="ExternalOutput")
    if debug:
        dbg_q = nc.dram_tensor("dbg_q", [128, 3, 4, C], F32,
                               kind="ExternalOutput")
        dbg_k = nc.dram_tensor("dbg_k", [128, 3, 4, C], F32,
                               kind="ExternalOutput")
        dbg_v = nc.dram_tensor("dbg_v", [128, 3, 4, C], F32,
                               kind="ExternalOutput")
        dbg_e = nc.dram_tensor("dbg_e", [8, 3, 3, C], F32,
                               kind="ExternalOutput")
        dbg_d = nc.dram_tensor("dbg_d", [8, 3, 2, C], F32,
                               kind="ExternalOutput")
        dbg_o = nc.dram_tensor("dbg_o", [128, 12, C], F32,
                               kind="ExternalOutput")
        dbg_h = nc.dram_tensor("dbg_h", [128, 2, C], F32,
                               kind="ExternalOutput")
        dbg_y = nc.dram_tensor("dbg_y", [128, 4, C], F32,
                               kind="ExternalOutput")
        dbg_mu = nc.dram_tensor("dbg_mu", [1, C], F32,
                                kind="ExternalOutput")
        dbg_var = nc.dram_tensor("dbg_var", [1, C], F32,
                                 kind="ExternalOutput")
        dbg_rstd = nc.dram_tensor("dbg_rstd", [1, C], F32,
                                  kind="ExternalOutput")
        dbg_fused = nc.dram_tensor("dbg_fused", [128, 4, C], F32,
                                   kind="ExternalOutput")

    with tile.TileContext(nc) as tc:
        with tc.tile_pool(name="wp", bufs=1) as wp, \
             tc.tile_pool(name="xp", bufs=2) as xp, \
             tc.tile_pool(name="qkvp", bufs=2) as qkvp, \
             tc.tile_pool(name="ap", bufs=2) as ap, \
             tc.tile_pool(name="spM", bufs=2) as spM, \
             tc.tile_pool(name="spZ", bufs=2) as spZ, \
             tc.tile_pool(name="spS", bufs=2) as spS, \
             tc.tile_pool(name="mrexp", bufs=2) as mrexp, \
             tc.tile_pool(name="sp2", bufs=3) as sp2, \
             tc.tile_pool(name="sp3", bufs=2) as sp3, \
             tc.tile_pool(name="psQ", bufs=3, space="PSUM") as psQ, \
             tc.tile_pool(name="psB", bufs=2, space="PSUM") as psB, \
             tc.tile_pool(name="psT", bufs=2, space="PSUM") as psT, \
             tc.tile_pool(name="psst", bufs=1, space="PSUM") as psst:

            # ---------------- persistent weights ----------------
            nc.scalar.add_instruction(mybir.InstLoadActFuncSet(
                name=nc.get_next_instruction_name(), ins=[], outs=[],
                act_func_set_id=6))
            whh_sb = wp.tile([128, 2, 2, 1536], FP8)
            wcr_sb = wp.tile([128, 4, 2, 1536], FP8)
            # first halves only — QKV m-tiles 0..5 can start ASAP
            nc.sync.dma_start(whh_sb[:, :, :, 0:768], whh[:, :, :, 0:768])
            nc.sync.dma_start(wcr_sb[:, :, :, 0:768], wcr[:, :, :, 0:768])
            bqkv_sb = wp.tile([128, 12], F32)
            sel_sb = wp.tile([128, 4, 8], BF16)
            exps_sb = wp.tile([8, 4, 128], BF16)
            eps_sb = wp.tile([1, 1], F32)

            def load_more_front():
                nc.sync.dma_start(whh_sb[:, :, :, 768:], whh[:, :, :, 768:])
                nc.sync.dma_start(wcr_sb[:, :, :, 768:], wcr[:, :, :, 768:])
                nc.sync.dma_start(bqkv_sb[:], bqkv[:])
                nc.sync.dma_start(sel_sb[:], sel[:])
                nc.sync.dma_start(exps_sb[:], exps[:])
                nc.vector.memset(eps_sb[:], 1e-5)
            w1o_sb = wp.tile([128, 12, 256], BF16)
            beff_sb = wp.tile([128, 2], F32)
            w2_sb = wp.tile([128, 2, 512], BF16)
            wmu_sb = wp.tile([128, 2], BF16)
            b2_sb = wp.tile([128, 4], F32)
            o512_sb = wp.tile([128, 1], BF16)
            ok1_sb = wp.tile([1, 128], F32R)
            waffB_sb = wp.tile([128, 4, 512], BF16)
            baffB_sb = wp.tile([128, 4], F32)
            maskex_sb = wp.tile([128, C], BF16)
            meanb2_sb = wp.tile([1, 1], F32)

            def load_tail_weights():
                nc.sync.dma_start(w1o_sb[:], w1o[:])
                nc.sync.dma_start(beff_sb[:], beff[:])
                nc.sync.dma_start(w2_sb[:], w2[:])
                nc.sync.dma_start(wmu_sb[:], wmu[:])
                nc.sync.dma_start(b2_sb[:], b2[:])
                nc.sync.dma_start(o512_sb[:], ones512[:])
                nc.sync.dma_start(ok1_sb[:], onesk1[:])
                nc.sync.dma_start(waffB_sb[:], waffB[:])
                nc.sync.dma_start(baffB_sb[:], baffB[:])
                nc.sync.dma_start(maskex_sb[:], maskex[:])
                nc.sync.dma_start(meanb2_sb[:], meanb2[:])

            def front_alloc(ch):
                """x + expert-weight loads, qkv tile allocation, chunk ch."""
                x_sb = xp.tile([128, 3, 4, 2, C], FP8, tag="x", name=f"x{ch}")
                for t in range(3):
                    nc.sync.dma_start(x_sb[:, t], xhl[ch, :, t])
                waff_sb = ap.tile([128, 4, 512], BF16, tag="waff",
                                  name=f"waff{ch}")
                nc.sync.dma_start(waff_sb[:], waffA[ch])
                baff_sb = ap.tile([128, 4], F32, tag="baff", name=f"baff{ch}")
                nc.sync.dma_start(baff_sb[:], baffA[ch])
                q_sb = qkvp.tile([128, 3, 4, C], BF16, tag="q", name=f"q{ch}")
                k_sb = qkvp.tile([128, 3, 4, C], BF16, tag="k", name=f"k{ch}")
                v_sb = qkvp.tile([128, 3, 4, C], BF16, tag="v", name=f"v{ch}")
                return {"ch": ch, "x": x_sb, "q": q_sb, "k": k_sb, "v": v_sb,
                        "waff": waff_sb, "baff": baff_sb}

            def front_qkv_mp(st, t, mp):
                """QKV m-pair mp (12 DR matmuls + 1 pair copy) — atomic."""
                ch = st["ch"]
                x_sb = st["x"]
                pair = psQ.tile([128, 2, C], F32, tag="qkv",
                                name=f"qkv{ch}_{t}_{mp}")
                for half in range(2):
                    m = 2 * mp + half
                    mc = slice(m * 128, (m + 1) * 128)
                    for kp in range(2):
                        nc.tensor.matmul(
                            pair[:, half, :],
                            whh_sb[:, kp, :, mc],
                            x_sb[:, t, 2 * kp:2 * kp + 2, 1, :],
                            start=(kp == 0), stop=False,
                            perf_mode=MM.DoubleRow)
                    for p in range(4):
                        nc.tensor.matmul(
                            pair[:, half, :],
                            wcr_sb[:, p, :, mc],
                            x_sb[:, t, p, :, :],
                            start=False, stop=(p == 3),
                            perf_mode=MM.DoubleRow)
                dst = (st["q"], st["k"], st["v"])[mp // 2]
                pp = mp % 2
                if zero_bias:
                    nc.scalar.activation(
                        dst[:, t, 2 * pp:2 * pp + 2, :], pair[:],
                        AF.Identity, scale=1.0 / 32.0)
                else:
                    for half in range(2):
                        m = 2 * mp + half
                        nc.scalar.activation(
                            dst[:, t, 2 * pp + half, :], pair[:, half, :],
                            AF.Identity, bias=bqkv_sb[:, m:m + 1],
                            scale=1.0 / 32.0)

            def middle(ch, st, pieces, tailph):
                """Scores/softmax/o for chunk ch; interleaves previous
                chunk's tail phases (tailph) and next-chunk QKV pieces."""
                def pull(n=1):
                    for _ in range(n):
                        if pieces:
                            pieces.pop(0)()
                q_sb, k_sb, v_sb = st["q"], st["k"], st["v"]

                # w_j = v_j - v_0 early on the idle Pool engine
                wv_sb = spM.tile([128, 2, 4, C], BF16, tag="wv",
                                 name=f"wv{ch}")
                for j in (1, 2):
                    nc.gpsimd.tensor_tensor(wv_sb[:, j - 1, :, :],
                                            v_sb[:, j, :, :],
                                            v_sb[:, 0, :, :],
                                            ALU.subtract)

                # scores + exp; tail W1o/fus2 phases fill the DVE-bound spots
                e_sb = spM.tile([8, 3, 3, C], BF16, tag="esb", name=f"e{ch}")
                for i in range(3):
                    for j in range(3):
                        prod = sp2.tile([128, 4, C], BF16, tag="prod",
                                        name=f"prod{ch}_{i}_{j}")
                        nc.vector.tensor_tensor(
                            prod[:], q_sb[:, i, :, :], k_sb[:, j, :, :],
                            ALU.mult)
                        s_ps = psB.tile([8, C], F32, tag="bc",
                                        name=f"s{ch}_{i}_{j}")
                        for p in range(4):
                            nc.tensor.matmul(
                                s_ps[:], sel_sb[:, p, :], prod[:, p, :],
                                start=(p == 0), stop=(p == 3))
                        nc.scalar.activation(e_sb[:, i, j, :], s_ps[:],
                                             AF.Exp)
                    if tailph:
                        tailph.pop(0)()
                    pull(1)

                # softmax -> p1, p2 ; previous tail's stats/LN fill PE here
                z_sb = spZ.tile([8, 3, C], F32, tag="z", name=f"z{ch}")
                nc.vector.tensor_tensor(z_sb[:], e_sb[:, :, 0, :],
                                        e_sb[:, :, 1, :], ALU.add)
                nc.vector.tensor_tensor(z_sb[:], z_sb[:], e_sb[:, :, 2, :],
                                        ALU.add)
                nc.vector.reciprocal_approx_fast(z_sb[:], z_sb[:])
                rz_sb = z_sb
                d_sb = spZ.tile([8, 3, 2, C], BF16, tag="dsb", name=f"d{ch}")
                for j in (1, 2):
                    nc.vector.tensor_tensor(d_sb[:, :, j - 1, :],
                                            e_sb[:, :, j, :], rz_sb[:],
                                            ALU.mult)
                pull(1)

                # o_i = v0 + p_i1*w1 + p_i2*w2 ; previous expert fills PE
                o_sb = ap.tile([128, 12, C], BF16, tag="o", name=f"o{ch}")
                for i in range(3):
                    pv = sp2.tile([128, 4, 2, C], BF16, tag="pv",
                                  name=f"pv{ch}_{i}")
                    for p in range(4):
                        bc = psB.tile([128, 2, C], F32, tag="bc",
                                      name=f"bc{ch}_{i}_{p}")
                        for j in (1, 2):
                            nc.tensor.matmul(
                                bc[:, j - 1, :], exps_sb[:, p, :],
                                d_sb[:, i, j - 1, :], start=True, stop=True)
                        nc.vector.tensor_tensor(pv[:, p, :, :], bc[:],
                                                wv_sb[:, :, p, :], ALU.mult)
                        pull(2)
                    if tailph:
                        tailph.pop(0)()
                    nc.vector.tensor_tensor(pv[:, :, 0, :], pv[:, :, 0, :],
                                            pv[:, :, 1, :], ALU.add)
                    nc.vector.tensor_tensor(o_sb[:, i * 4:(i + 1) * 4, :],
                                            pv[:, :, 0, :],
                                            v_sb[:, 0, :, :], ALU.add)
                    if tailph:
                        tailph.pop(0)()
                return o_sb

            def make_tail(ch, o_sb):
                """Return tail phase closures for chunk ch (uses o_sb)."""
                waff_sb = ap.tile([128, 4, 512], BF16, tag="waff",
                                  name=f"waff{ch}")
                nc.sync.dma_start(waff_sb[:], waffA[ch])
                baff_sb = ap.tile([128, 4], F32, tag="baff", name=f"baff{ch}")
                nc.sync.dma_start(baff_sb[:], baffA[ch])
                state = {}

                def ph_w1o():
                    hp = psT.tile([128, 2, C], F32, tag="tail",
                                  name=f"hp{ch}")
                    for m2t in range(2):
                        for kip in range(12):
                            nc.tensor.matmul(
                                hp[:, m2t, :],
                                w1o_sb[:, kip, m2t * 128:(m2t + 1) * 128],
                                o_sb[:, kip, :],
                                start=(kip == 0), stop=(kip == 11))
                    hpre = ap.tile([128, 2, C], BF16, tag="hpre",
                                   name=f"hpre{ch}")
                    for m2t in range(2):
                        nc.scalar.activation(hpre[:, m2t, :], hp[:, m2t, :],
                                             AF.Relu,
                                             bias=beff_sb[:, m2t:m2t + 1])
                    state["hpre"] = hpre

                def ph_fus2():
                    hpre = state["hpre"]
                    st_ps = psst.tile([1, 2, C], F32, tag="st",
                                      name=f"st{ch}")
                    for ks in range(2):
                        nc.tensor.matmul(st_ps[:, 0, :], wmu_sb[:, ks:ks + 1],
                                         hpre[:, ks, :],
                                         start=(ks == 0), stop=(ks == 1))
                    y_sb = ap.tile([128, 4, C], BF16, tag="y", name=f"y{ch}")
                    ysq = sp3.tile([128, 4, C], BF16, tag="ysq",
                                   name=f"ysq{ch}")
                    for yp2 in range(2):
                        yp = psT.tile([128, 2, C], F32, tag="tail",
                                      name=f"yp{ch}_{yp2}")
                        for half in range(2):
                            m4 = 2 * yp2 + half
                            for ks in range(2):
                                nc.tensor.matmul(
                                    yp[:, half, :],
                                    w2_sb[:, ks, m4 * 128:(m4 + 1) * 128],
                                    hpre[:, ks, :], start=(ks == 0),
                                    stop=(ks == 1))
                        if zero_bias:
                            nc.scalar.activation(
                                y_sb[:, 2 * yp2:2 * yp2 + 2, :], yp[:],
                                AF.Identity)
                            nc.scalar.activation(
                                ysq[:, 2 * yp2:2 * yp2 + 2, :], yp[:],
                                AF.Square)
                        else:
                            for half in range(2):
                                m4 = 2 * yp2 + half
                                nc.scalar.activation(
                                    y_sb[:, m4, :], yp[:, half, :],
                                    AF.Identity, bias=b2_sb[:, m4:m4 + 1])
                                nc.scalar.activation(
                                    ysq[:, m4, :], yp[:, half, :], AF.Square,
                                    bias=b2_sb[:, m4:m4 + 1])
                    state["st_ps"] = st_ps
                    state["y"] = y_sb
                    state["ysq"] = ysq

                def ph_stats():
                    st_ps, y_sb, ysq = (state["st_ps"], state["y"],
                                        state["ysq"])
                    for p in range(4):
                        nc.tensor.matmul(st_ps[:, 1, :], o512_sb[:],
                                         ysq[:, p, :],
                                         start=(p == 0), stop=(p == 3))
                    mu_sb = spS.tile([1, C], F32, tag="musb",
                                     name=f"musb{ch}")
                    if zero_bias:
                        nc.vector.tensor_scalar_add(mu_sb[:], st_ps[:, 0, :],
                                                    0.0)
                    else:
                        nc.vector.tensor_scalar_add(mu_sb[:], st_ps[:, 0, :],
                                                    meanb2_sb[:])
                    musq = spS.tile([1, C], F32, tag="musq", name=f"musq{ch}")
                    nc.gpsimd.tensor_tensor(musq[:], mu_sb[:], mu_sb[:],
                                            ALU.mult)
                    var_sb = spS.tile([1, C], F32, tag="varsb",
                                      name=f"var{ch}")
                    nc.vector.tensor_tensor(var_sb[:], st_ps[:, 1, :],
                                            musq[:], ALU.subtract)
                    lnv = spS.tile([1, C], F32, tag="lnv", name=f"lnv{ch}")
                    nc.scalar.activation(lnv[:], var_sb[:], AF.Ln,
                                         bias=eps_sb[:])
                    rstd_sb = spS.tile([1, C], F32R, tag="rstd",
                                       name=f"rstd{ch}")
                    nc.scalar.activation(rstd_sb[:], lnv[:], AF.Exp,
                                         scale=-0.5)
                    murs = spS.tile([1, C], F32R, tag="murs",
                                    name=f"murs{ch}")
                    nc.vector.tensor_tensor(murs[:], mu_sb[:], rstd_sb[:],
                                            ALU.mult)
                    bcp = psT.tile([128, 2, C], F32, tag="tail",
                                   name=f"bcp{ch}")
                    nc.tensor.matmul(bcp[:, 0, :], ok1_sb[:], murs[:],
                                     start=True, stop=True)
                    nc.tensor.matmul(bcp[:, 1, :], ok1_sb[:], rstd_sb[:],
                                     start=True, stop=True)
                    mrex = mrexp.tile([128, 2, C], BF16, tag="mrex",
                                      name=f"mrex{ch}")
                    nc.scalar.activation(mrex[:], bcp[:], AF.Identity)
                    fused = ap.tile([128, 4, C], BF16, tag="fused",
                                    name=f"fused{ch}")
                    nc.vector.tensor_tensor(
                        fused[:], y_sb[:],
                        mrex[:, 1, None, :].to_broadcast((128, 4, C)),
                        ALU.mult)
                    nc.vector.tensor_tensor(
                        fused[:], fused[:],
                        mrex[:, 0, None, :].to_broadcast((128, 4, C)),
                        ALU.subtract)
                    state["fused"] = fused

                def ph_expert():
                    fused = state["fused"]
                    ot = sp3.tile([128, 4, C], BF16, tag="ot", name=f"ot{ch}")
                    is_mixed = (ch == NCH - 1)
                    for op2 in range(2):
                        op = psT.tile([128, 2, C], F32, tag="tail",
                                      name=f"op{ch}_{op2}")
                        for half in range(2):
                            m4 = 2 * op2 + half
                            for ks in range(4):
                                nc.tensor.matmul(
                                    op[:, half, :],
                                    waff_sb[:, ks, m4 * 128:(m4 + 1) * 128],
                                    fused[:, ks, :], start=(ks == 0),
                                    stop=(ks == 3))
                        if not is_mixed:
                            if zero_bias:
                                nc.scalar.activation(
                                    ot[:, 2 * op2:2 * op2 + 2, :], op[:],
                                    AF.Identity)
                            else:
                                for half in range(2):
                                    m4 = 2 * op2 + half
                                    nc.scalar.activation(
                                        ot[:, m4, :], op[:, half, :],
                                        AF.Identity,
                                        bias=baff_sb[:, m4:m4 + 1])
                        else:
                            opB = psB.tile([128, 2, C], F32, tag="bc",
                                           name=f"opB{ch}_{op2}")
                            for half in range(2):
                                m4 = 2 * op2 + half
                                for ks in range(4):
                                    nc.tensor.matmul(
                                        opB[:, half, :],
                                        waffB_sb[:, ks,
                                                 m4 * 128:(m4 + 1) * 128],
                                        fused[:, ks, :], start=(ks == 0),
                                        stop=(ks == 3))
                            oA = sp3.tile([128, 2, C], BF16, tag="oA",
                                          name=f"oA{ch}_{op2}")
                            oB = sp3.tile([128, 2, C], BF16, tag="oB",
                                          name=f"oB{ch}_{op2}")
                            if zero_bias:
                                nc.scalar.activation(oA[:], op[:],
                                                     AF.Identity)
                                nc.scalar.activation(oB[:], opB[:],
                                                     AF.Identity)
                            else:
                                for half in range(2):
                                    m4 = 2 * op2 + half
                                    nc.scalar.activation(
                                        oA[:, half, :], op[:, half, :],
                                        AF.Identity,
                                        bias=baff_sb[:, m4:m4 + 1])
                                    nc.scalar.activation(
                                        oB[:, half, :], opB[:, half, :],
                                        AF.Identity,
                                        bias=baffB_sb[:, m4:m4 + 1])
                            dAB = sp3.tile([128, 2, C], BF16, tag="dAB",
                                           name=f"dAB{ch}_{op2}")
                            nc.vector.tensor_tensor(dAB[:], oA[:], oB[:],
                                                    ALU.subtract)
                            nc.vector.tensor_tensor(
                                dAB[:], dAB[:],
                                maskex_sb[:, None, :].to_broadcast(
                                    (128, 2, C)), ALU.mult)
                            nc.vector.tensor_tensor(
                                ot[:, 2 * op2:2 * op2 + 2, :], oB[:], dAB[:],
                                ALU.add)
                    nc.sync.dma_start(outT[ch], ot[:])

                return [lambda: None, lambda: None, ph_w1o, ph_fus2,
                        ph_stats, ph_expert]

            cur = front_alloc(0)
            first = True
            for t in range(3):
                for mp in range(6):
                    front_qkv_mp(cur, t, mp)
                    if first:
                        load_more_front()
                        first = False
                if t == 0:
                    load_tail_weights()
            prev_tail = []
            for ch in range(NCH):
                if ch + 1 < NCH:
                    nxt = front_alloc(ch + 1)
                    pieces = [lambda t=t, mp=mp, s=nxt: front_qkv_mp(s, t, mp)
                              for t in range(3) for mp in range(6)]
                else:
                    nxt, pieces = None, []
                o_sb = middle(ch, cur, pieces, prev_tail)
                for ph in prev_tail:
                    ph()
                prev_tail = make_tail(ch, o_sb)
                for p_ in pieces:
                    p_()
                cur = nxt
            # drain: last chunk's tail
            for ph in prev_tail:
                ph()

    nc.finalize()
    _NC_CACHE[key] = nc
    return nc


def _fp8_split(a):
    """Return (lo, hi) fp8e4m3 arrays with hi + lo ~= a."""
    hi = a.astype(FP8NP)
    lo = (a - hi.astype(np.float32)).astype(FP8NP)
    return lo, hi


def _prep_weights(inputs):
    in_proj_w = np.asarray(inputs["in_proj_w"], np.float32)
    in_proj_b = np.asarray(inputs["in_proj_b"], np.float32)
    out_proj_w = np.asarray(inputs["out_proj_w"], np.float32)
    out_proj_b = np.asarray(inputs["out_proj_b"], np.float32)
    fus_w1 = np.asarray(inputs["fus_w1"], np.float32)
    fus_b1 = np.asarray(inputs["fus_b1"], np.float32)
    fus_w2 = np.asarray(inputs["fus_w2"], np.float32)
    fus_b2 = np.asarray(inputs["fus_b2"], np.float32)
    ln_g = np.asarray(inputs["ln_g"], np.float32)
    ln_b = np.asarray(inputs["ln_b"], np.float32)
    aff_w = np.asarray(inputs["aff_w"], np.float32)
    aff_b = np.asarray(inputs["aff_b"], np.float32)

    scale = 1.0 / np.sqrt(np.float32(HD))
    W = in_proj_w.copy()
    W[:E] *= scale
    bq = in_proj_b.copy()
    bq[:E] *= scale
    # pre-scale W by 2^5 so the fp8 hi/lo planes stay out of e4m3's
    # subnormal range (W sigma ~0.02); undone by the Act copy scale 2^-5
    W *= 32.0

    # W.T is [512(k), 1536(m)]
    WT = np.ascontiguousarray(W.T)
    WT_lo, WT_hi = _fp8_split(WT)
    WT_lo = WT_lo.reshape(4, 128, 1536)
    WT_hi = WT_hi.reshape(4, 128, 1536)
    # hi-hi pairs: [128, kp, e, 1536] with (kp,e) -> k-subtile 2*kp+e
    whh_h = np.empty((128, 2, 2, 1536), FP8NP)
    for kp in range(2):
        for e_ in range(2):
            whh_h[:, kp, e_, :] = WT_hi[2 * kp + e_]
    # cross pairs: [128, p, {hi,lo}, 1536], paired with x (lo, hi)
    wcr_h = np.empty((128, 4, 2, 1536), FP8NP)
    for p in range(4):
        wcr_h[:, p, 0, :] = WT_hi[p]
        wcr_h[:, p, 1, :] = WT_lo[p]
    bqkv_h = np.ascontiguousarray(bq.reshape(12, 128).T)

    # fold out_proj into fus_w1; permute (h,d) -> (p, hl, d) to match v layout
    perm = np.empty(E, np.int64)
    for h in range(NH):
        for d in range(HD):
            perm[(h // 2) * 128 + (h % 2) * 64 + d] = h * HD + d
    blocks = []
    for i in range(3):
        blk = fus_w1[:, i * E:(i + 1) * E] @ out_proj_w  # [256, 512]
        blocks.append(blk[:, perm])
    W1o = np.concatenate(blocks, axis=1)  # [256, 1536]
    w1o_h = np.ascontiguousarray(
        W1o.T.reshape(12, 128, 256).transpose(1, 0, 2)).astype(BF16NP)
    # v bias folds into beff: o_i includes +bv for each i
    beff = fus_b1 + fus_w1 @ np.tile(out_proj_b, 3)
    beff_h = np.ascontiguousarray(beff.reshape(2, 128).T)

    w2_h = np.ascontiguousarray(
        fus_w2.T.reshape(2, 128, 512).transpose(1, 0, 2)).astype(BF16NP)
    wmu_h = np.ascontiguousarray(
        (fus_w2.mean(axis=0)).reshape(2, 128).T).astype(BF16NP)
    b2_h = np.ascontiguousarray(fus_b2.reshape(4, 128).T)

    # gamma/beta folded into expert weights/biases
    Wp = aff_w * ln_g[None, None, :]                   # [NE, 512, 512]
    bp = aff_w @ ln_b + aff_b                          # [NE, 512]
    waff_e = []
    for e_ in range(NE):
        A = np.ascontiguousarray(
            Wp[e_].T.reshape(4, 128, 512).transpose(1, 0, 2))
        waff_e.append(A.astype(BF16NP))
    baff_e = [np.ascontiguousarray(bp[e_].reshape(4, 128).T)
              for e_ in range(NE)]

    sel_h = np.zeros((128, 4, 8), np.float32)
    for r in range(128):
        for p in range(4):
            sel_h[r, p, 2 * p + r // 64] = 1.0
    exps_h = np.zeros((8, 4, 128), np.float32)
    for p in range(4):
        for c in range(128):
            exps_h[2 * p + c // 64, p, c] = 1.0

    zero_bias = (np.all(bq == 0) and np.all(in_proj_b[E:] == 0)
                 and np.all(fus_b2 == 0)
                 and all(np.all(b == 0) for b in baff_e))

    base = {
        "whh": whh_h, "wcr": wcr_h, "bqkv": bqkv_h,
        "w1o": w1o_h, "beff": beff_h, "w2": w2_h, "wmu": wmu_h, "b2": b2_h,
        "sel": sel_h.astype(BF16NP), "exps": exps_h.astype(BF16NP),
        "ones512": np.full((128, 1), 1.0 / E, np.float32).astype(BF16NP),
        "onesk1": np.ones((1, 128), np.float32),
        "meanb2": np.full((1, 1), fus_b2.mean(), np.float32),
    }
    return base, waff_e, baff_e, zero_bias, perm


def _pack_slots(labels):
    """Assign samples to 64 slots of 256; return per-core chunk plans."""
    order_ids = [np.nonzero(labels == e_)[0] for e_ in range(NE)]
    counts = [len(x) for x in order_ids]
    assert sum(counts) == B
    slots = []          # list of (ids[256], eA, eB, nA)
    leftovers = []      # (expert, ids)
    for e_ in range(NE):
        ids = order_ids[e_]
        nfull = len(ids) // C
        for s in range(nfull):
            slots.append((ids[s * C:(s + 1) * C], e_, e_, C))
        if len(ids) % C:
            leftovers.append((e_, ids[nfull * C:]))
    # pack leftovers into mixed slots (each must span <= 2 experts)
    mixed = []
    stream = []
    for e_, ids in leftovers:
        stream.append((e_, list(ids)))
    while stream:
        eA, idsA = stream[0]
        if len(idsA) >= C:
            mixed.append((np.array(idsA[:C]), eA, eA, C))
            stream[0] = (eA, idsA[C:])
            if not stream[0][1]:
                stream.pop(0)
            continue
        if len(stream) == 1:
            assert len(idsA) == 0 or len(idsA) == C, \
                f"unpackable remainder {len(idsA)}"
            if idsA:
                mixed.append((np.array(idsA), eA, eA, C))
            stream.pop(0)
            continue
        eB, idsB = stream[1]
        take = C - len(idsA)
        assert len(idsB) >= take, (
            f"slot would span 3 experts: {len(idsA)} + {len(idsB)} < {C}")
        ids = np.concatenate([idsA, idsB[:take]])
        mixed.append((ids, eA, eB, len(idsA)))
        stream.pop(0)
        stream[1 - 1] = (eB, idsB[take:])
        if not stream[0][1]:
            stream.pop(0)
    assert len(slots) + len(mixed) == B // C
    assert len(mixed) <= NCORES, f"{len(mixed)} mixed slots > {NCORES} cores"

    # per core: 8 slots, mixed slot (if any) at position 7
    plans = []
    si = 0
    for c in range(NCORES):
        mine = []
        if c < len(mixed):
            n_pure = NCH - 1
        else:
            n_pure = NCH
        mine = slots[si:si + n_pure]
        si += n_pure
        if c < len(mixed):
            mine = mine + [mixed[c]]
        plans.append(mine)
    assert si == len(slots)
    return plans


def kernel(**inputs):
    img = np.asarray(inputs["image_embeddings"], np.float32)
    txt = np.asarray(inputs["text_embeddings"], np.float32)
    kno = np.asarray(inputs["knowledge_embeddings"], np.float32)
    labels = np.asarray(inputs["affective_labels"]).astype(np.int64).ravel()
    assert img.shape == (B, E)

    base, waff_e, baff_e, zero_bias, perm = _prep_weights(inputs)
    plans = _pack_slots(labels)

    xs = np.stack([img, txt, kno])                     # [3, B, 512]

    in_maps = []
    for c in range(NCORES):
        plan = plans[c]
        gi = np.concatenate([p[0] for p in plan])      # [2048]
        xg = xs[:, gi, :].transpose(0, 2, 1)           # [3, 512, R]
        xg = xg.reshape(3, 4, 128, NCH, C)             # [t, p, r, ch, c]
        x_hi = xg.astype(FP8NP)
        x_lo = (xg - x_hi.astype(np.float32)).astype(FP8NP)
        xhl_h = np.empty((NCH, 128, 3, 4, 2, C), FP8NP)
        xhl_h[:, :, :, :, 0, :] = x_lo.transpose(3, 2, 0, 1, 4)
        xhl_h[:, :, :, :, 1, :] = x_hi.transpose(3, 2, 0, 1, 4)

        waffA_h = np.stack([waff_e[p[1]] for p in plan])     # [NCH,128,4,512]
        baffA_h = np.stack([baff_e[p[1]] for p in plan])     # [NCH,128,4]
        eB = plan[NCH - 1][2]
        waffB_h = waff_e[eB]
        baffB_h = baff_e[eB]
        nA = plan[NCH - 1][3]
        mask = np.zeros((128, C), np.float32)
        mask[:, :nA] = 1.0

        m = dict(base)
        m["xhl"] = xhl_h
        m["waffA"] = waffA_h
        m["baffA"] = baffA_h
        m["waffB"] = waffB_h
        m["baffB"] = baffB_h
        m["maskex"] = mask.astype(BF16NP)
        in_maps.append(m)

    nc = _build_program(zero_bias)
    res = run_bass_kernel_spmd(nc, in_maps, core_ids=list(range(NCORES)))
    global LAST_RESULTS, LAST_NC
    LAST_RESULTS = res
    LAST_NC = nc

    out_full = np.zeros((B, E), np.float32)
    for c in range(NCORES):
        oT = res.results[c]["outT"]                    # [NCH,128,4,C] bf16
        oT = np.asarray(oT, dtype=np.float32)
        plan = plans[c]
        for ch in range(NCH):
            ids = plan[ch][0]
            # [128, 4, C] -> [C, 512] with feature f = 128*p + r
            blk = oT[ch].transpose(2, 1, 0).reshape(C, 4 * 128)
            out_full[ids] = blk
    return out_full


if __name__ == "__main__":
    rng = np.random.default_rng(0)
    fake = {
        "image_embeddings": rng.standard_normal((B, E)).astype(np.float32),
        "text_embeddings": rng.standard_normal((B, E)).astype(np.float32),
        "knowledge_embeddings": rng.standard_normal((B, E)).astype(np.float32),
        "affective_labels": rng.integers(0, NE, B),
        "in_proj_w": (rng.standard_normal((3 * E, E)) * 0.02).astype(np.float32),
        "in_proj_b": np.zeros(3 * E, np.float32),
        "out_proj_w": (rng.standard_normal((E, E)) * 0.02).astype(np.float32),
        "out_proj_b": np.zeros(E, np.float32),
        "fus_w1": (rng.standard_normal((H, 3 * E)) * 0.02).astype(np.float32),
        "fus_b1": np.zeros(H, np.float32),
        "fus_w2": (rng.standard_normal((E, H)) * 0.02).astype(np.float32),
        "fus_b2": np.zeros(E, np.float32),
        "ln_g": np.ones(E, np.float32),
        "ln_b": np.zeros(E, np.float32),
        "aff_w": (rng.standard_normal((NE, E, E)) * 0.02).astype(np.float32),
        "aff_b": np.zeros((NE, E), np.float32),
    }
    out = kernel(**fake)
    print("kernel ran, out:", out.shape, out.dtype, np.abs(out).max())


# revision 8
# speedup vs baseline: 1.0982x; 1.0015x over previous
"""Trainium2 Bass kernel v2 for nn_MultiModalFusion (moe_routing).

Design (driven by the TimelineSim cost model; 246.7us vs 385.3us baseline):
- Pure data parallel over 8 cores; host sorts samples by expert into 64
  slots of exactly 256 columns (zero padding waste). At most 3 slots mix
  two experts; those sit at chunk 7 of cores 0..2 where the program computes
  both experts and blends with a host-built column mask.
- QKV in fp8e4m3 DoubleRow matmuls (0.5 cyc/row) with host-side hi/lo error
  compensation: x = x_hi + x_lo, W = W_hi + W_lo, keeping W_hi*x_hi,
  W_hi*x_lo + W_lo*x_hi (2 hi-hi pairs + 4 cross pairs per (token, m-tile) =
  3C cycles vs 4C for fp32r). W is pre-scaled by 2^5 so its fp8 planes stay
  clear of e4m3's subnormal range (undone by the free Activation copy scale).
- All other matmul moving operands bf16 (1 cyc/row at any width); DVE
  element-wise work in 2x bf16 mode where possible.
- Attention via o_i = v0 + p_i1*(v1-v0) + p_i2*(v2-v0): only 2 of 3
  probability broadcasts per query token.
- LayerNorm: mean via a host-precomputed column-mean row folded into fus2;
  rstd = exp(-0.5*ln(var+eps)) on the scalar engine — ln/exp/identity/relu/
  square all live in one activation table set (set 6, preloaded manually),
  so there are zero LoadActFuncSet switches. gamma/beta are folded into the
  expert weights on the host.
- PSUM pair tiles (2x256 f32 = one 2KB bank) let one Activation instruction
  evacuate two matmul outputs.
- Software pipelining: each chunk's tail (W1o/fus2/LN/expert) is emitted one
  chunk late, phase-interleaved into the next chunk's attention middle; the
  next chunk's QKV is split into 18 atomic pieces pulled into every
  dependency stall window. v_j - v_0 runs early on the otherwise idle
  GPSIMD engine.
"""

import numpy as np
import ml_dtypes

import concourse.bass as bass
import concourse.mybir as mybir
import concourse.tile as tile
from concourse import bacc
from concourse.bass_utils import run_bass_kernel_spmd

E = 512
H = 256
NH = 8
HD = 64
NE = 4
B = 16384
NCORES = 8
C = 256              # chunk columns
NCH = 8              # chunks per core
R = NE * 512         # 2048 columns per core

LAST_RESULTS = None
LAST_NC = None

F32 = mybir.dt.float32
F32R = mybir.dt.float32r
BF16 = mybir.dt.bfloat16
FP8 = mybir.dt.float8e4
AF = mybir.ActivationFunctionType
ALU = mybir.AluOpType
MM = mybir.MatmulPerfMode

FP8NP = ml_dtypes.float8_e4m3
BF16NP = ml_dtypes.bfloat16

_NC_CACHE = {}


def _build_program(zero_bias, debug=False):
    key = (bool(zero_bias), bool(debug))
    if key in _NC_CACHE:
        return _NC_CACHE[key]
    nc = bacc.Bacc("TRN2")

    # ---------------- DRAM I/O ----------------
    # x hi/lo fp8: [ch, 128, t, p, plane(0=lo,1=hi), C]
    xhl = nc.dram_tensor("xhl", [NCH, 128, 3, 4, 2, C], FP8,
                         kind="ExternalInput")
    # QKV weights fp8: hi-hi k-pairs and (hi,lo) cross pairs
    whh = nc.dram_tensor("whh", [128, 2, 2, 1536], FP8, kind="ExternalInput")
    wcr = nc.dram_tensor("wcr", [128, 4, 2, 1536], FP8, kind="ExternalInput")
    bqkv = nc.dram_tensor("bqkv", [128, 12], F32, kind="ExternalInput")
    w1o = nc.dram_tensor("w1o", [128, 12, 256], BF16, kind="ExternalInput")
    beff = nc.dram_tensor("beff", [128, 2], F32, kind="ExternalInput")
    w2 = nc.dram_tensor("w2", [128, 2, 512], BF16, kind="ExternalInput")
    wmu = nc.dram_tensor("wmu", [128, 2], BF16, kind="ExternalInput")
    b2 = nc.dram_tensor("b2", [128, 4], F32, kind="ExternalInput")
    # per-chunk expert weights, host-selected: A for all chunks, B for ch 7
    waffA = nc.dram_tensor("waffA", [NCH, 128, 4, 512], BF16,
                           kind="ExternalInput")
    waffB = nc.dram_tensor("waffB", [128, 4, 512], BF16,
                           kind="ExternalInput")
    baffA = nc.dram_tensor("baffA", [NCH, 128, 4], F32,
                           kind="ExternalInput")
    baffB = nc.dram_tensor("baffB", [128, 4], F32, kind="ExternalInput")
    maskex = nc.dram_tensor("maskex", [128, C], BF16, kind="ExternalInput")
    meanb2 = nc.dram_tensor("meanb2", [1, 1], F32, kind="ExternalInput")
    sel = nc.dram_tensor("sel", [128, 4, 8], BF16, kind="ExternalInput")
    exps = nc.dram_tensor("exps", [8, 4, 128], BF16, kind="ExternalInput")
    ones512 = nc.dram_tensor("ones512", [128, 1], BF16, kind="ExternalInput")
    onesk1 = nc.dram_tensor("onesk1", [1, 128], F32R, kind="ExternalInput")
    outT = nc.dram_tensor("outT", [NCH, 128, 4, C], BF16,
                          kind="ExternalOutput")
    if debug:
        dbg_q = nc.dram_tensor("dbg_q", [128, 3, 4, C], F32,
                               kind="ExternalOutput")
        dbg_k = nc.dram_tensor("dbg_k", [128, 3, 4, C], F32,
                               kind="ExternalOutput")
        dbg_v = nc.dram_tensor("dbg_v", [128, 3, 4, C], F32,
                               kind="ExternalOutput")
        dbg_e = nc.dram_tensor("dbg_e", [8, 3, 3, C], F32,
                               kind="ExternalOutput")
        dbg_d = nc.dram_tensor("dbg_d", [8, 3, 2, C], F32,
                               kind="ExternalOutput")
        dbg_o = nc.dram_tensor("dbg_o", [128, 12, C], F32,
                               kind="ExternalOutput")
        dbg_h = nc.dram_tensor("dbg_h", [128, 2, C], F32,
                               kind="ExternalOutput")
        dbg_y = nc.dram_tensor("dbg_y", [128, 4, C], F32,
                               kind="ExternalOutput")
        dbg_mu = nc.dram_tensor("dbg_mu", [1, C], F32,
                                kind="ExternalOutput")
        dbg_var = nc.dram_tensor("dbg_var", [1, C], F32,
                                 kind="ExternalOutput")
        dbg_rstd = nc.dram_tensor("dbg_rstd", [1, C], F32,
                                  kind="ExternalOutput")
        dbg_fused = nc.dram_tensor("dbg_fused", [128, 4, C], F32,
                                   kind="ExternalOutput")

    with tile.TileContext(nc) as tc:
        with tc.tile_pool(name="wp", bufs=1) as wp, \
             tc.tile_pool(name="xp", bufs=2) as xp, \
             tc.tile_pool(name="qkvp", bufs=2) as qkvp, \
             tc.tile_pool(name="ap", bufs=2) as ap, \
             tc.tile_pool(name="spM", bufs=2) as spM, \
             tc.tile_pool(name="spZ", bufs=2) as spZ, \
             tc.tile_pool(name="spS", bufs=2) as spS, \
             tc.tile_pool(name="mrexp", bufs=2) as mrexp, \
             tc.tile_pool(name="sp2", bufs=4) as sp2, \
             tc.tile_pool(name="sp3", bufs=2) as sp3, \
             tc.tile_pool(name="psQ", bufs=3, space="PSUM") as psQ, \
             tc.tile_pool(name="psB", bufs=2, space="PSUM") as psB, \
             tc.tile_pool(name="psT", bufs=2, space="PSUM") as psT, \
             tc.tile_pool(name="psst", bufs=1, space="PSUM") as psst:

            # ---------------- persistent weights ----------------
            nc.scalar.add_instruction(mybir.InstLoadActFuncSet(
                name=nc.get_next_instruction_name(), ins=[], outs=[],
                act_func_set_id=6))
            whh_sb = wp.tile([128, 2, 2, 1536], FP8)
            wcr_sb = wp.tile([128, 4, 2, 1536], FP8)
            # first halves only — QKV m-tiles 0..5 can start ASAP
            nc.sync.dma_start(whh_sb[:, :, :, 0:768], whh[:, :, :, 0:768])
            nc.sync.dma_start(wcr_sb[:, :, :, 0:768], wcr[:, :, :, 0:768])
            bqkv_sb = wp.tile([128, 12], F32)
            sel_sb = wp.tile([128, 4, 8], BF16)
            exps_sb = wp.tile([8, 4, 128], BF16)
            eps_sb = wp.tile([1, 1], F32)

            def load_more_front():
                nc.sync.dma_start(whh_sb[:, :, :, 768:], whh[:, :, :, 768:])
                nc.sync.dma_start(wcr_sb[:, :, :, 768:], wcr[:, :, :, 768:])
                nc.sync.dma_start(bqkv_sb[:], bqkv[:])
                nc.sync.dma_start(sel_sb[:], sel[:])
                nc.sync.dma_start(exps_sb[:], exps[:])
                nc.vector.memset(eps_sb[:], 1e-5)
            w1o_sb = wp.tile([128, 12, 256], BF16)
            beff_sb = wp.tile([128, 2], F32)
            w2_sb = wp.tile([128, 2, 512], BF16)
            wmu_sb = wp.tile([128, 2], BF16)
            b2_sb = wp.tile([128, 4], F32)
            o512_sb = wp.tile([128, 1], BF16)
            ok1_sb = wp.tile([1, 128], F32R)
            waffB_sb = wp.tile([128, 4, 512], BF16)
            baffB_sb = wp.tile([128, 4], F32)
            maskex_sb = wp.tile([128, C], BF16)
            meanb2_sb = wp.tile([1, 1], F32)

            def load_tail_weights():
                nc.sync.dma_start(w1o_sb[:], w1o[:])
                nc.sync.dma_start(beff_sb[:], beff[:])
                nc.sync.dma_start(w2_sb[:], w2[:])
                nc.sync.dma_start(wmu_sb[:], wmu[:])
                nc.sync.dma_start(b2_sb[:], b2[:])
                nc.sync.dma_start(o512_sb[:], ones512[:])
                nc.sync.dma_start(ok1_sb[:], onesk1[:])
                nc.sync.dma_start(waffB_sb[:], waffB[:])
                nc.sync.dma_start(baffB_sb[:], baffB[:])
                nc.sync.dma_start(maskex_sb[:], maskex[:])
                nc.sync.dma_start(meanb2_sb[:], meanb2[:])

            def front_alloc(ch):
                """x + expert-weight loads, qkv tile allocation, chunk ch."""
                x_sb = xp.tile([128, 3, 4, 2, C], FP8, tag="x", name=f"x{ch}")
                for t in range(3):
                    nc.sync.dma_start(x_sb[:, t], xhl[ch, :, t])
                waff_sb = ap.tile([128, 4, 512], BF16, tag="waff",
                                  name=f"waff{ch}")
                nc.sync.dma_start(waff_sb[:], waffA[ch])
                baff_sb = ap.tile([128, 4], F32, tag="baff", name=f"baff{ch}")
                nc.sync.dma_start(baff_sb[:], baffA[ch])
                q_sb = qkvp.tile([128, 3, 4, C], BF16, tag="q", name=f"q{ch}")
                k_sb = qkvp.tile([128, 3, 4, C], BF16, tag="k", name=f"k{ch}")
                v_sb = qkvp.tile([128, 3, 4, C], BF16, tag="v", name=f"v{ch}")
                return {"ch": ch, "x": x_sb, "q": q_sb, "k": k_sb, "v": v_sb,
                        "waff": waff_sb, "baff": baff_sb}

            def front_qkv_mp(st, t, mp):
                """QKV m-pair mp (12 DR matmuls + 1 pair copy) — atomic."""
                ch = st["ch"]
                x_sb = st["x"]
                pair = psQ.tile([128, 2, C], F32, tag="qkv",
                                name=f"qkv{ch}_{t}_{mp}")
                for half in range(2):
                    m = 2 * mp + half
                    mc = slice(m * 128, (m + 1) * 128)
                    for kp in range(2):
                        nc.tensor.matmul(
                            pair[:, half, :],
                            whh_sb[:, kp, :, mc],
                            x_sb[:, t, 2 * kp:2 * kp + 2, 1, :],
                            start=(kp == 0), stop=False,
                            perf_mode=MM.DoubleRow)
                    for p in range(4):
                        nc.tensor.matmul(
                            pair[:, half, :],
                            wcr_sb[:, p, :, mc],
                            x_sb[:, t, p, :, :],
                            start=False, stop=(p == 3),
                            perf_mode=MM.DoubleRow)
                dst = (st["q"], st["k"], st["v"])[mp // 2]
                pp = mp % 2
                if zero_bias:
                    nc.scalar.activation(
                        dst[:, t, 2 * pp:2 * pp + 2, :], pair[:],
                        AF.Identity, scale=1.0 / 32.0)
                else:
                    for half in range(2):
                        m = 2 * mp + half
                        nc.scalar.activation(
                            dst[:, t, 2 * pp + half, :], pair[:, half, :],
                            AF.Identity, bias=bqkv_sb[:, m:m + 1],
                            scale=1.0 / 32.0)

            def middle(ch, st, pieces, tailph):
                """Scores/softmax/o for chunk ch; interleaves previous
                chunk's tail phases (tailph) and next-chunk QKV pieces."""
                def pull(n=1):
                    for _ in range(n):
                        if pieces:
                            pieces.pop(0)()
                q_sb, k_sb, v_sb = st["q"], st["k"], st["v"]

                # w_j = v_j - v_0 early on the idle Pool engine
                wv_sb = spM.tile([128, 2, 4, C], BF16, tag="wv",
                                 name=f"wv{ch}")
                for j in (1, 2):
                    nc.gpsimd.tensor_tensor(wv_sb[:, j - 1, :, :],
                                            v_sb[:, j, :, :],
                                            v_sb[:, 0, :, :],
                                            ALU.subtract)

                # scores + exp; tail W1o/fus2 phases fill the DVE-bound spots
                e_sb = spM.tile([8, 3, 3, C], BF16, tag="esb", name=f"e{ch}")
                for i in range(3):
                    for j in range(3):
                        prod = sp2.tile([128, 4, C], BF16, tag="prod",
                                        name=f"prod{ch}_{i}_{j}")
                        nc.vector.tensor_tensor(
                            prod[:], q_sb[:, i, :, :], k_sb[:, j, :, :],
                            ALU.mult)
                        s_ps = psB.tile([8, C], F32, tag="bc",
                                        name=f"s{ch}_{i}_{j}")
                        for p in range(4):
                            nc.tensor.matmul(
                                s_ps[:], sel_sb[:, p, :], prod[:, p, :],
                                start=(p == 0), stop=(p == 3))
                        nc.scalar.activation(e_sb[:, i, j, :], s_ps[:],
                                             AF.Exp)
                    if tailph:
                        tailph.pop(0)()
                    pull(1)

                # softmax -> p1, p2 ; previous tail's stats/LN fill PE here
                z_sb = spZ.tile([8, 3, C], F32, tag="z", name=f"z{ch}")
                nc.vector.tensor_tensor(z_sb[:], e_sb[:, :, 0, :],
                                        e_sb[:, :, 1, :], ALU.add)
                nc.vector.tensor_tensor(z_sb[:], z_sb[:], e_sb[:, :, 2, :],
                                        ALU.add)
                nc.vector.reciprocal_approx_fast(z_sb[:], z_sb[:])
                rz_sb = z_sb
                d_sb = spZ.tile([8, 3, 2, C], BF16, tag="dsb", name=f"d{ch}")
                for j in (1, 2):
                    nc.vector.tensor_tensor(d_sb[:, :, j - 1, :],
                                            e_sb[:, :, j, :], rz_sb[:],
                                            ALU.mult)
                pull(2)

                # o_i = v0 + p_i1*w1 + p_i2*w2 ; previous expert fills PE
                o_sb = ap.tile([128, 12, C], BF16, tag="o", name=f"o{ch}")
                for i in range(3):
                    pv = sp2.tile([128, 4, 2, C], BF16, tag="pv",
                                  name=f"pv{ch}_{i}")
                    for p in range(4):
                        bc = psB.tile([128, 2, C], F32, tag="bc",
                                      name=f"bc{ch}_{i}_{p}")
                        for j in (1, 2):
                            nc.tensor.matmul(
                                bc[:, j - 1, :], exps_sb[:, p, :],
                                d_sb[:, i, j - 1, :], start=True, stop=True)
                        nc.vector.tensor_tensor(pv[:, p, :, :], bc[:],
                                                wv_sb[:, :, p, :], ALU.mult)
                        pull(2)
                    if tailph:
                        tailph.pop(0)()
                    nc.vector.tensor_tensor(pv[:, :, 0, :], pv[:, :, 0, :],
                                            pv[:, :, 1, :], ALU.add)
                    nc.vector.tensor_tensor(o_sb[:, i * 4:(i + 1) * 4, :],
                                            pv[:, :, 0, :],
                                            v_sb[:, 0, :, :], ALU.add)
                    if tailph:
                        tailph.pop(0)()
                return o_sb

            def make_tail(ch, o_sb):
                """Return tail phase closures for chunk ch (uses o_sb)."""
                waff_sb = ap.tile([128, 4, 512], BF16, tag="waff",
                                  name=f"waff{ch}")
                nc.sync.dma_start(waff_sb[:], waffA[ch])
                baff_sb = ap.tile([128, 4], F32, tag="baff", name=f"baff{ch}")
                nc.sync.dma_start(baff_sb[:], baffA[ch])
                state = {}

                def ph_w1o():
                    hp = psT.tile([128, 2, C], F32, tag="tail",
                                  name=f"hp{ch}")
                    for m2t in range(2):
                        for kip in range(12):
                            nc.tensor.matmul(
                                hp[:, m2t, :],
                                w1o_sb[:, kip, m2t * 128:(m2t + 1) * 128],
                                o_sb[:, kip, :],
                                start=(kip == 0), stop=(kip == 11))
                    hpre = ap.tile([128, 2, C], BF16, tag="hpre",
                                   name=f"hpre{ch}")
                    for m2t in range(2):
                        nc.scalar.activation(hpre[:, m2t, :], hp[:, m2t, :],
                                             AF.Relu,
                                             bias=beff_sb[:, m2t:m2t + 1])
                    state["hpre"] = hpre

                def ph_fus2():
                    hpre = state["hpre"]
                    st_ps = psst.tile([1, 2, C], F32, tag="st",
                                      name=f"st{ch}")
                    for ks in range(2):
                        nc.tensor.matmul(st_ps[:, 0, :], wmu_sb[:, ks:ks + 1],
                                         hpre[:, ks, :],
                                         start=(ks == 0), stop=(ks == 1))
                    y_sb = ap.tile([128, 4, C], BF16, tag="y", name=f"y{ch}")
                    ysq = sp3.tile([128, 4, C], BF16, tag="ysq",
                                   name=f"ysq{ch}")
                    for yp2 in range(2):
                        yp = psT.tile([128, 2, C], F32, tag="tail",
                                      name=f"yp{ch}_{yp2}")
                        for half in range(2):
                            m4 = 2 * yp2 + half
                            for ks in range(2):
                                nc.tensor.matmul(
                                    yp[:, half, :],
                                    w2_sb[:, ks, m4 * 128:(m4 + 1) * 128],
                                    hpre[:, ks, :], start=(ks == 0),
                                    stop=(ks == 1))
                        if zero_bias:
                            nc.scalar.activation(
                                y_sb[:, 2 * yp2:2 * yp2 + 2, :], yp[:],
                                AF.Identity)
                            nc.scalar.activation(
                                ysq[:, 2 * yp2:2 * yp2 + 2, :], yp[:],
                                AF.Square)
                        else:
                            for half in range(2):
                                m4 = 2 * yp2 + half
                                nc.scalar.activation(
                                    y_sb[:, m4, :], yp[:, half, :],
                                    AF.Identity, bias=b2_sb[:, m4:m4 + 1])
                                nc.scalar.activation(
                                    ysq[:, m4, :], yp[:, half, :], AF.Square,
                                    bias=b2_sb[:, m4:m4 + 1])
                    state["st_ps"] = st_ps
                    state["y"] = y_sb
                    state["ysq"] = ysq

                def ph_stats():
                    st_ps, y_sb, ysq = (state["st_ps"], state["y"],
                                        state["ysq"])
                    for p in range(4):
                        nc.tensor.matmul(st_ps[:, 1, :], o512_sb[:],
                                         ysq[:, p, :],
                                         start=(p == 0), stop=(p == 3))
                    mu_sb = spS.tile([1, C], F32, tag="musb",
                                     name=f"musb{ch}")
                    if zero_bias:
                        nc.vector.tensor_scalar_add(mu_sb[:], st_ps[:, 0, :],
                                                    0.0)
                    else:
                        nc.vector.tensor_scalar_add(mu_sb[:], st_ps[:, 0, :],
                                                    meanb2_sb[:])
                    musq = spS.tile([1, C], F32, tag="musq", name=f"musq{ch}")
                    nc.gpsimd.tensor_tensor(musq[:], mu_sb[:], mu_sb[:],
                                            ALU.mult)
                    var_sb = spS.tile([1, C], F32, tag="varsb",
                                      name=f"var{ch}")
                    nc.vector.tensor_tensor(var_sb[:], st_ps[:, 1, :],
                                            musq[:], ALU.subtract)
                    lnv = spS.tile([1, C], F32, tag="lnv", name=f"lnv{ch}")
                    nc.scalar.activation(lnv[:], var_sb[:], AF.Ln,
                                         bias=eps_sb[:])
                    rstd_sb = spS.tile([1, C], F32R, tag="rstd",
                                       name=f"rstd{ch}")
                    nc.scalar.activation(rstd_sb[:], lnv[:], AF.Exp,
                                         scale=-0.5)
                    murs = spS.tile([1, C], F32R, tag="murs",
                                    name=f"murs{ch}")
                    nc.vector.tensor_tensor(murs[:], mu_sb[:], rstd_sb[:],
                                            ALU.mult)
                    bcp = psT.tile([128, 2, C], F32, tag="tail",
                                   name=f"bcp{ch}")
                    nc.tensor.matmul(bcp[:, 0, :], ok1_sb[:], murs[:],
                                     start=True, stop=True)
                    nc.tensor.matmul(bcp[:, 1, :], ok1_sb[:], rstd_sb[:],
                                     start=True, stop=True)
                    mrex = mrexp.tile([128, 2, C], BF16, tag="mrex",
                                      name=f"mrex{ch}")
                    nc.scalar.activation(mrex[:], bcp[:], AF.Identity)
                    fused = ap.tile([128, 4, C], BF16, tag="fused",
                                    name=f"fused{ch}")
                    nc.vector.tensor_tensor(
                        fused[:], y_sb[:],
                        mrex[:, 1, None, :].to_broadcast((128, 4, C)),
                        ALU.mult)
                    nc.vector.tensor_tensor(
                        fused[:], fused[:],
                        mrex[:, 0, None, :].to_broadcast((128, 4, C)),
                        ALU.subtract)
                    state["fused"] = fused

                def ph_expert():
                    fused = state["fused"]
                    ot = sp3.tile([128, 4, C], BF16, tag="ot", name=f"ot{ch}")
                    is_mixed = (ch == NCH - 1)
                    for op2 in range(2):
                        op = psT.tile([128, 2, C], F32, tag="tail",
                                      name=f"op{ch}_{op2}")
                        for half in range(2):
                            m4 = 2 * op2 + half
                            for ks in range(4):
                                nc.tensor.matmul(
                                    op[:, half, :],
                                    waff_sb[:, ks, m4 * 128:(m4 + 1) * 128],
                                    fused[:, ks, :], start=(ks == 0),
                                    stop=(ks == 3))
                        if not is_mixed:
                            if zero_bias:
                                nc.scalar.activation(
                                    ot[:, 2 * op2:2 * op2 + 2, :], op[:],
                                    AF.Identity)
                            else:
                                for half in range(2):
                                    m4 = 2 * op2 + half
                                    nc.scalar.activation(
                                        ot[:, m4, :], op[:, half, :],
                                        AF.Identity,
                                        bias=baff_sb[:, m4:m4 + 1])
                        else:
                            opB = psB.tile([128, 2, C], F32, tag="bc",
                                           name=f"opB{ch}_{op2}")
                            for half in range(2):
                                m4 = 2 * op2 + half
                                for ks in range(4):
                                    nc.tensor.matmul(
                                        opB[:, half, :],
                                        waffB_sb[:, ks,
                                                 m4 * 128:(m4 + 1) * 128],
                                        fused[:, ks, :], start=(ks == 0),
                                        stop=(ks == 3))
                            oA = sp3.tile([128, 2, C], BF16, tag="oA",
                                          name=f"oA{ch}_{op2}")
                            oB = sp3.tile([128, 2, C], BF16, tag="oB",
                                          name=f"oB{ch}_{op2}")
                            if zero_bias:
                                nc.scalar.activation(oA[:], op[:],
                                                     AF.Identity)
                                nc.scalar.activation(oB[:], opB[:],
                                                     AF.Identity)
                            else:
                                for half in range(2):
                                    m4 = 2 * op2 + half
                                    nc.scalar.activation(
                                        oA[:, half, :], op[:, half, :],
                                        AF.Identity,
                                        bias=baff_sb[:, m4:m4 + 1])
                                    nc.scalar.activation(
                                        oB[:, half, :], opB[:, half, :],
                                        AF.Identity,
                                        bias=baffB_sb[:, m4:m4 + 1])
                            dAB = sp3.tile([128, 2, C], BF16, tag="dAB",
                                           name=f"dAB{ch}_{op2}")
                            nc.vector.tensor_tensor(dAB[:], oA[:], oB[:],
                                                    ALU.subtract)
                            nc.vector.tensor_tensor(
                                dAB[:], dAB[:],
                                maskex_sb[:, None, :].to_broadcast(
                                    (128, 2, C)), ALU.mult)
                            nc.vector.tensor_tensor(
                                ot[:, 2 * op2:2 * op2 + 2, :], oB[:], dAB[:],
                                ALU.add)
                    nc.sync.dma_start(outT[ch], ot[:])

                return [lambda: None, lambda: None, ph_w1o, ph_fus2,
                        ph_stats, ph_expert]

            cur = front_alloc(0)
            first = True
            for t in range(3):
                for mp in range(6):
                    front_qkv_mp(cur, t, mp)
                    if first:
                        load_more_front()
                        first = False
                if t == 0:
                    load_tail_weights()
            prev_tail = []
            for ch in range(NCH):
                if ch + 1 < NCH:
                    nxt = front_alloc(ch + 1)
                    pieces = [lambda t=t, mp=mp, s=nxt: front_qkv_mp(s, t, mp)
                              for t in range(3) for mp in range(6)]
                else:
                    nxt, pieces = None, []
                o_sb = middle(ch, cur, pieces, prev_tail)
                for ph in prev_tail:
                    ph()
                prev_tail = make_tail(ch, o_sb)
                for p_ in pieces:
                    p_()
                cur = nxt
            # drain: last chunk's tail
            for ph in prev_tail:
                ph()

    nc.finalize()
    _NC_CACHE[key] = nc
    return nc


def _fp8_split(a):
    """Return (lo, hi) fp8e4m3 arrays with hi + lo ~= a."""
    hi = a.astype(FP8NP)
    lo = (a - hi.astype(np.float32)).astype(FP8NP)
    return lo, hi


def _prep_weights(inputs):
    in_proj_w = np.asarray(inputs["in_proj_w"], np.float32)
    in_proj_b = np.asarray(inputs["in_proj_b"], np.float32)
    out_proj_w = np.asarray(inputs["out_proj_w"], np.float32)
    out_proj_b = np.asarray(inputs["out_proj_b"], np.float32)
    fus_w1 = np.asarray(inputs["fus_w1"], np.float32)
    fus_b1 = np.asarray(inputs["fus_b1"], np.float32)
    fus_w2 = np.asarray(inputs["fus_w2"], np.float32)
    fus_b2 = np.asarray(inputs["fus_b2"], np.float32)
    ln_g = np.asarray(inputs["ln_g"], np.float32)
    ln_b = np.asarray(inputs["ln_b"], np.float32)
    aff_w = np.asarray(inputs["aff_w"], np.float32)
    aff_b = np.asarray(inputs["aff_b"], np.float32)

    scale = 1.0 / np.sqrt(np.float32(HD))
    W = in_proj_w.copy()
    W[:E] *= scale
    bq = in_proj_b.copy()
    bq[:E] *= scale
    # pre-scale W by 2^5 so the fp8 hi/lo planes stay out of e4m3's
    # subnormal range (W sigma ~0.02); undone by the Act copy scale 2^-5
    W *= 32.0

    # W.T is [512(k), 1536(m)]
    WT = np.ascontiguousarray(W.T)
    WT_lo, WT_hi = _fp8_split(WT)
    WT_lo = WT_lo.reshape(4, 128, 1536)
    WT_hi = WT_hi.reshape(4, 128, 1536)
    # hi-hi pairs: [128, kp, e, 1536] with (kp,e) -> k-subtile 2*kp+e
    whh_h = np.empty((128, 2, 2, 1536), FP8NP)
    for kp in range(2):
        for e_ in range(2):
            whh_h[:, kp, e_, :] = WT_hi[2 * kp + e_]
    # cross pairs: [128, p, {hi,lo}, 1536], paired with x (lo, hi)
    wcr_h = np.empty((128, 4, 2, 1536), FP8NP)
    for p in range(4):
        wcr_h[:, p, 0, :] = WT_hi[p]
        wcr_h[:, p, 1, :] = WT_lo[p]
    bqkv_h = np.ascontiguousarray(bq.reshape(12, 128).T)

    # fold out_proj into fus_w1; permute (h,d) -> (p, hl, d) to match v layout
    perm = np.empty(E, np.int64)
    for h in range(NH):
        for d in range(HD):
            perm[(h // 2) * 128 + (h % 2) * 64 + d] = h * HD + d
    blocks = []
    for i in range(3):
        blk = fus_w1[:, i * E:(i + 1) * E] @ out_proj_w  # [256, 512]
        blocks.append(blk[:, perm])
    W1o = np.concatenate(blocks, axis=1)  # [256, 1536]
    w1o_h = np.ascontiguousarray(
        W1o.T.reshape(12, 128, 256).transpose(1, 0, 2)).astype(BF16NP)
    # v bias folds into beff: o_i includes +bv for each i
    beff = fus_b1 + fus_w1 @ np.tile(out_proj_b, 3)
    beff_h = np.ascontiguousarray(beff.reshape(2, 128).T)

    w2_h = np.ascontiguousarray(
        fus_w2.T.reshape(2, 128, 512).transpose(1, 0, 2)).astype(BF16NP)
    wmu_h = np.ascontiguousarray(
        (fus_w2.mean(axis=0)).reshape(2, 128).T).astype(BF16NP)
    b2_h = np.ascontiguousarray(fus_b2.reshape(4, 128).T)

    # gamma/beta folded into expert weights/biases
    Wp = aff_w * ln_g[None, None, :]                   # [NE, 512, 512]
    bp = aff_w @ ln_b + aff_b                          # [NE, 512]
    waff_e = []
    for e_ in range(NE):
        A = np.ascontiguousarray(
            Wp[e_].T.reshape(4, 128, 512).transpose(1, 0, 2))
        waff_e.append(A.astype(BF16NP))
    baff_e = [np.ascontiguousarray(bp[e_].reshape(4, 128).T)
              for e_ in range(NE)]

    sel_h = np.zeros((128, 4, 8), np.float32)
    for r in range(128):
        for p in range(4):
            sel_h[r, p, 2 * p + r // 64] = 1.0
    exps_h = np.zeros((8, 4, 128), np.float32)
    for p in range(4):
        for c in range(128):
            exps_h[2 * p + c // 64, p, c] = 1.0

    zero_bias = (np.all(bq == 0) and np.all(in_proj_b[E:] == 0)
                 and np.all(fus_b2 == 0)
                 and all(np.all(b == 0) for b in baff_e))

    base = {
        "whh": whh_h, "wcr": wcr_h, "bqkv": bqkv_h,
        "w1o": w1o_h, "beff": beff_h, "w2": w2_h, "wmu": wmu_h, "b2": b2_h,
        "sel": sel_h.astype(BF16NP), "exps": exps_h.astype(BF16NP),
        "ones512": np.full((128, 1), 1.0 / E, np.float32).astype(BF16NP),
        "onesk1": np.ones((1, 128), np.float32),
        "meanb2": np.full((1, 1), fus_b2.mean(), np.float32),
    }
    return base, waff_e, baff_e, zero_bias, perm


def _pack_slots(labels):
    """Assign samples to 64 slots of 256; return per-core chunk plans."""
    order_ids = [np.nonzero(labels == e_)[0] for e_ in range(NE)]
    counts = [len(x) for x in order_ids]
    assert sum(counts) == B
    slots = []          # list of (ids[256], eA, eB, nA)
    leftovers = []      # (expert, ids)
    for e_ in range(NE):
        ids = order_ids[e_]
        nfull = len(ids) // C
        for s in range(nfull):
            slots.append((ids[s * C:(s + 1) * C], e_, e_, C))
        if len(ids) % C:
            leftovers.append((e_, ids[nfull * C:]))
    # pack leftovers into mixed slots (each must span <= 2 experts)
    mixed = []
    stream = []
    for e_, ids in leftovers:
        stream.append((e_, list(ids)))
    while stream:
        eA, idsA = stream[0]
        if len(idsA) >= C:
            mixed.append((np.array(idsA[:C]), eA, eA, C))
            stream[0] = (eA, idsA[C:])
            if not stream[0][1]:
                stream.pop(0)
            continue
        if len(stream) == 1:
            assert len(idsA) == 0 or len(idsA) == C, \
                f"unpackable remainder {len(idsA)}"
            if idsA:
                mixed.append((np.array(idsA), eA, eA, C))
            stream.pop(0)
            continue
        eB, idsB = stream[1]
        take = C - len(idsA)
        assert len(idsB) >= take, (
            f"slot would span 3 experts: {len(idsA)} + {len(idsB)} < {C}")
        ids = np.concatenate([idsA, idsB[:take]])
        mixed.append((ids, eA, eB, len(idsA)))
        stream.pop(0)
        stream[1 - 1] = (eB, idsB[take:])
        if not stream[0][1]:
            stream.pop(0)
    assert len(slots) + len(mixed) == B // C
    assert len(mixed) <= NCORES, f"{len(mixed)} mixed slots > {NCORES} cores"

    # per core: 8 slots, mixed slot (if any) at position 7
    plans = []
    si = 0
    for c in range(NCORES):
        mine = []
        if c < len(mixed):
            n_pure = NCH - 1
        else:
            n_pure = NCH
        mine = slots[si:si + n_pure]
        si += n_pure
        if c < len(mixed):
            mine = mine + [mixed[c]]
        plans.append(mine)
    assert si == len(slots)
    return plans


def kernel(**inputs):
    img = np.asarray(inputs["image_embeddings"], np.float32)
    txt = np.asarray(inputs["text_embeddings"], np.float32)
    kno = np.asarray(inputs["knowledge_embeddings"], np.float32)
    labels = np.asarray(inputs["affective_labels"]).astype(np.int64).ravel()
    assert img.shape == (B, E)

    base, waff_e, baff_e, zero_bias, perm = _prep_weights(inputs)
    plans = _pack_slots(labels)

    xs = np.stack([img, txt, kno])                     # [3, B, 512]

    in_maps = []
    for c in range(NCORES):
        plan = plans[c]
        gi = np.concatenate([p[0] for p in plan])      # [2048]
        xg = xs[:, gi, :].transpose(0, 2, 1)           # [3, 512, R]
        xg = xg.reshape(3, 4, 128, NCH, C)             # [t, p, r, ch, c]
        x_hi = xg.astype(FP8NP)
        x_lo = (xg - x_hi.astype(np.float32)).astype(FP8NP)
        xhl_h = np.empty((NCH, 128, 3, 4, 2, C), FP8NP)
        xhl_h[:, :, :, :, 0, :] = x_lo.transpose(3, 2, 0, 1, 4)
        xhl_h[:, :, :, :, 1, :] = x_hi.transpose(3, 2, 0, 1, 4)

        waffA_h = np.stack([waff_e[p[1]] for p in plan])     # [NCH,128,4,512]
        baffA_h = np.stack([baff_e[p[1]] for p in plan])     # [NCH,128,4]
        eB = plan[NCH - 1][2]
        waffB_h = waff_e[eB]
        baffB_h = baff_e[eB]
        nA = plan[NCH - 1][3]
        mask = np.zeros((128, C), np.float32)
        mask[:, :nA] = 1.0

        m = dict(base)
        m["xhl"] = xhl_h
        m["waffA"] = waffA_h
        m["baffA"] = baffA_h
        m["waffB"] = waffB_h
        m["baffB"] = baffB_h
        m["maskex"] = mask.astype(BF16NP)
        in_maps.append(m)

    nc = _build_program(zero_bias)
    res = run_bass_kernel_spmd(nc, in_maps, core_ids=list(range(NCORES)))
    global LAST_RESULTS, LAST_NC
    LAST_RESULTS = res
    LAST_NC = nc

    out_full = np.zeros((B, E), np.float32)
    for c in range(NCORES):
        oT = res.results[c]["outT"]                    # [NCH,128,4,C] bf16
        oT = np.asarray(oT, dtype=np.float32)
        plan = plans[c]
        for ch in range(NCH):
            ids = plan[ch][0]
            # [128, 4, C] -> [C, 512] with feature f = 128*p + r
            blk = oT[ch].transpose(2, 1, 0).reshape(C, 4 * 128)
            out_full[ids] = blk
    return out_full


if __name__ == "__main__":
    rng = np.random.default_rng(0)
    fake = {
        "image_embeddings": rng.standard_normal((B, E)).astype(np.float32),
        "text_embeddings": rng.standard_normal((B, E)).astype(np.float32),
        "knowledge_embeddings": rng.standard_normal((B, E)).astype(np.float32),
        "affective_labels": rng.integers(0, NE, B),
        "in_proj_w": (rng.standard_normal((3 * E, E)) * 0.02).astype(np.float32),
        "in_proj_b": np.zeros(3 * E, np.float32),
        "out_proj_w": (rng.standard_normal((E, E)) * 0.02).astype(np.float32),
        "out_proj_b": np.zeros(E, np.float32),
        "fus_w1": (rng.standard_normal((H, 3 * E)) * 0.02).astype(np.float32),
        "fus_b1": np.zeros(H, np.float32),
        "fus_w2": (rng.standard_normal((E, H)) * 0.02).astype(np.float32),
        "fus_b2": np.zeros(E, np.float32),
        "ln_g": np.ones(E, np.float32),
        "ln_b": np.zeros(E, np.float32),
        "aff_w": (rng.standard_normal((NE, E, E)) * 0.02).astype(np.float32),
        "aff_b": np.zeros((NE, E), np.float32),
    }
    out = kernel(**fake)
    print("kernel ran, out:", out.shape, out.dtype, np.abs(out).max())
